# revision 48
# baseline (speedup 1.0000x reference)
"""Trainium2 Bass kernel for a SAM/ViTDet-style windowed-attention transformer
block (DIM=768, 12 heads, window 14, decomposed rel-pos bias, exact-gelu MLP).

Contract: kernel(**inputs) takes the FULL unsharded inputs from
reference.setup_inputs() and returns the FULL (2, 64, 64, 768) float32 output.

Strategy (8 NeuronCores, SPMD, data-parallel):
  Dispatch A (attention): shard the 50 real windows (padded to 56) as 7
    windows/core. Per core: LN1 -> qkv -> windowed attention with the
    decomposed rel-pos bias folded into an augmented-key matmul -> proj.
  Host: window-unpartition, crop, residual add.
  Dispatch B (MLP): shard the 8192 tokens as 1024/core. Per core:
    LN2 -> fc1 -> exact GELU -> fc2 -> residual.

Perf notes (v2):
  * The LN affine (w, b) is absorbed host-side into the following matmul
    weights/biases, so on-device LN is a pure standardize: bf16 stats
    matmuls + bf16 broadcast tiles + two 4x-rate DVE tensor_tensor ops.
  * Rel-pos rows are produced by per-(h or w) batched matmuls (112 instead
    of 392) whose PSUM outputs land at partition bases 0/32/64/96, cutting
    eviction traffic.
  * The two score matmuls of one (window, head) share a [128, 392] PSUM
    tile -> a single exp instruction per head.
  * AV outputs are 6-head-batched in PSUM; the softmax normalization is one
    DVE multiply with a stride-0 (broadcast) reciprocal operand.
  * Weight DMAs are chunked and issued up front so compute starts ~10us in.

Augmented-key rel-pos layout (q/k head-pair blocks, per j-chunk):
  even head: q 0:64,  rel_h 64:78,  zeros 78:96, rel_w 96:110, zeros 110:128
  odd head:  rel_h 0:14, zeros 14:32, rel_w 32:46, zeros 46:64, q 64:128
khat holds k values in the q rows and one-hot key-position masks in the rel
rows; zero gaps make the extra contraction rows inert. S^T = khat^T qhat then
includes the decomposed bias exactly.
"""

import sys

sys.path.insert(0, "/opt/trn_rl_repo")

from contextlib import ExitStack

import numpy as np
import ml_dtypes

import concourse.bacc as bacc
import concourse.mybir as mybir
import concourse.tile as tile
from concourse.bass_utils import run_bass_kernel_spmd
from concourse.masks import make_identity

dt = mybir.dt
AF = mybir.ActivationFunctionType
ALU = mybir.AluOpType

DIM = 768
HEADS = 12
HD = 64
WS = 14
N = WS * WS          # 196 tokens / window
NW = 7               # windows per core
T = NW * N           # 1372 token slots per core (dispatch A)
TB = 1024            # tokens per core (dispatch B)
MLP = 3072
NCORES = 8
JC = DIM // 128      # 6 feature chunks
HC = MLP // 128      # 24 hidden chunks
EPS = 1e-5
SCALE = HD ** -0.5   # 0.125
TPAD = 1376          # T rounded up so fp8 DoubleRow pair strides are 16B-aligned
NPAD = 208           # N rounded up likewise (aT)
BF16 = ml_dtypes.bfloat16
F8 = ml_dtypes.float8_e4m3
W8 = 64.0   # fp8 weight pre-scale (avoids e4m3 subnormals); undone at eviction

# window-aligned token chunks for dispatch A (2+2+2+1 windows)
_NSL = [(0, 392), (392, 784), (784, 1176), (1176, 1372)]
_NSL_LN = [(i * N, (i + 1) * N) for i in range(NW)]        # LN chunks, A
_NSL_B = [(0, 256), (256, 512), (512, 768), (768, 1024)]   # LN chunks, B
_FC_B = [(0, 512), (512, 1024)]                            # matmul chunks, B
_SUBS = [(0, 128), (128, 196)]                             # within-window subchunks

# augmented-key row layout per parity: (q_lo, relh_lo, relw_lo)
_EVEN = (0, 64, 96)    # q 0:64,  rel rows above
_ODD = (64, 0, 32)     # q 64:128, rel rows below


def _standardize(nc, tc, nsl_list, jc, load_chunk, xn, ones1, onesP1, rows_p,
                 cast_engine="gpsimd", bf16_in=False):
    """Pure LN standardize: xn[:, j, c] = (x - mu[c]) * rsig[c], bf16 out.

    load_chunk(ci, lo, hi) -> AP [128, jc, w] for that chunk (may DMA into a
    fresh tile or return a view of a resident one); fp32 unless bf16_in (then
    it is used directly, no cast).  Stats run as bf16 matmuls vs a ones
    vector; mu/rsig are kept as bf16 rows, broadcast across partitions via
    tiny bf16 matmuls, evicted to bf16 SBUF and applied with two DVE
    tensor_tensor ops (all-bf16, stride-1 -> DVE fast mode).
    ones1: [128,1] bf16 ones; onesP1: [1,128] bf16 ones; rows_p: pool for rows.
    """
    nch = len(nsl_list)
    dimn = jc * 128

    eng_cast = getattr(nc, cast_engine)

    epsr = rows_p.tile([1, 1], dt.float32, tag="epsr")
    nc.vector.memset(epsr[:], EPS)

    with tc.tile_pool(name="ln_xb", bufs=nch) as xbp, \
         tc.tile_pool(name="ln_sq", bufs=2) as sqp, \
         tc.tile_pool(name="ln_st", bufs=2, space="PSUM") as st_ps, \
         tc.tile_pool(name="ln_bc", bufs=2, space="PSUM") as bc_ps, \
         tc.tile_pool(name="ln_bcs", bufs=4) as bcs, \
         tc.tile_pool(name="ln_rows", bufs=nch + 1) as rp:
        # per-chunk stats emitted with the apply of the PREVIOUS chunk
        # interleaved (one-chunk lag): PE runs stats back to back while the
        # row math / broadcast / apply of the prior chunk drains on Act/DVE
        xbs, mus, rss = [], [], []

        def emit_stats(ci, lo, hi):
            w = hi - lo
            xt = load_chunk(ci, lo, hi)
            if bf16_in:
                xb = xt
            else:
                xb = xbp.tile([128, jc, w], dt.bfloat16, tag="xb",
                              name=f"xb{ci}")
                eng_cast.tensor_copy(xb[:], xt)
                xb = xb[:]
            xbs.append(xb)
            # per-token sums -> mu
            pmu = st_ps.tile([1, w], dt.float32, tag="st")
            for j in range(jc):
                nc.tensor.matmul(pmu[:], ones1[:], xb[:, j, :],
                                 start=(j == 0), stop=(j == jc - 1))
            mu = rp.tile([1, w], dt.bfloat16, tag="mu", name=f"mu{ci}")
            nc.scalar.activation(mu[:], pmu[:], AF.Copy, scale=1.0 / dimn)
            mus.append(mu)
            # per-token sum of squares -> E[x^2]
            sq = sqp.tile([128, jc, w], dt.bfloat16, tag="sq")
            nc.vector.tensor_tensor(out=sq[:], in0=xb, in1=xb, op=ALU.mult)
            pmq = st_ps.tile([1, w], dt.float32, tag="st")
            for j in range(jc):
                nc.tensor.matmul(pmq[:], ones1[:], sq[:, j, :],
                                 start=(j == 0), stop=(j == jc - 1))
            mq = rp.tile([1, w], dt.float32, tag="mq")
            nc.scalar.activation(mq[:], pmq[:], AF.Copy, scale=1.0 / dimn)
            # rsig = 1/sqrt(E[x^2] - mu^2 + eps)
            m2 = rp.tile([1, w], dt.float32, tag="m2")
            nc.vector.tensor_tensor(out=m2[:], in0=mu[:], in1=mu[:],
                                    op=ALU.mult)
            nc.vector.tensor_tensor(out=mq[:], in0=mq[:], in1=m2[:],
                                    op=ALU.subtract)
            sd = rp.tile([1, w], dt.float32, tag="sd")
            nc.scalar.activation(sd[:], mq[:], AF.Sqrt, bias=epsr[:])
            rsig = rp.tile([1, w], dt.bfloat16, tag="rs", name=f"rs{ci}")
            with nc.allow_low_precision(reason="bf16 rsig row; 0.4% rel err "
                                        "matches the bf16 matmul noise "
                                        "floor"):
                nc.vector.reciprocal(rsig[:], sd[:])
            rss.append(rsig)

        def emit_apply(ci, lo, hi):
            w = hi - lo
            xb, mu, rsig = xbs[ci], mus[ci], rss[ci]
            bmu_p = bc_ps.tile([128, w], dt.float32, tag="bc")
            nc.tensor.matmul(bmu_p[:], onesP1[:], mu[:], start=True, stop=True)
            brs_p = bc_ps.tile([128, w], dt.float32, tag="bc")
            nc.tensor.matmul(brs_p[:], onesP1[:], rsig[:], start=True,
                             stop=True)
            bmu = bcs.tile([128, w], dt.bfloat16, tag="bmu")
            nc.scalar.copy(bmu[:], bmu_p[:])
            brs = bcs.tile([128, w], dt.bfloat16, tag="brs")
            nc.scalar.copy(brs[:], brs_p[:])
            # xn = (x - mu) * rsig   (two all-bf16 DVE ops, j-broadcast)
            cen = sqp.tile([128, jc, w], dt.bfloat16, tag="cen")
            nc.vector.tensor_tensor(
                out=cen[:], in0=xb,
                in1=bmu[:].unsqueeze(1).to_broadcast([128, jc, w]),
                op=ALU.subtract)
            with nc.allow_low_precision(reason="xn storage dtype (bf16/fp8) "
                                        "is the matmul operand precision"):
                nc.vector.tensor_tensor(
                    out=xn[:, :, lo:hi], in0=cen[:],
                    in1=brs[:].unsqueeze(1).to_broadcast([128, jc, w]),
                    op=ALU.mult)

        # chunk 0's apply is emitted right after its stats so the first xn
        # chunk (the qk-phase gate) is produced as early as possible
        emit_stats(0, *nsl_list[0])
        emit_apply(0, *nsl_list[0])
        for ci in range(1, nch):
            emit_stats(ci, *nsl_list[ci])
        for ci in range(1, nch):
            emit_apply(ci, *nsl_list[ci])


def build_attn():
    """Dispatch A: LN1 + qkv + windowed attention (+rel-pos) + proj."""
    nc = bacc.Bacc("TRN2", target_bir_lowering=False, debug=False)
    f32, bf16 = dt.float32, dt.bfloat16

    xT = nc.dram_tensor("xT", [128, JC, T], bf16, kind="ExternalInput").ap()
    f8 = dt.float8e4
    qkW = nc.dram_tensor("qkW", [128, 12, JC * 128], f8, kind="ExternalInput").ap()
    wvT = nc.dram_tensor("wvT", [128, JC, DIM], f8, kind="ExternalInput").ap()
    wpT = nc.dram_tensor("wpT", [128, JC, JC, 128], f8, kind="ExternalInput").ap()
    bqT = nc.dram_tensor("bqT", [128, JC], f32, kind="ExternalInput").ap()
    bkT = nc.dram_tensor("bkT", [128, JC], f32, kind="ExternalInput").ap()
    bvT = nc.dram_tensor("bvT", [1, 2, 384], bf16, kind="ExternalInput").ap()
    pbT = nc.dram_tensor("pbT", [128, JC], f32, kind="ExternalInput").ap()
    RhT = nc.dram_tensor("RhT", [64, WS, WS], f8, kind="ExternalInput").ap()
    RwT = nc.dram_tensor("RwT", [64, WS, WS], f8, kind="ExternalInput").ap()
    EhT = nc.dram_tensor("EhT", [WS, JC * NW, N], f8, kind="ExternalInput").ap()
    EwT = nc.dram_tensor("EwT", [WS, JC * NW, N], f8, kind="ExternalInput").ap()
    xoT = nc.dram_tensor("xoT", [128, JC, T], f32, kind="ExternalOutput").ap()

    with tile.TileContext(nc) as tc, ExitStack() as ctx:
        const = ctx.enter_context(tc.tile_pool(name="const", bufs=1))
        big = ctx.enter_context(tc.tile_pool(name="big", bufs=1))
        lnrows = ctx.enter_context(tc.tile_pool(name="lnrows", bufs=1))

        # ---- big persistent tensors (declared first so memsets start at t=0)
        # qhat/khat live in a partition-paired fp8 layout [64, two, j, t]:
        # logical contraction row r maps to (r % 64, r // 64), so the score
        # matmuls run fp8 DoubleRow.  Plane assignment per parity:
        #   even head: q = plane 0, rel_h rows 0:14 / rel_w 32:46 on plane 1
        #   odd head:  q = plane 1, rel_h rows 0:14 / rel_w 32:46 on plane 0
        xn = big.tile([128, JC, TPAD], dt.float8e4)
        qhE = big.tile([64, 2, JC, TPAD], dt.float8e4)
        khE = big.tile([64, 2, JC, TPAD], dt.float8e4)
        qhB = big.tile([64, 2, JC, TPAD], dt.float8e4)
        khB = big.tile([64, 2, JC, TPAD], dt.float8e4)
        vtok = big.tile([128, NW, 2, HEADS, HD + 1], bf16)

        # ---- constants ----
        ones1 = const.tile([128, 1], bf16)
        nc.vector.memset(ones1[:], 1.0)
        onesP1 = const.tile([1, 128], bf16)
        nc.vector.memset(onesP1[:], 1.0)
        onesT = const.tile([1, 128], bf16)
        nc.vector.memset(onesT[:], 1.0)
        ident = const.tile([128, 128], bf16)
        make_identity(nc, ident[:])

        rh = const.tile([64, WS, WS], f8)
        nc.scalar.dma_start(rh[:], RhT)
        rw = const.tile([64, WS, WS], f8)
        nc.scalar.dma_start(rw[:], RwT)
        bq = const.tile([128, JC], f32)
        nc.scalar.dma_start(bq[:], bqT)
        bk = const.tile([128, JC], f32)
        nc.scalar.dma_start(bk[:], bkT)
        bv = const.tile([1, 2, 384], bf16)
        nc.scalar.dma_start(bv[:], bvT)
        pb = const.tile([128, JC], f32)
        nc.scalar.dma_start(pb[:], pbT)
        # wv/wp tiles are created here but their loads are issued on the sync
        # queue after the x/qk-weight DMAs so the global DMA device serves x
        # first (sync-queue program order == DMA device order).
        wv = const.tile([128, JC, DIM], f8)
        wp = const.tile([128, JC, JC, 128], f8)

        # ---- LN1 (pure standardize; affine absorbed into weights) ----
        # x arrives already bf16 (host cast) -> no on-device cast, half DMA;
        # window-sized chunks get the first xn out early for the qk start
        with tc.tile_pool(name="ln_x", bufs=4) as xp:
            def load_chunk(ci, lo, hi):
                xt = xp.tile([128, JC, hi - lo], bf16, tag="x", name=f"x{ci}")
                nc.sync.dma_start(xt[:], xT[:, :, lo:hi])
                return xt[:]
            _standardize(nc, tc, _NSL, JC, load_chunk, xn, ones1, onesP1,
                         lnrows, bf16_in=True)

        # zero the rel/one-hot halves (gaps must be exactly 0; rel rows and
        # one-hot rows overlay these ranges later).  Issued after the LN body
        # so the Pool queue serves the LN casts first; the Tile deps still
        # order these before the mask DMAs / rel evictions below.
        nc.gpsimd.memset(khE[:, 1, :, :], 0.0)
        nc.gpsimd.memset(khB[:, 0, :, :], 0.0)
        nc.gpsimd.memset(qhE[:, 1, :, :], 0.0)
        nc.gpsimd.memset(qhB[:, 0, :, :], 0.0)
        # ones column in vtok (AV matmul also yields the softmax denominator)
        nc.gpsimd.memset(
            vtok[:].rearrange("p w s h o -> p (w s h) o")[:, :, HD:HD + 1], 1.0)

        # ---- q/k (feature-major, split by parity) + v (token-major) ----
        # PE program order is tuned so the in-order PE queue never waits on
        # slow producers: qk c0,c1 | v w0,w1 | qk c2,c3 | rel matmuls |
        # v w2..w6 (covers the rel-eviction drain) | pipelined core.
        _WHALF = [(0, 4), (4, 7)]
        with tc.tile_pool(name="qk_w", bufs=12) as wqk_sb, \
             tc.tile_pool(name="qk_stg", bufs=2) as stg_sb, \
             tc.tile_pool(name="qk_ps", bufs=3, space="PSUM") as qk_ps, \
             tc.tile_pool(name="v_ps", bufs=2, space="PSUM") as v_ps, \
             tc.tile_pool(name="rel_ps", bufs=3, space="PSUM") as rel_ps:
            wms = []
            for m in range(12):
                wm = wqk_sb.tile([128, JC, 128], f8, tag="wqk", name=f"w{m}")
                nc.sync.dma_start(
                    wm[:].rearrange("p j c -> p (j c)"), qkW[:, m, :])
                wms.append(wm)
            nc.sync.dma_start(wv[:], wvT)
            nc.sync.dma_start(wp[:], wpT)
            # one-hot key-position masks into khat rel rows; issued last on
            # the sync queue so x/weight transfers win the DMA device first
            for (msrc, mdst, tw, r0) in ((EhT, khE, 1, 0), (EwT, khE, 1, 32),
                                         (EhT, khB, 0, 0), (EwT, khB, 0, 32)):
                nc.sync.dma_start(
                    mdst[r0:r0 + WS, tw, :, 0:T].rearrange(
                        "p j (w n) -> p j w n", n=N),
                    msrc.rearrange("p (j w) n -> p j w n", w=NW))

            def emit_qk_chunk(ci):
                lo, hi = _NSL[ci]
                w = hi - lo
                # full-height evicts into per-chunk staging tiles (the bias
                # AP is per-partition, so one op covers both parity halves);
                # per chunk just 4 SBUF->SBUF DMAs distribute the halves
                qstg = stg_sb.tile([128, JC, 392], dt.float8e4, tag="qstg")
                kstg = stg_sb.tile([128, JC, 392], dt.float8e4, tag="kstg")
                for m in range(12):
                    is_q = m < JC
                    e = m % JC
                    pt = qk_ps.tile([128, 392], f32, tag="qk")
                    for jp in range(JC // 2):
                        nc.tensor.matmul(pt[:, :w],
                                         wms[m][:, 2 * jp:2 * jp + 2, :],
                                         xn[:, 2 * jp:2 * jp + 2, lo:hi],
                                         start=(jp == 0),
                                         stop=(jp == JC // 2 - 1),
                                         perf_mode=mybir.MatmulPerfMode.DoubleRow)
                    if is_q:
                        nc.scalar.activation(qstg[:, e, :w], pt[:, :w],
                                             AF.Identity, bias=bq[:, m:m + 1],
                                             scale=SCALE ** 0.5 / W8)
                    else:
                        nc.vector.tensor_scalar(
                            out=kstg[:, e, :w], in0=pt[:, :w],
                            scalar1=SCALE ** 0.5 / W8, scalar2=bk[:, e:e + 1],
                            op0=ALU.mult, op1=ALU.add)
                nc.sync.dma_start(qhE[:, 0, :, lo:hi], qstg[0:64, :, :w])
                nc.sync.dma_start(qhB[:, 1, :, lo:hi], qstg[64:128, :, :w])
                nc.sync.dma_start(khE[:, 0, :, lo:hi], kstg[0:64, :, :w])
                nc.sync.dma_start(khB[:, 1, :, lo:hi], kstg[64:128, :, :w])

            def emit_v(win):
                for si, (slo, shi) in enumerate(_SUBS):
                    ssz = shi - slo
                    base = win * N + slo
                    for half in range(2):
                        pv = v_ps.tile([128, 384], f32, tag="v")
                        for jp in range(JC // 2):
                            nc.tensor.matmul(
                                pv[:ssz, :],
                                xn[:, 2 * jp:2 * jp + 2, base:base + ssz],
                                wv[:, 2 * jp:2 * jp + 2,
                                   half * 384:(half + 1) * 384],
                                start=(jp == 0), stop=False,
                                perf_mode=mybir.MatmulPerfMode.DoubleRow)
                        nc.tensor.matmul(
                            pv[:ssz, :], onesT[:, :ssz], bv[:, half, :],
                            start=False, stop=True)
                        if (si + half) % 2 == 0:
                            nc.scalar.activation(
                                vtok[0:ssz, win, si,
                                     6 * half:6 * half + 6, 0:HD],
                                pv[:ssz, :].rearrange("p (h d) -> p h d",
                                                      d=HD),
                                AF.Copy, scale=1.0 / W8)
                        else:
                            nc.vector.tensor_scalar(
                                out=vtok[0:ssz, win, si,
                                         6 * half:6 * half + 6, 0:HD],
                                in0=pv[:ssz, :].rearrange(
                                    "p (h d) -> p h d", d=HD),
                                scalar1=1.0 / W8, scalar2=None, op0=ALU.mult)

            def emit_rel():
                # rel-pos rows into qhat planes, batched per h (rel_h) / per
                # w (rel_w); window halves keep PSUM cols <= 336.  Both
                # parities read q from partitions 0:64 of their q plane and
                # write rel rows 0:14 (rel_h) / 32:46 (rel_w) of the other.
                ri = 0
                for (w0, w1) in _WHALF:   # window halves outermost: the core
                    # can start on windows 0..3 while half 4..7 still drains
                    for par in range(2):
                        qh = qhE if par == 0 else qhB
                        qtw = 0 if par == 0 else 1       # q plane
                        rtw = 1 - qtw                    # rel plane
                        for typ in range(2):
                            r0 = 0 if typ == 0 else 32
                            stat = rh if typ == 0 else rw
                            for hh in range(WS):
                                nwn = w1 - w0
                                if typ == 0:
                                    mov = qh[:, qtw, :, 0:T].rearrange(
                                        "p j (win n) -> p j win n", n=N)[
                                        :, :, w0:w1, hh * WS:(hh + 1) * WS]
                                else:
                                    mov = qh[:, qtw, :, 0:T].rearrange(
                                        "p j (win kh kw) -> p j win kh kw",
                                        kh=WS, kw=WS)[:, :, w0:w1, :, hh]
                                ncols = JC * nwn * WS
                                prel = rel_ps.tile([128, 336], f32, tag="rel")
                                # skip_group_check: sim-only guard; its
                                # flat-address region view aliases across
                                # banks for <128-partition outputs
                                nc.tensor.matmul(
                                    prel[r0:r0 + WS, :ncols],
                                    stat[0:64, hh, :], mov,
                                    start=True, stop=True,
                                    tile_position=(0, r0),
                                    skip_group_check=True)
                                if typ == 0:
                                    dst = qh[r0:r0 + WS, rtw, :, 0:T].rearrange(
                                        "p j (win n) -> p j win n", n=N)[
                                        :, :, w0:w1, hh * WS:(hh + 1) * WS]
                                else:
                                    dst = qh[r0:r0 + WS, rtw, :, 0:T].rearrange(
                                        "p j (win kh kw) -> p j win kh kw",
                                        kh=WS, kw=WS)[:, :, w0:w1, :, hh]
                                src = prel[r0:r0 + WS, :ncols].rearrange(
                                    "p (j win k) -> p j win k", j=JC, win=nwn)
                                with nc.allow_low_precision(
                                        reason="fp8 rel rows; absolute score "
                                        "error ~3e-3 vs budget 0.1"):
                                    if ri % 2 == 0:
                                        nc.scalar.activation(
                                            dst, src, AF.Copy, scale=1.0 / W8)
                                    else:
                                        nc.vector.tensor_scalar(
                                            out=dst, in0=src,
                                            scalar1=1.0 / W8, scalar2=None,
                                            op0=ALU.mult)
                                ri += 1

            emit_qk_chunk(0)
            emit_qk_chunk(1)
            emit_v(0)
            emit_v(1)
            emit_qk_chunk(2)
            emit_qk_chunk(3)
            emit_rel()
            for win in range(2, NW):
                emit_v(win)

        # ---- attention core + proj: software-pipelined across windows ----
        # per iteration: AV+normalize(w) | scores+exp(w+1) | transp+proj(w);
        # window w+1's score matmuls keep PE busy while w's softmax
        # normalization drains on DVE.
        with tc.tile_pool(name="s_ps", bufs=1, space="PSUM") as s_ps, \
             tc.tile_pool(name="av_ps", bufs=2, space="PSUM") as av_ps, \
             tc.tile_pool(name="t_ps", bufs=2, space="PSUM") as t_ps, \
             tc.tile_pool(name="pj_ps", bufs=2, space="PSUM") as pj_ps, \
             tc.tile_pool(name="pt_sb", bufs=26) as pt_sb, \
             tc.tile_pool(name="ao_sb", bufs=2) as ao_sb, \
             tc.tile_pool(name="at_sb", bufs=2) as at_sb, \
             tc.tile_pool(name="xo_sb", bufs=2) as xo_sb, \
             tc.tile_pool(name="r_sb", bufs=4) as r_sb:
            # two persistent score tiles, rotated manually: the exp reads the
            # full [128, 392] tile, so the region no matmul covers (rows
            # 68:128 of the second key chunk) is zeroed exactly once
            sbufs = []
            for i in range(2):
                st = s_ps.tile([128, 392], f32, tag=f"s{i}", name=f"s{i}")
                # partition start must be 32-aligned on PSUM; rows 64:68 are
                # re-written by every second score matmul afterwards
                nc.vector.memset(st[64:128, 196:392], 0.0)
                sbufs.append(st)
            state = {"hidx": 0}

            def emit_scores(win):
                pts = []
                for head in range(HEADS):
                    blk = head // 2
                    par = head % 2
                    qh = qhE if par == 0 else qhB
                    kh = khE if par == 0 else khB
                    ps_t = sbufs[state["hidx"] % 2]
                    state["hidx"] += 1
                    for si, (slo, shi) in enumerate(_SUBS):
                        ssz = shi - slo
                        nc.tensor.matmul(
                            ps_t[:ssz, si * N:si * N + N],
                            kh[:, :, blk, win * N + slo:win * N + shi],
                            qh[:, :, blk, win * N:(win + 1) * N],
                            start=True, stop=True,
                            perf_mode=mybir.MatmulPerfMode.DoubleRow)
                    ptile = pt_sb.tile([128, 392], bf16, tag="pt")
                    nc.scalar.activation(ptile[:], ps_t[:], AF.Exp)
                    pts.append(ptile)
                return pts

            def emit_av(win, pts):
                ao0 = ao_sb.tile([128, DIM], bf16, tag="ao0")
                ao1 = ao_sb.tile([68, DIM], bf16, tag="ao1")
                for qi, (qlo, qhi) in enumerate(_SUBS):
                    qsz = qhi - qlo
                    ao = ao0 if qi == 0 else ao1
                    for hg in range(2):
                        pav = av_ps.tile([128, 6, HD + 1], f32, tag="av")
                        for hl in range(6):
                            head = hg * 6 + hl
                            for si, (slo, shi) in enumerate(_SUBS):
                                ssz = shi - slo
                                nc.tensor.matmul(
                                    pav[:qsz, hl, :],
                                    pts[head][0:ssz, si * N + qlo:si * N + qhi],
                                    vtok[0:ssz, win, si, head, :],
                                    start=(si == 0), stop=(si == 1))
                        rec = r_sb.tile([128, 6], f32, tag="rec")
                        nc.vector.reciprocal(
                            rec[:qsz, :],
                            pav[:qsz, :, HD:HD + 1].rearrange(
                                "p h o -> p (h o)"))
                        nc.vector.tensor_tensor(
                            out=ao[0:qsz, hg * 384:(hg + 1) * 384].rearrange(
                                "p (h d) -> p h d", d=HD),
                            in0=pav[:qsz, :, 0:HD],
                            in1=rec[:qsz, :].unsqueeze(2).to_broadcast(
                                [qsz, 6, HD]),
                            op=ALU.mult)
                return ao0, ao1

            def emit_transp_proj(win, ao0, ao1):
                aT = at_sb.tile([128, JC, NPAD], dt.float8e4, tag="at")
                ti = 0
                for j in range(JC):
                    for qi, (qlo, qhi) in enumerate(_SUBS):
                        qsz = qhi - qlo
                        src = ao0 if qi == 0 else ao1
                        ptt = t_ps.tile([128, 128], bf16, tag="tp")
                        nc.tensor.transpose(ptt[:, :qsz],
                                            src[0:qsz, j * 128:(j + 1) * 128],
                                            ident[0:qsz, 0:qsz])
                        if ti % 3 == 0:
                            nc.scalar.copy(aT[:, j, qlo:qhi], ptt[:, :qsz])
                        else:
                            nc.vector.tensor_copy(aT[:, j, qlo:qhi],
                                                  ptt[:, :qsz])
                        ti += 1
                xo_t = xo_sb.tile([128, JC, N], f32, tag="xo")
                for m in range(JC):
                    pp = pj_ps.tile([128, N], f32, tag="pj")
                    for jp in range(JC // 2):
                        nc.tensor.matmul(
                            pp[:], wp[:, 2 * jp:2 * jp + 2, m, :],
                            aT[:, 2 * jp:2 * jp + 2, 0:N],
                            start=(jp == 0), stop=(jp == JC // 2 - 1),
                            perf_mode=mybir.MatmulPerfMode.DoubleRow)
                    nc.vector.tensor_scalar(
                        out=xo_t[:, m, :], in0=pp[:],
                        scalar1=1.0 / W8, scalar2=pb[:, m:m + 1],
                        op0=ALU.mult, op1=ALU.add)
                nc.sync.dma_start(
                    xoT.rearrange("p j (w n) -> p j w n", n=N)[:, :, win, :],
                    xo_t[:])

            pts = emit_scores(0)
            for win in range(NW):
                ao0, ao1 = emit_av(win, pts)
                if win + 1 < NW:
                    pts = emit_scores(win + 1)
                emit_transp_proj(win, ao0, ao1)
    nc.compile()
    return nc


def build_mlp():
    """Dispatch B: y = x + fc2(gelu(fc1(LN2(x)))), 1024 tokens/core.
    LN2 affine is absorbed into fc1 host-side."""
    nc = bacc.Bacc("TRN2", target_bir_lowering=False, debug=False)
    f32, bf16 = dt.float32, dt.bfloat16

    xT = nc.dram_tensor("xT", [128, JC, TB], bf16, kind="ExternalInput").ap()
    f8 = dt.float8e4
    fc1W = nc.dram_tensor("fc1W", [128, HC, DIM], f8, kind="ExternalInput").ap()
    fc1S = nc.dram_tensor("fc1S", [128, HC], f32, kind="ExternalInput").ap()
    fc2W = nc.dram_tensor("fc2W", [128, HC, DIM], bf16, kind="ExternalInput").ap()
    fc1B = nc.dram_tensor("fc1B", [128, HC], f32, kind="ExternalInput").ap()
    fc2B = nc.dram_tensor("fc2B", [128, JC], f32, kind="ExternalInput").ap()
    yT = nc.dram_tensor("yT", [128, JC, TB], f32, kind="ExternalOutput").ap()

    with tile.TileContext(nc) as tc, ExitStack() as ctx:
        const = ctx.enter_context(tc.tile_pool(name="const", bufs=1))
        big = ctx.enter_context(tc.tile_pool(name="big", bufs=1))
        lnrows = ctx.enter_context(tc.tile_pool(name="lnrows", bufs=1))

        ones1 = const.tile([128, 1], bf16)
        nc.vector.memset(ones1[:], 1.0)
        onesP1 = const.tile([1, 128], bf16)
        nc.vector.memset(onesP1[:], 1.0)
        b1t = const.tile([128, HC], f32)
        nc.scalar.dma_start(b1t[:], fc1B)
        s1t = const.tile([128, HC], f32)
        nc.scalar.dma_start(s1t[:], fc1S)
        b2t = const.tile([128, JC], f32)
        nc.scalar.dma_start(b2t[:], fc2B)

        xtiles = big.tile([128, JC, TB], bf16)
        xn = big.tile([128, JC, TB], dt.float8e4)
        h = big.tile([128, HC, TB], bf16)

        with tc.tile_pool(name="w1_sb", bufs=HC) as w1p, \
             tc.tile_pool(name="w2_sb", bufs=1) as w2p:
            # x chunks first on the sync queue -> served first by the DMA
            # device; weights follow in need order (w1 chunks, then w2)
            for lo, hi in _NSL_B:
                nc.sync.dma_start(xtiles[:, :, lo:hi], xT[:, :, lo:hi])
            w1s = []
            for m in range(HC):
                w1m = w1p.tile([128, JC, 128], f8, tag="w1", name=f"w1_{m}")
                nc.sync.dma_start(
                    w1m[:].rearrange("p j c -> p (j c)"), fc1W[:, m, :])
                w1s.append(w1m)
            w2t = w2p.tile([128, HC, DIM], bf16)
            nc.sync.dma_start(w2t[:], fc2W)

            def load_chunk(ci, lo, hi):
                return xtiles[:, :, lo:hi]

            _standardize(nc, tc, _NSL_B, JC, load_chunk, xn, ones1, onesP1,
                         lnrows, bf16_in=True)

            with tc.tile_pool(name="f1_ps", bufs=4, space="PSUM") as f1_ps, \
                 tc.tile_pool(name="f2_ps", bufs=3, space="PSUM") as f2_ps, \
                 tc.tile_pool(name="out_sb", bufs=3) as out_sb:
                for ci, (lo, hi) in enumerate(_FC_B):
                    w = hi - lo
                    for m in range(HC):
                        pt = f1_ps.tile([128, w], f32, tag="f1")
                        for jp in range(JC // 2):
                            nc.tensor.matmul(
                                pt[:], w1s[m][:, 2 * jp:2 * jp + 2, :],
                                xn[:, 2 * jp:2 * jp + 2, lo:hi],
                                start=(jp == 0), stop=(jp == JC // 2 - 1),
                                perf_mode=mybir.MatmulPerfMode.DoubleRow)
                        # per-out-channel fp8 descale via the activation
                        # scale AP; bias applies after the scale
                        nc.scalar.activation(h[:, m, lo:hi], pt[:], AF.Gelu,
                                             bias=b1t[:, m:m + 1],
                                             scale=s1t[:, m:m + 1])
                for ci, (lo, hi) in enumerate(_FC_B):
                    w = hi - lo
                    for m in range(JC):
                        pt = f2_ps.tile([128, w], f32, tag="f2")
                        for j in range(HC):
                            nc.tensor.matmul(pt[:], w2t[:, j, m * 128:(m + 1) * 128],
                                             h[:, j, lo:hi],
                                             start=(j == 0), stop=(j == HC - 1))
                        ot = out_sb.tile([128, w], f32, tag="out")
                        # ot = (psum + fc2_b) + x   in one DVE pass
                        nc.vector.scalar_tensor_tensor(
                            out=ot[:], in0=pt[:], scalar=b2t[:, m:m + 1],
                            in1=xtiles[:, m, lo:hi], op0=ALU.add, op1=ALU.add)
                        nc.sync.dma_start(yT[:, m, lo:hi], ot[:])
    nc.compile()
    return nc


# ---------------- host glue ----------------

_CACHE = {}


def _get(name, builder):
    if name not in _CACHE:
        _CACHE[name] = builder()
    return _CACHE[name]


def _featmajor(a):
    """(T, 768) fp32 -> [128, 6, T]"""
    Tn = a.shape[0]
    return np.ascontiguousarray(a.T.reshape(JC, 128, Tn).transpose(1, 0, 2))


def _unfeat(aT):
    """[128, 6, T] -> (T, 768)"""
    return np.asarray(aT).transpose(1, 0, 2).reshape(DIM, -1).T


def _chunkvec(v):
    """(c*128,) -> [128, c] fp32"""
    v = np.asarray(v, np.float32)
    return np.ascontiguousarray(v.reshape(-1, 128).T)


def _wchunk(w, nchunk, dtype=BF16):
    """(768, nchunk*128) weight -> [128, nchunk, 768] (m-major chunks:
    out[p, m, j*128 + c] = w[j*128 + p, m*128 + c])."""
    w = np.asarray(w, np.float32)
    kin = w.shape[0] // 128
    out = w.reshape(kin, 128, nchunk, 128).transpose(1, 2, 0, 3)
    return np.ascontiguousarray(out.reshape(128, nchunk, kin * 128)).astype(dtype)


def _bf16(a):
    return np.asarray(a, dtype=BF16)


def _build_rel(rel_pos, ws=WS):
    """[64, 14, 14] fp8: out[c, h, k] = rel_pos[idx[h,k], c] * W8 / sqrt(SCALE)
    (qhat holds sqrt(SCALE)*q; W8 prescale is undone at the rel eviction)."""
    idx = np.arange(ws)[:, None] - np.arange(ws)[None, :] + (ws - 1)
    R = np.asarray(rel_pos, np.float32)[idx] * (W8 / SCALE ** 0.5)
    return R.transpose(2, 0, 1).astype(F8)


def _build_onehots():
    """Eh[r, :, k] = 1 if k//14 == r;  Ew[r, :, k] = 1 if k%14 == r,
    pre-expanded over the (j, win) axis for big contiguous DMA runs."""
    k = np.arange(N)
    Eh = (k[None, :] // WS == np.arange(WS)[:, None]).astype(np.float32)
    Ew = (k[None, :] % WS == np.arange(WS)[:, None]).astype(np.float32)
    Eh = np.ascontiguousarray(np.broadcast_to(Eh[:, None, :], (WS, JC * NW, N)))
    Ew = np.ascontiguousarray(np.broadcast_to(Ew[:, None, :], (WS, JC * NW, N)))
    return Eh.astype(F8), Ew.astype(F8)


kernel_last_perf = {}

try:
    from antenv.axon_hooks import get_axon_ntff_profile_hook as _hook  # noqa: F401
    _HAVE_TRACE = True
except ImportError:
    _HAVE_TRACE = False
    import os as _os
    _os.environ["BASS_NEVER_TRACE"] = "1"   # bass_utils re-reads BASS_TRACE


def window_x(x):
    """(2, 64, 64, 768) -> (56, 196, 768) padded window tokens."""
    B, H, W, C = x.shape
    xp = np.zeros((B, 70, 70, C), np.float32)
    xp[:, :64, :64] = x
    xw = xp.reshape(B, 5, WS, 5, WS, C).transpose(0, 1, 3, 2, 4, 5).reshape(50, N, C)
    xall = np.zeros((56, N, C), np.float32)
    xall[:50] = xw
    return xall


def attn_consts(norm1_w, norm1_b, qkv_w, qkv_b, proj_w, proj_b,
                rel_pos_h, rel_pos_w):
    """Host-side constant tensors for dispatch A (LN1 affine absorbed)."""
    n1w = np.asarray(norm1_w, np.float32)
    n1b = np.asarray(norm1_b, np.float32)
    qkvw = np.asarray(qkv_w, np.float32)
    qkvb = np.asarray(qkv_b, np.float32)
    Wq = n1w[:, None] * qkvw                 # (768, 2304)
    bfull = n1b @ qkvw + qkvb                # (2304,)
    Eh, Ew = _build_onehots()
    return {
        "qkW": _wchunk(Wq[:, 0:2 * DIM] * W8, 12, F8),
        "wvT": np.ascontiguousarray(
            Wq[:, 2 * DIM:].reshape(JC, 128, DIM).transpose(1, 0, 2)
            * W8).astype(F8),
        "wpT": np.ascontiguousarray(
            np.asarray(proj_w, np.float32).reshape(JC, 128, JC, 128)
            .transpose(1, 0, 2, 3) * W8).astype(F8),
        "bqT": _chunkvec(bfull[0:DIM] * SCALE ** 0.5),
        "bkT": _chunkvec(bfull[DIM:2 * DIM] * SCALE ** 0.5),
        "bvT": _bf16(bfull[2 * DIM:].reshape(1, 2, 384) * W8),
        "pbT": _chunkvec(proj_b),
        "RhT": _build_rel(rel_pos_h),
        "RwT": _build_rel(rel_pos_w),
        "EhT": Eh,
        "EwT": Ew,
    }


def mlp_consts(norm2_w, norm2_b, fc1_w, fc1_b, fc2_w, fc2_b):
    """Host-side constant tensors for dispatch B (LN2 affine absorbed)."""
    n2w = np.asarray(norm2_w, np.float32)
    n2b = np.asarray(norm2_b, np.float32)
    f1w = np.asarray(fc1_w, np.float32)
    W1 = n2w[:, None] * f1w                  # (768, 3072)
    b1 = n2b @ f1w + np.asarray(fc1_b, np.float32)
    # per-out-channel power-of-2 fp8 scaling for fc1 (exactly undone by the
    # gelu activation's per-partition scale AP)
    colmax = np.abs(W1).max(axis=0)                        # (3072,)
    sexp = np.clip(np.floor(np.log2(224.0 / np.maximum(colmax, 1e-30))),
                   -20, 20)
    wscale = np.exp2(sexp)                                 # (3072,)
    return {
        "fc1W": _wchunk(W1 * wscale[None, :], HC, F8),
        "fc1S": _chunkvec(1.0 / wscale),
        "fc2W": _bf16(np.ascontiguousarray(
            np.asarray(fc2_w, np.float32).reshape(HC, 128, DIM)
            .transpose(1, 0, 2))),
        "fc1B": _chunkvec(b1),
        "fc2B": _chunkvec(fc2_b),
    }


def kernel(x, norm1_w, norm1_b, qkv_w, qkv_b, proj_w, proj_b,
           rel_pos_h, rel_pos_w, norm2_w, norm2_b,
           fc1_w, fc1_b, fc2_w, fc2_b):
    import os
    trace = bool(os.environ.get("BASS_TRACE")) and _HAVE_TRACE
    x = np.asarray(x, np.float32)
    B, H, W, C = x.shape
    assert (B, H, W, C) == (2, 64, 64, DIM)

    # ---- dispatch A: windowed attention ----
    nc_a = _get("attn", build_attn)
    xall = window_x(x)
    consts_a = attn_consts(norm1_w, norm1_b, qkv_w, qkv_b, proj_w, proj_b,
                           rel_pos_h, rel_pos_w)
    in_maps = []
    for c in range(NCORES):
        m = dict(consts_a)
        m["xT"] = _featmajor(
            xall[c * NW:(c + 1) * NW].reshape(T, C)).astype(BF16)
        in_maps.append(m)
    res_a = run_bass_kernel_spmd(nc_a, in_maps, core_ids=list(range(NCORES)),
                                 trace=trace)
    kernel_last_perf["attn"] = res_a.exec_time_ns
    xo_all = np.stack([_unfeat(res_a.results[c]["xoT"]) for c in range(NCORES)])
    xo = xo_all.reshape(56, N, C)[:50]
    xo = xo.reshape(B, 5, 5, WS, WS, C).transpose(0, 1, 3, 2, 4, 5).reshape(B, 70, 70, C)
    x2 = x + xo[:, :64, :64]

    # ---- dispatch B: MLP ----
    nc_b = _get("mlp", build_mlp)
    consts_b = mlp_consts(norm2_w, norm2_b, fc1_w, fc1_b, fc2_w, fc2_b)
    x2f = np.ascontiguousarray(x2.reshape(B * H * W, C))
    in_maps = []
    for c in range(NCORES):
        m = dict(consts_b)
        m["xT"] = _featmajor(x2f[c * TB:(c + 1) * TB]).astype(BF16)
        in_maps.append(m)
    res_b = run_bass_kernel_spmd(nc_b, in_maps, core_ids=list(range(NCORES)),
                                 trace=trace)
    kernel_last_perf["mlp"] = res_b.exec_time_ns
    y = np.concatenate([_unfeat(res_b.results[c]["yT"]) for c in range(NCORES)])
    return y.reshape(B, H, W, C).astype(np.float32)


# revision 49
# speedup vs baseline: 1.1538x; 1.1538x over previous
"""Trainium2 Bass kernel for a SAM/ViTDet-style windowed-attention transformer
block (DIM=768, 12 heads, window 14, decomposed rel-pos bias, exact-gelu MLP).

Contract: kernel(**inputs) takes the FULL unsharded inputs from
reference.setup_inputs() and returns the FULL (2, 64, 64, 768) float32 output.

Strategy (8 NeuronCores, SPMD, data-parallel):
  Dispatch A (attention): shard the 50 real windows (padded to 56) as 7
    windows/core. Per core: LN1 -> qkv -> windowed attention with the
    decomposed rel-pos bias folded into an augmented-key matmul -> proj.
  Host: window-unpartition, crop, residual add.
  Dispatch B (MLP): shard the 8192 tokens as 1024/core. Per core:
    LN2 -> fc1 -> exact GELU -> fc2 -> residual.

Perf notes (v2):
  * The LN affine (w, b) is absorbed host-side into the following matmul
    weights/biases, so on-device LN is a pure standardize: bf16 stats
    matmuls + bf16 broadcast tiles + two 4x-rate DVE tensor_tensor ops.
  * Rel-pos rows are produced by per-(h or w) batched matmuls (112 instead
    of 392) whose PSUM outputs land at partition bases 0/32/64/96, cutting
    eviction traffic.
  * The two score matmuls of one (window, head) share a [128, 392] PSUM
    tile -> a single exp instruction per head.
  * AV outputs are 6-head-batched in PSUM; the softmax normalization is one
    DVE multiply with a stride-0 (broadcast) reciprocal operand.
  * Weight DMAs are chunked and issued up front so compute starts ~10us in.

Augmented-key rel-pos layout (q/k head-pair blocks, per j-chunk):
  even head: q 0:64,  rel_h 64:78,  zeros 78:96, rel_w 96:110, zeros 110:128
  odd head:  rel_h 0:14, zeros 14:32, rel_w 32:46, zeros 46:64, q 64:128
khat holds k values in the q rows and one-hot key-position masks in the rel
rows; zero gaps make the extra contraction rows inert. S^T = khat^T qhat then
includes the decomposed bias exactly.
"""

import sys

sys.path.insert(0, "/opt/trn_rl_repo")

from contextlib import ExitStack

import numpy as np
import ml_dtypes

import concourse.bacc as bacc
import concourse.mybir as mybir
import concourse.tile as tile
from concourse.bass_utils import run_bass_kernel_spmd
from concourse.masks import make_identity

dt = mybir.dt
AF = mybir.ActivationFunctionType
ALU = mybir.AluOpType

DIM = 768
HEADS = 12
HD = 64
WS = 14
N = WS * WS          # 196 tokens / window
NW = 7               # windows per core
T = NW * N           # 1372 token slots per core (dispatch A)
TB = 1024            # tokens per core (dispatch B)
MLP = 3072
NCORES = 8
JC = DIM // 128      # 6 feature chunks
HC = MLP // 128      # 24 hidden chunks
EPS = 1e-5
SCALE = HD ** -0.5   # 0.125
TPAD = 1376          # T rounded up so fp8 DoubleRow pair strides are 16B-aligned
NPAD = 208           # N rounded up likewise (aT)
BF16 = ml_dtypes.bfloat16
F8 = ml_dtypes.float8_e4m3
W8 = 64.0   # fp8 weight pre-scale (avoids e4m3 subnormals); undone at eviction

# window-aligned token chunks for dispatch A (2+2+2+1 windows)
_NSL = [(0, 392), (392, 784), (784, 1176), (1176, 1372)]
_NSL_LN = [(i * N, (i + 1) * N) for i in range(NW)]        # LN chunks, A
_NSL_B = [(0, 256), (256, 512), (512, 768), (768, 1024)]   # LN chunks, B
_FC_B = [(0, 512), (512, 1024)]                            # matmul chunks, B
_SUBS = [(0, 128), (128, 196)]                             # within-window subchunks

# augmented-key row layout per parity: (q_lo, relh_lo, relw_lo)
_EVEN = (0, 64, 96)    # q 0:64,  rel rows above
_ODD = (64, 0, 32)     # q 64:128, rel rows below


def _standardize(nc, tc, nsl_list, jc, load_chunk, xn, ones1, onesP1, rows_p,
                 cast_engine="gpsimd", bf16_in=False):
    """Pure LN standardize: xn[:, j, c] = (x - mu[c]) * rsig[c], bf16 out.

    load_chunk(ci, lo, hi) -> AP [128, jc, w] for that chunk (may DMA into a
    fresh tile or return a view of a resident one); fp32 unless bf16_in (then
    it is used directly, no cast).  Stats run as bf16 matmuls vs a ones
    vector; mu/rsig are kept as bf16 rows, broadcast across partitions via
    tiny bf16 matmuls, evicted to bf16 SBUF and applied with two DVE
    tensor_tensor ops (all-bf16, stride-1 -> DVE fast mode).
    ones1: [128,1] bf16 ones; onesP1: [1,128] bf16 ones; rows_p: pool for rows.
    """
    nch = len(nsl_list)
    dimn = jc * 128

    eng_cast = getattr(nc, cast_engine)

    epsr = rows_p.tile([1, 1], dt.float32, tag="epsr")
    nc.vector.memset(epsr[:], EPS)

    with tc.tile_pool(name="ln_xb", bufs=nch) as xbp, \
         tc.tile_pool(name="ln_sq", bufs=2) as sqp, \
         tc.tile_pool(name="ln_st", bufs=2, space="PSUM") as st_ps, \
         tc.tile_pool(name="ln_bc", bufs=2, space="PSUM") as bc_ps, \
         tc.tile_pool(name="ln_bcs", bufs=4) as bcs, \
         tc.tile_pool(name="ln_rows", bufs=nch + 1) as rp:
        # per-chunk stats emitted with the apply of the PREVIOUS chunk
        # interleaved (one-chunk lag): PE runs stats back to back while the
        # row math / broadcast / apply of the prior chunk drains on Act/DVE
        xbs, mus, rss = [], [], []

        def emit_stats(ci, lo, hi):
            w = hi - lo
            xt = load_chunk(ci, lo, hi)
            if bf16_in:
                xb = xt
            else:
                xb = xbp.tile([128, jc, w], dt.bfloat16, tag="xb",
                              name=f"xb{ci}")
                eng_cast.tensor_copy(xb[:], xt)
                xb = xb[:]
            xbs.append(xb)
            # per-token sums -> mu
            pmu = st_ps.tile([1, w], dt.float32, tag="st")
            for j in range(jc):
                nc.tensor.matmul(pmu[:], ones1[:], xb[:, j, :],
                                 start=(j == 0), stop=(j == jc - 1))
            mu = rp.tile([1, w], dt.bfloat16, tag="mu", name=f"mu{ci}")
            nc.scalar.activation(mu[:], pmu[:], AF.Copy, scale=1.0 / dimn)
            mus.append(mu)
            # per-token sum of squares -> E[x^2]
            sq = sqp.tile([128, jc, w], dt.bfloat16, tag="sq")
            nc.vector.tensor_tensor(out=sq[:], in0=xb, in1=xb, op=ALU.mult)
            pmq = st_ps.tile([1, w], dt.float32, tag="st")
            for j in range(jc):
                nc.tensor.matmul(pmq[:], ones1[:], sq[:, j, :],
                                 start=(j == 0), stop=(j == jc - 1))
            mq = rp.tile([1, w], dt.float32, tag="mq")
            nc.scalar.activation(mq[:], pmq[:], AF.Copy, scale=1.0 / dimn)
            # rsig = 1/sqrt(E[x^2] - mu^2 + eps)
            m2 = rp.tile([1, w], dt.float32, tag="m2")
            nc.vector.tensor_tensor(out=m2[:], in0=mu[:], in1=mu[:],
                                    op=ALU.mult)
            nc.vector.tensor_tensor(out=mq[:], in0=mq[:], in1=m2[:],
                                    op=ALU.subtract)
            sd = rp.tile([1, w], dt.float32, tag="sd")
            nc.scalar.activation(sd[:], mq[:], AF.Sqrt, bias=epsr[:])
            rsig = rp.tile([1, w], dt.bfloat16, tag="rs", name=f"rs{ci}")
            with nc.allow_low_precision(reason="bf16 rsig row; 0.4% rel err "
                                        "matches the bf16 matmul noise "
                                        "floor"):
                nc.vector.reciprocal(rsig[:], sd[:])
            rss.append(rsig)

        def emit_apply(ci, lo, hi):
            w = hi - lo
            xb, mu, rsig = xbs[ci], mus[ci], rss[ci]
            bmu_p = bc_ps.tile([128, w], dt.float32, tag="bc")
            nc.tensor.matmul(bmu_p[:], onesP1[:], mu[:], start=True, stop=True)
            brs_p = bc_ps.tile([128, w], dt.float32, tag="bc")
            nc.tensor.matmul(brs_p[:], onesP1[:], rsig[:], start=True,
                             stop=True)
            bmu = bcs.tile([128, w], dt.bfloat16, tag="bmu")
            nc.scalar.copy(bmu[:], bmu_p[:])
            brs = bcs.tile([128, w], dt.bfloat16, tag="brs")
            nc.scalar.copy(brs[:], brs_p[:])
            # xn = (x - mu) * rsig   (two all-bf16 DVE ops, j-broadcast)
            cen = sqp.tile([128, jc, w], dt.bfloat16, tag="cen")
            nc.vector.tensor_tensor(
                out=cen[:], in0=xb,
                in1=bmu[:].unsqueeze(1).to_broadcast([128, jc, w]),
                op=ALU.subtract)
            with nc.allow_low_precision(reason="xn storage dtype (bf16/fp8) "
                                        "is the matmul operand precision"):
                nc.vector.tensor_tensor(
                    out=xn[:, :, lo:hi], in0=cen[:],
                    in1=brs[:].unsqueeze(1).to_broadcast([128, jc, w]),
                    op=ALU.mult)

        # chunk 0's apply is emitted right after its stats so the first xn
        # chunk (the qk-phase gate) is produced as early as possible
        emit_stats(0, *nsl_list[0])
        emit_apply(0, *nsl_list[0])
        for ci in range(1, nch):
            emit_stats(ci, *nsl_list[ci])
        for ci in range(1, nch):
            emit_apply(ci, *nsl_list[ci])


def build_attn(with_vbias=True):
    """Dispatch A: LN1 + qkv + windowed attention (+rel-pos) + proj."""
    nc = bacc.Bacc("TRN2", target_bir_lowering=False, debug=False)
    f32, bf16 = dt.float32, dt.bfloat16

    xT = nc.dram_tensor("xT", [128, JC, T], bf16, kind="ExternalInput").ap()
    f8 = dt.float8e4
    qkW = nc.dram_tensor("qkW", [128, 12, JC * 128], f8, kind="ExternalInput").ap()
    wvT = nc.dram_tensor("wvT", [128, JC, DIM], f8, kind="ExternalInput").ap()
    wpT = nc.dram_tensor("wpT", [128, JC, JC, 128], f8, kind="ExternalInput").ap()
    bqT = nc.dram_tensor("bqT", [128, JC], f32, kind="ExternalInput").ap()
    bkT = nc.dram_tensor("bkT", [128, JC], f32, kind="ExternalInput").ap()
    bvT = nc.dram_tensor("bvT", [1, 2, 384], bf16, kind="ExternalInput").ap()
    pbT = nc.dram_tensor("pbT", [128, JC], f32, kind="ExternalInput").ap()
    RhT = nc.dram_tensor("RhT", [64, WS, WS], f8, kind="ExternalInput").ap()
    RwT = nc.dram_tensor("RwT", [64, WS, WS], f8, kind="ExternalInput").ap()
    EhT = nc.dram_tensor("EhT", [WS, JC * NW, N], f8, kind="ExternalInput").ap()
    EwT = nc.dram_tensor("EwT", [WS, JC * NW, N], f8, kind="ExternalInput").ap()
    xoT = nc.dram_tensor("xoT", [128, JC, T], f32, kind="ExternalOutput").ap()

    with tile.TileContext(nc) as tc, ExitStack() as ctx:
        const = ctx.enter_context(tc.tile_pool(name="const", bufs=1))
        big = ctx.enter_context(tc.tile_pool(name="big", bufs=1))
        lnrows = ctx.enter_context(tc.tile_pool(name="lnrows", bufs=1))

        # ---- big persistent tensors (declared first so memsets start at t=0)
        # qhat/khat live in a partition-paired fp8 layout [64, two, j, t]:
        # logical contraction row r maps to (r % 64, r // 64), so the score
        # matmuls run fp8 DoubleRow.  Plane assignment per parity:
        #   even head: q = plane 0, rel_h rows 0:14 / rel_w 32:46 on plane 1
        #   odd head:  q = plane 1, rel_h rows 0:14 / rel_w 32:46 on plane 0
        xn = big.tile([128, JC, TPAD], dt.float8e4)
        qhE = big.tile([64, 2, JC, TPAD], dt.float8e4)
        khE = big.tile([64, 2, JC, TPAD], dt.float8e4)
        qhB = big.tile([64, 2, JC, TPAD], dt.float8e4)
        khB = big.tile([64, 2, JC, TPAD], dt.float8e4)
        vtok = big.tile([128, NW, 2, HEADS, HD + 1], bf16)

        # ---- constants ----
        ones1 = const.tile([128, 1], bf16)
        nc.vector.memset(ones1[:], 1.0)
        onesP1 = const.tile([1, 128], bf16)
        nc.vector.memset(onesP1[:], 1.0)
        onesT = const.tile([1, 128], bf16)
        nc.vector.memset(onesT[:], 1.0)
        ident = const.tile([128, 128], bf16)
        make_identity(nc, ident[:])

        rh = const.tile([64, WS, WS], f8)
        nc.scalar.dma_start(rh[:], RhT)
        rw = const.tile([64, WS, WS], f8)
        nc.scalar.dma_start(rw[:], RwT)
        bq = const.tile([128, JC], f32)
        nc.scalar.dma_start(bq[:], bqT)
        bk = const.tile([128, JC], f32)
        nc.scalar.dma_start(bk[:], bkT)
        bv = const.tile([1, 2, 384], bf16)
        nc.scalar.dma_start(bv[:], bvT)
        pb = const.tile([128, JC], f32)
        nc.scalar.dma_start(pb[:], pbT)
        # wv/wp tiles are created here but their loads are issued on the sync
        # queue after the x/qk-weight DMAs so the global DMA device serves x
        # first (sync-queue program order == DMA device order).
        wv = const.tile([128, JC, DIM], f8)
        wp = const.tile([128, JC, JC, 128], f8)

        # ---- LN1 (pure standardize; affine absorbed into weights) ----
        # x arrives already bf16 (host cast) -> no on-device cast, half DMA;
        # window-sized chunks get the first xn out early for the qk start
        with tc.tile_pool(name="ln_x", bufs=4) as xp:
            def load_chunk(ci, lo, hi):
                xt = xp.tile([128, JC, hi - lo], bf16, tag="x", name=f"x{ci}")
                nc.sync.dma_start(xt[:], xT[:, :, lo:hi])
                return xt[:]
            _standardize(nc, tc, _NSL, JC, load_chunk, xn, ones1, onesP1,
                         lnrows, bf16_in=True)

        # zero the rel/one-hot halves (gaps must be exactly 0; rel rows and
        # one-hot rows overlay these ranges later).  Issued after the LN body
        # so the Pool queue serves the LN casts first; the Tile deps still
        # order these before the mask DMAs / rel evictions below.
        nc.gpsimd.memset(khE[:, 1, :, :], 0.0)
        nc.gpsimd.memset(khB[:, 0, :, :], 0.0)
        nc.gpsimd.memset(qhE[:, 1, :, :], 0.0)
        nc.gpsimd.memset(qhB[:, 0, :, :], 0.0)
        # ones column in vtok (AV matmul also yields the softmax denominator)
        nc.gpsimd.memset(
            vtok[:].rearrange("p w s h o -> p (w s h) o")[:, :, HD:HD + 1], 1.0)

        # ---- q/k (feature-major, split by parity) + v (token-major) ----
        # PE program order is tuned so the in-order PE queue never waits on
        # slow producers: qk c0,c1 | v w0,w1 | qk c2,c3 | rel matmuls |
        # v w2..w6 (covers the rel-eviction drain) | pipelined core.
        _WHALF = [(0, 4), (4, 7)]
        with tc.tile_pool(name="qk_w", bufs=12) as wqk_sb, \
             tc.tile_pool(name="qk_stg", bufs=2) as stg_sb, \
             tc.tile_pool(name="qk_ps", bufs=3, space="PSUM") as qk_ps, \
             tc.tile_pool(name="v_ps", bufs=2, space="PSUM") as v_ps, \
             tc.tile_pool(name="rel_ps", bufs=3, space="PSUM") as rel_ps:
            wms = []
            for m in range(12):
                wm = wqk_sb.tile([128, JC, 128], f8, tag="wqk", name=f"w{m}")
                nc.sync.dma_start(
                    wm[:].rearrange("p j c -> p (j c)"), qkW[:, m, :])
                wms.append(wm)
            nc.sync.dma_start(wv[:], wvT)
            nc.sync.dma_start(wp[:], wpT)
            # one-hot key-position masks into khat rel rows; issued last on
            # the sync queue so x/weight transfers win the DMA device first
            for (msrc, mdst, tw, r0) in ((EhT, khE, 1, 0), (EwT, khE, 1, 32),
                                         (EhT, khB, 0, 0), (EwT, khB, 0, 32)):
                nc.sync.dma_start(
                    mdst[r0:r0 + WS, tw, :, 0:T].rearrange(
                        "p j (w n) -> p j w n", n=N),
                    msrc.rearrange("p (j w) n -> p j w n", w=NW))

            def emit_qk_chunk(ci):
                lo, hi = _NSL[ci]
                w = hi - lo
                # full-height evicts into per-chunk staging tiles (the bias
                # AP is per-partition, so one op covers both parity halves);
                # per chunk just 4 SBUF->SBUF DMAs distribute the halves
                qstg = stg_sb.tile([128, JC, 392], dt.float8e4, tag="qstg")
                kstg = stg_sb.tile([128, JC, 392], dt.float8e4, tag="kstg")
                for m in range(12):
                    is_q = m < JC
                    e = m % JC
                    pt = qk_ps.tile([128, 392], f32, tag="qk")
                    for jp in range(JC // 2):
                        nc.tensor.matmul(pt[:, :w],
                                         wms[m][:, 2 * jp:2 * jp + 2, :],
                                         xn[:, 2 * jp:2 * jp + 2, lo:hi],
                                         start=(jp == 0),
                                         stop=(jp == JC // 2 - 1),
                                         perf_mode=mybir.MatmulPerfMode.DoubleRow)
                    if is_q:
                        nc.scalar.activation(qstg[:, e, :w], pt[:, :w],
                                             AF.Identity, bias=bq[:, m:m + 1],
                                             scale=SCALE ** 0.5 / W8)
                    else:
                        nc.vector.tensor_scalar(
                            out=kstg[:, e, :w], in0=pt[:, :w],
                            scalar1=SCALE ** 0.5 / W8, scalar2=bk[:, e:e + 1],
                            op0=ALU.mult, op1=ALU.add)
                nc.sync.dma_start(qhE[:, 0, :, lo:hi], qstg[0:64, :, :w])
                nc.sync.dma_start(qhB[:, 1, :, lo:hi], qstg[64:128, :, :w])
                nc.sync.dma_start(khE[:, 0, :, lo:hi], kstg[0:64, :, :w])
                nc.sync.dma_start(khB[:, 1, :, lo:hi], kstg[64:128, :, :w])

            def emit_v(win):
                for si, (slo, shi) in enumerate(_SUBS):
                    ssz = shi - slo
                    base = win * N + slo
                    for half in range(2):
                        pv = v_ps.tile([128, 384], f32, tag="v")
                        for jp in range(JC // 2):
                            nc.tensor.matmul(
                                pv[:ssz, :],
                                xn[:, 2 * jp:2 * jp + 2, base:base + ssz],
                                wv[:, 2 * jp:2 * jp + 2,
                                   half * 384:(half + 1) * 384],
                                start=(jp == 0),
                                stop=(not with_vbias
                                      and jp == JC // 2 - 1),
                                perf_mode=mybir.MatmulPerfMode.DoubleRow)
                        if with_vbias:
                            nc.tensor.matmul(
                                pv[:ssz, :], onesT[:, :ssz], bv[:, half, :],
                                start=False, stop=True)
                        if (si + half) % 2 == 0:
                            nc.scalar.activation(
                                vtok[0:ssz, win, si,
                                     6 * half:6 * half + 6, 0:HD],
                                pv[:ssz, :].rearrange("p (h d) -> p h d",
                                                      d=HD),
                                AF.Copy, scale=1.0 / W8)
                        else:
                            nc.vector.tensor_scalar(
                                out=vtok[0:ssz, win, si,
                                         6 * half:6 * half + 6, 0:HD],
                                in0=pv[:ssz, :].rearrange(
                                    "p (h d) -> p h d", d=HD),
                                scalar1=1.0 / W8, scalar2=None, op0=ALU.mult)

            def emit_rel():
                # rel-pos rows into qhat planes, batched per h (rel_h) / per
                # w (rel_w); window halves keep PSUM cols <= 336.  Both
                # parities read q from partitions 0:64 of their q plane and
                # write rel rows 0:14 (rel_h) / 32:46 (rel_w) of the other.
                ri = 0
                for (w0, w1) in _WHALF:   # window halves outermost: the core
                    # can start on windows 0..3 while half 4..7 still drains
                    for par in range(2):
                        qh = qhE if par == 0 else qhB
                        qtw = 0 if par == 0 else 1       # q plane
                        rtw = 1 - qtw                    # rel plane
                        for typ in range(2):
                            r0 = 0 if typ == 0 else 32
                            stat = rh if typ == 0 else rw
                            for hh in range(WS):
                                nwn = w1 - w0
                                if typ == 0:
                                    mov = qh[:, qtw, :, 0:T].rearrange(
                                        "p j (win n) -> p j win n", n=N)[
                                        :, :, w0:w1, hh * WS:(hh + 1) * WS]
                                else:
                                    mov = qh[:, qtw, :, 0:T].rearrange(
                                        "p j (win kh kw) -> p j win kh kw",
                                        kh=WS, kw=WS)[:, :, w0:w1, :, hh]
                                ncols = JC * nwn * WS
                                prel = rel_ps.tile([128, 336], f32, tag="rel")
                                # skip_group_check: sim-only guard; its
                                # flat-address region view aliases across
                                # banks for <128-partition outputs
                                nc.tensor.matmul(
                                    prel[r0:r0 + WS, :ncols],
                                    stat[0:64, hh, :], mov,
                                    start=True, stop=True,
                                    tile_position=(0, r0),
                                    skip_group_check=True)
                                if typ == 0:
                                    dst = qh[r0:r0 + WS, rtw, :, 0:T].rearrange(
                                        "p j (win n) -> p j win n", n=N)[
                                        :, :, w0:w1, hh * WS:(hh + 1) * WS]
                                else:
                                    dst = qh[r0:r0 + WS, rtw, :, 0:T].rearrange(
                                        "p j (win kh kw) -> p j win kh kw",
                                        kh=WS, kw=WS)[:, :, w0:w1, :, hh]
                                src = prel[r0:r0 + WS, :ncols].rearrange(
                                    "p (j win k) -> p j win k", j=JC, win=nwn)
                                with nc.allow_low_precision(
                                        reason="fp8 rel rows; absolute score "
                                        "error ~3e-3 vs budget 0.1"):
                                    if ri % 2 == 0:
                                        nc.scalar.activation(
                                            dst, src, AF.Copy, scale=1.0 / W8)
                                    else:
                                        nc.vector.tensor_scalar(
                                            out=dst, in0=src,
                                            scalar1=1.0 / W8, scalar2=None,
                                            op0=ALU.mult)
                                ri += 1

            emit_qk_chunk(0)
            emit_qk_chunk(1)
            emit_v(0)
            emit_v(1)
            emit_qk_chunk(2)
            emit_qk_chunk(3)
            emit_rel()
            for win in range(2, NW):
                emit_v(win)

        # ---- attention core + proj: software-pipelined across windows ----
        # per iteration: AV+normalize(w) | scores+exp(w+1) | transp+proj(w);
        # window w+1's score matmuls keep PE busy while w's softmax
        # normalization drains on DVE.
        with tc.tile_pool(name="s_ps", bufs=1, space="PSUM") as s_ps, \
             tc.tile_pool(name="av_ps", bufs=2, space="PSUM") as av_ps, \
             tc.tile_pool(name="t_ps", bufs=2, space="PSUM") as t_ps, \
             tc.tile_pool(name="pj_ps", bufs=2, space="PSUM") as pj_ps, \
             tc.tile_pool(name="pt_sb", bufs=26) as pt_sb, \
             tc.tile_pool(name="ao_sb", bufs=2) as ao_sb, \
             tc.tile_pool(name="at_sb", bufs=2) as at_sb, \
             tc.tile_pool(name="xo_sb", bufs=2) as xo_sb, \
             tc.tile_pool(name="r_sb", bufs=4) as r_sb:
            # two persistent score tiles, rotated manually: the exp reads the
            # full [128, 392] tile, so the region no matmul covers (rows
            # 68:128 of the second key chunk) is zeroed exactly once
            sbufs = []
            for i in range(2):
                st = s_ps.tile([128, 392], f32, tag=f"s{i}", name=f"s{i}")
                # partition start must be 32-aligned on PSUM; rows 64:68 are
                # re-written by every second score matmul afterwards
                nc.vector.memset(st[64:128, 196:392], 0.0)
                sbufs.append(st)
            state = {"hidx": 0}

            def emit_scores(win):
                pts = []
                for head in range(HEADS):
                    blk = head // 2
                    par = head % 2
                    qh = qhE if par == 0 else qhB
                    kh = khE if par == 0 else khB
                    ps_t = sbufs[state["hidx"] % 2]
                    state["hidx"] += 1
                    for si, (slo, shi) in enumerate(_SUBS):
                        ssz = shi - slo
                        nc.tensor.matmul(
                            ps_t[:ssz, si * N:si * N + N],
                            kh[:, :, blk, win * N + slo:win * N + shi],
                            qh[:, :, blk, win * N:(win + 1) * N],
                            start=True, stop=True,
                            perf_mode=mybir.MatmulPerfMode.DoubleRow)
                    ptile = pt_sb.tile([128, 392], bf16, tag="pt")
                    nc.scalar.activation(ptile[:], ps_t[:], AF.Exp)
                    pts.append(ptile)
                return pts

            def emit_av(win, pts):
                ao0 = ao_sb.tile([128, DIM], bf16, tag="ao0")
                ao1 = ao_sb.tile([68, DIM], bf16, tag="ao1")
                for qi, (qlo, qhi) in enumerate(_SUBS):
                    qsz = qhi - qlo
                    ao = ao0 if qi == 0 else ao1
                    for hg in range(2):
                        pav = av_ps.tile([128, 6, HD + 1], f32, tag="av")
                        for hl in range(6):
                            head = hg * 6 + hl
                            for si, (slo, shi) in enumerate(_SUBS):
                                ssz = shi - slo
                                nc.tensor.matmul(
                                    pav[:qsz, hl, :],
                                    pts[head][0:ssz, si * N + qlo:si * N + qhi],
                                    vtok[0:ssz, win, si, head, :],
                                    start=(si == 0), stop=(si == 1))
                        rec = r_sb.tile([128, 6], f32, tag="rec")
                        nc.vector.reciprocal(
                            rec[:qsz, :],
                            pav[:qsz, :, HD:HD + 1].rearrange(
                                "p h o -> p (h o)"))
                        nc.vector.tensor_tensor(
                            out=ao[0:qsz, hg * 384:(hg + 1) * 384].rearrange(
                                "p (h d) -> p h d", d=HD),
                            in0=pav[:qsz, :, 0:HD],
                            in1=rec[:qsz, :].unsqueeze(2).to_broadcast(
                                [qsz, 6, HD]),
                            op=ALU.mult)
                return ao0, ao1

            def emit_transp_proj(win, ao0, ao1):
                aT = at_sb.tile([128, JC, NPAD], dt.float8e4, tag="at")
                ti = 0
                for j in range(JC):
                    for qi, (qlo, qhi) in enumerate(_SUBS):
                        qsz = qhi - qlo
                        src = ao0 if qi == 0 else ao1
                        ptt = t_ps.tile([128, 128], bf16, tag="tp")
                        nc.tensor.transpose(ptt[:, :qsz],
                                            src[0:qsz, j * 128:(j + 1) * 128],
                                            ident[0:qsz, 0:qsz])
                        if ti % 3 == 0:
                            nc.scalar.copy(aT[:, j, qlo:qhi], ptt[:, :qsz])
                        else:
                            nc.vector.tensor_copy(aT[:, j, qlo:qhi],
                                                  ptt[:, :qsz])
                        ti += 1
                xo_t = xo_sb.tile([128, JC, N], f32, tag="xo")
                for m in range(JC):
                    pp = pj_ps.tile([128, N], f32, tag="pj")
                    for jp in range(JC // 2):
                        nc.tensor.matmul(
                            pp[:], wp[:, 2 * jp:2 * jp + 2, m, :],
                            aT[:, 2 * jp:2 * jp + 2, 0:N],
                            start=(jp == 0), stop=(jp == JC // 2 - 1),
                            perf_mode=mybir.MatmulPerfMode.DoubleRow)
                    nc.vector.tensor_scalar(
                        out=xo_t[:, m, :], in0=pp[:],
                        scalar1=1.0 / W8, scalar2=pb[:, m:m + 1],
                        op0=ALU.mult, op1=ALU.add)
                nc.sync.dma_start(
                    xoT.rearrange("p j (w n) -> p j w n", n=N)[:, :, win, :],
                    xo_t[:])

            pts = emit_scores(0)
            for win in range(NW):
                ao0, ao1 = emit_av(win, pts)
                if win + 1 < NW:
                    pts = emit_scores(win + 1)
                emit_transp_proj(win, ao0, ao1)
    nc.compile()
    return nc


def build_mlp():
    """Dispatch B: y = x + fc2(gelu(fc1(LN2(x)))), 1024 tokens/core.
    LN2 affine is absorbed into fc1 host-side."""
    nc = bacc.Bacc("TRN2", target_bir_lowering=False, debug=False)
    f32, bf16 = dt.float32, dt.bfloat16

    xT = nc.dram_tensor("xT", [128, JC, TB], bf16, kind="ExternalInput").ap()
    f8 = dt.float8e4
    fc1W = nc.dram_tensor("fc1W", [128, HC, DIM], f8, kind="ExternalInput").ap()
    fc1S = nc.dram_tensor("fc1S", [128, HC], f32, kind="ExternalInput").ap()
    fc2W = nc.dram_tensor("fc2W", [128, HC, DIM], bf16, kind="ExternalInput").ap()
    fc1B = nc.dram_tensor("fc1B", [128, HC], f32, kind="ExternalInput").ap()
    fc2B = nc.dram_tensor("fc2B", [128, JC], f32, kind="ExternalInput").ap()
    yT = nc.dram_tensor("yT", [128, JC, TB], f32, kind="ExternalOutput").ap()

    with tile.TileContext(nc) as tc, ExitStack() as ctx:
        const = ctx.enter_context(tc.tile_pool(name="const", bufs=1))
        big = ctx.enter_context(tc.tile_pool(name="big", bufs=1))
        lnrows = ctx.enter_context(tc.tile_pool(name="lnrows", bufs=1))

        ones1 = const.tile([128, 1], bf16)
        nc.vector.memset(ones1[:], 1.0)
        onesP1 = const.tile([1, 128], bf16)
        nc.vector.memset(onesP1[:], 1.0)
        b1t = const.tile([128, HC], f32)
        nc.scalar.dma_start(b1t[:], fc1B)
        s1t = const.tile([128, HC], f32)
        nc.scalar.dma_start(s1t[:], fc1S)
        b2t = const.tile([128, JC], f32)
        nc.scalar.dma_start(b2t[:], fc2B)

        xtiles = big.tile([128, JC, TB], bf16)
        xn = big.tile([128, JC, TB], dt.float8e4)
        h = big.tile([128, HC, TB], bf16)

        with tc.tile_pool(name="w1_sb", bufs=HC) as w1p, \
             tc.tile_pool(name="w2_sb", bufs=1) as w2p:
            # x chunks first on the sync queue -> served first by the DMA
            # device; weights follow in need order (w1 chunks, then w2)
            for lo, hi in _NSL_B:
                nc.sync.dma_start(xtiles[:, :, lo:hi], xT[:, :, lo:hi])
            w1s = []
            for m in range(HC):
                w1m = w1p.tile([128, JC, 128], f8, tag="w1", name=f"w1_{m}")
                nc.sync.dma_start(
                    w1m[:].rearrange("p j c -> p (j c)"), fc1W[:, m, :])
                w1s.append(w1m)
            w2t = w2p.tile([128, HC, DIM], bf16)
            nc.sync.dma_start(w2t[:], fc2W)

            def load_chunk(ci, lo, hi):
                return xtiles[:, :, lo:hi]

            _standardize(nc, tc, _NSL_B, JC, load_chunk, xn, ones1, onesP1,
                         lnrows, bf16_in=True)

            with tc.tile_pool(name="f1_ps", bufs=4, space="PSUM") as f1_ps, \
                 tc.tile_pool(name="f2_ps", bufs=3, space="PSUM") as f2_ps, \
                 tc.tile_pool(name="out_sb", bufs=3) as out_sb:
                for ci, (lo, hi) in enumerate(_FC_B):
                    w = hi - lo
                    for m in range(HC):
                        pt = f1_ps.tile([128, w], f32, tag="f1")
                        for jp in range(JC // 2):
                            nc.tensor.matmul(
                                pt[:], w1s[m][:, 2 * jp:2 * jp + 2, :],
                                xn[:, 2 * jp:2 * jp + 2, lo:hi],
                                start=(jp == 0), stop=(jp == JC // 2 - 1),
                                perf_mode=mybir.MatmulPerfMode.DoubleRow)
                        # per-out-channel fp8 descale via the activation
                        # scale AP; bias applies after the scale
                        nc.scalar.activation(h[:, m, lo:hi], pt[:], AF.Gelu,
                                             bias=b1t[:, m:m + 1],
                                             scale=s1t[:, m:m + 1])
                for ci, (lo, hi) in enumerate(_FC_B):
                    w = hi - lo
                    for m in range(JC):
                        pt = f2_ps.tile([128, w], f32, tag="f2")
                        for j in range(HC):
                            nc.tensor.matmul(pt[:], w2t[:, j, m * 128:(m + 1) * 128],
                                             h[:, j, lo:hi],
                                             start=(j == 0), stop=(j == HC - 1))
                        ot = out_sb.tile([128, w], f32, tag="out")
                        # ot = (psum + fc2_b) + x   in one DVE pass
                        nc.vector.scalar_tensor_tensor(
                            out=ot[:], in0=pt[:], scalar=b2t[:, m:m + 1],
                            in1=xtiles[:, m, lo:hi], op0=ALU.add, op1=ALU.add)
                        nc.sync.dma_start(yT[:, m, lo:hi], ot[:])
    nc.compile()
    return nc


# ---------------- host glue ----------------

_CACHE = {}


def _get(name, builder):
    if name not in _CACHE:
        _CACHE[name] = builder()
    return _CACHE[name]


def _featmajor(a):
    """(T, 768) fp32 -> [128, 6, T]"""
    Tn = a.shape[0]
    return np.ascontiguousarray(a.T.reshape(JC, 128, Tn).transpose(1, 0, 2))


def _unfeat(aT):
    """[128, 6, T] -> (T, 768)"""
    return np.asarray(aT).transpose(1, 0, 2).reshape(DIM, -1).T


def _chunkvec(v):
    """(c*128,) -> [128, c] fp32"""
    v = np.asarray(v, np.float32)
    return np.ascontiguousarray(v.reshape(-1, 128).T)


def _wchunk(w, nchunk, dtype=BF16):
    """(768, nchunk*128) weight -> [128, nchunk, 768] (m-major chunks:
    out[p, m, j*128 + c] = w[j*128 + p, m*128 + c])."""
    w = np.asarray(w, np.float32)
    kin = w.shape[0] // 128
    out = w.reshape(kin, 128, nchunk, 128).transpose(1, 2, 0, 3)
    return np.ascontiguousarray(out.reshape(128, nchunk, kin * 128)).astype(dtype)


def _bf16(a):
    return np.asarray(a, dtype=BF16)


def _build_rel(rel_pos, ws=WS):
    """[64, 14, 14] fp8: out[c, h, k] = rel_pos[idx[h,k], c] * W8 / sqrt(SCALE)
    (qhat holds sqrt(SCALE)*q; W8 prescale is undone at the rel eviction)."""
    idx = np.arange(ws)[:, None] - np.arange(ws)[None, :] + (ws - 1)
    R = np.asarray(rel_pos, np.float32)[idx] * (W8 / SCALE ** 0.5)
    return R.transpose(2, 0, 1).astype(F8)


def _build_onehots():
    """Eh[r, :, k] = 1 if k//14 == r;  Ew[r, :, k] = 1 if k%14 == r,
    pre-expanded over the (j, win) axis for big contiguous DMA runs."""
    k = np.arange(N)
    Eh = (k[None, :] // WS == np.arange(WS)[:, None]).astype(np.float32)
    Ew = (k[None, :] % WS == np.arange(WS)[:, None]).astype(np.float32)
    Eh = np.ascontiguousarray(np.broadcast_to(Eh[:, None, :], (WS, JC * NW, N)))
    Ew = np.ascontiguousarray(np.broadcast_to(Ew[:, None, :], (WS, JC * NW, N)))
    return Eh.astype(F8), Ew.astype(F8)


kernel_last_perf = {}

try:
    from antenv.axon_hooks import get_axon_ntff_profile_hook as _hook  # noqa: F401
    _HAVE_TRACE = True
except ImportError:
    _HAVE_TRACE = False
    import os as _os
    _os.environ["BASS_NEVER_TRACE"] = "1"   # bass_utils re-reads BASS_TRACE


def window_x(x):
    """(2, 64, 64, 768) -> (56, 196, 768) padded window tokens."""
    B, H, W, C = x.shape
    xp = np.zeros((B, 70, 70, C), np.float32)
    xp[:, :64, :64] = x
    xw = xp.reshape(B, 5, WS, 5, WS, C).transpose(0, 1, 3, 2, 4, 5).reshape(50, N, C)
    xall = np.zeros((56, N, C), np.float32)
    xall[:50] = xw
    return xall


def attn_consts(norm1_w, norm1_b, qkv_w, qkv_b, proj_w, proj_b,
                rel_pos_h, rel_pos_w):
    """Host-side constant tensors for dispatch A (LN1 affine absorbed)."""
    n1w = np.asarray(norm1_w, np.float32)
    n1b = np.asarray(norm1_b, np.float32)
    qkvw = np.asarray(qkv_w, np.float32)
    qkvb = np.asarray(qkv_b, np.float32)
    Wq = n1w[:, None] * qkvw                 # (768, 2304)
    bfull = n1b @ qkvw + qkvb                # (2304,)
    Eh, Ew = _build_onehots()
    return {
        "qkW": _wchunk(Wq[:, 0:2 * DIM] * W8, 12, F8),
        "wvT": np.ascontiguousarray(
            Wq[:, 2 * DIM:].reshape(JC, 128, DIM).transpose(1, 0, 2)
            * W8).astype(F8),
        "wpT": np.ascontiguousarray(
            np.asarray(proj_w, np.float32).reshape(JC, 128, JC, 128)
            .transpose(1, 0, 2, 3) * W8).astype(F8),
        "bqT": _chunkvec(bfull[0:DIM] * SCALE ** 0.5),
        "bkT": _chunkvec(bfull[DIM:2 * DIM] * SCALE ** 0.5),
        "bvT": _bf16(bfull[2 * DIM:].reshape(1, 2, 384) * W8),
        "pbT": _chunkvec(proj_b),
        "RhT": _build_rel(rel_pos_h),
        "RwT": _build_rel(rel_pos_w),
        "EhT": Eh,
        "EwT": Ew,
    }


def mlp_consts(norm2_w, norm2_b, fc1_w, fc1_b, fc2_w, fc2_b):
    """Host-side constant tensors for dispatch B (LN2 affine absorbed)."""
    n2w = np.asarray(norm2_w, np.float32)
    n2b = np.asarray(norm2_b, np.float32)
    f1w = np.asarray(fc1_w, np.float32)
    W1 = n2w[:, None] * f1w                  # (768, 3072)
    b1 = n2b @ f1w + np.asarray(fc1_b, np.float32)
    # per-out-channel power-of-2 fp8 scaling for fc1 (exactly undone by the
    # gelu activation's per-partition scale AP)
    colmax = np.abs(W1).max(axis=0)                        # (3072,)
    sexp = np.clip(np.floor(np.log2(224.0 / np.maximum(colmax, 1e-30))),
                   -20, 20)
    wscale = np.exp2(sexp)                                 # (3072,)
    return {
        "fc1W": _wchunk(W1 * wscale[None, :], HC, F8),
        "fc1S": _chunkvec(1.0 / wscale),
        "fc2W": _bf16(np.ascontiguousarray(
            np.asarray(fc2_w, np.float32).reshape(HC, 128, DIM)
            .transpose(1, 0, 2))),
        "fc1B": _chunkvec(b1),
        "fc2B": _chunkvec(fc2_b),
    }


def kernel(x, norm1_w, norm1_b, qkv_w, qkv_b, proj_w, proj_b,
           rel_pos_h, rel_pos_w, norm2_w, norm2_b,
           fc1_w, fc1_b, fc2_w, fc2_b):
    import os
    trace = bool(os.environ.get("BASS_TRACE")) and _HAVE_TRACE
    x = np.asarray(x, np.float32)
    B, H, W, C = x.shape
    assert (B, H, W, C) == (2, 64, 64, DIM)

    # ---- dispatch A: windowed attention ----
    xall = window_x(x)
    consts_a = attn_consts(norm1_w, norm1_b, qkv_w, qkv_b, proj_w, proj_b,
                           rel_pos_h, rel_pos_w)
    with_vbias = bool(np.any(np.asarray(consts_a["bvT"], np.float32)))
    nc_a = _get(f"attn{int(with_vbias)}",
                lambda: build_attn(with_vbias=with_vbias))
    in_maps = []
    for c in range(NCORES):
        m = dict(consts_a)
        m["xT"] = _featmajor(
            xall[c * NW:(c + 1) * NW].reshape(T, C)).astype(BF16)
        in_maps.append(m)
    res_a = run_bass_kernel_spmd(nc_a, in_maps, core_ids=list(range(NCORES)),
                                 trace=trace)
    kernel_last_perf["attn"] = res_a.exec_time_ns
    xo_all = np.stack([_unfeat(res_a.results[c]["xoT"]) for c in range(NCORES)])
    xo = xo_all.reshape(56, N, C)[:50]
    xo = xo.reshape(B, 5, 5, WS, WS, C).transpose(0, 1, 3, 2, 4, 5).reshape(B, 70, 70, C)
    x2 = x + xo[:, :64, :64]

    # ---- dispatch B: MLP ----
    nc_b = _get("mlp", build_mlp)
    consts_b = mlp_consts(norm2_w, norm2_b, fc1_w, fc1_b, fc2_w, fc2_b)
    x2f = np.ascontiguousarray(x2.reshape(B * H * W, C))
    in_maps = []
    for c in range(NCORES):
        m = dict(consts_b)
        m["xT"] = _featmajor(x2f[c * TB:(c + 1) * TB]).astype(BF16)
        in_maps.append(m)
    res_b = run_bass_kernel_spmd(nc_b, in_maps, core_ids=list(range(NCORES)),
                                 trace=trace)
    kernel_last_perf["mlp"] = res_b.exec_time_ns
    y = np.concatenate([_unfeat(res_b.results[c]["yT"]) for c in range(NCORES)])
    return y.reshape(B, H, W, C).astype(np.float32)


# revision 54
# speedup vs baseline: 1.1578x; 1.0034x over previous
"""Trainium2 Bass kernel for a SAM/ViTDet-style windowed-attention transformer
block (DIM=768, 12 heads, window 14, decomposed rel-pos bias, exact-gelu MLP).

Contract: kernel(**inputs) takes the FULL unsharded inputs from
reference.setup_inputs() and returns the FULL (2, 64, 64, 768) float32 output.

Strategy (8 NeuronCores, SPMD, data-parallel):
  Dispatch A (attention): shard the 50 real windows (padded to 56) as 7
    windows/core. Per core: LN1 -> qkv -> windowed attention with the
    decomposed rel-pos bias folded into an augmented-key matmul -> proj.
  Host: window-unpartition, crop, residual add.
  Dispatch B (MLP): shard the 8192 tokens as 1024/core. Per core:
    LN2 -> fc1 -> exact GELU -> fc2 -> residual.

Perf notes (v2):
  * The LN affine (w, b) is absorbed host-side into the following matmul
    weights/biases, so on-device LN is a pure standardize: bf16 stats
    matmuls + bf16 broadcast tiles + two 4x-rate DVE tensor_tensor ops.
  * Rel-pos rows are produced by per-(h or w) batched matmuls (112 instead
    of 392) whose PSUM outputs land at partition bases 0/32/64/96, cutting
    eviction traffic.
  * The two score matmuls of one (window, head) share a [128, 392] PSUM
    tile -> a single exp instruction per head.
  * AV outputs are 6-head-batched in PSUM; the softmax normalization is one
    DVE multiply with a stride-0 (broadcast) reciprocal operand.
  * Weight DMAs are chunked and issued up front so compute starts ~10us in.

Augmented-key rel-pos layout (q/k head-pair blocks, per j-chunk):
  even head: q 0:64,  rel_h 64:78,  zeros 78:96, rel_w 96:110, zeros 110:128
  odd head:  rel_h 0:14, zeros 14:32, rel_w 32:46, zeros 46:64, q 64:128
khat holds k values in the q rows and one-hot key-position masks in the rel
rows; zero gaps make the extra contraction rows inert. S^T = khat^T qhat then
includes the decomposed bias exactly.
"""

import sys

sys.path.insert(0, "/opt/trn_rl_repo")

from contextlib import ExitStack

import numpy as np
import ml_dtypes

import concourse.bacc as bacc
import concourse.mybir as mybir
import concourse.tile as tile
from concourse.bass_utils import run_bass_kernel_spmd
from concourse.masks import make_identity

dt = mybir.dt
AF = mybir.ActivationFunctionType
ALU = mybir.AluOpType

DIM = 768
HEADS = 12
HD = 64
WS = 14
N = WS * WS          # 196 tokens / window
NW = 7               # windows per core
T = NW * N           # 1372 token slots per core (dispatch A)
TB = 1024            # tokens per core (dispatch B)
MLP = 3072
NCORES = 8
JC = DIM // 128      # 6 feature chunks
HC = MLP // 128      # 24 hidden chunks
EPS = 1e-5
SCALE = HD ** -0.5   # 0.125
TPAD = 1376          # T rounded up so fp8 DoubleRow pair strides are 16B-aligned
NPAD = 208           # N rounded up likewise (aT)
BF16 = ml_dtypes.bfloat16
F8 = ml_dtypes.float8_e4m3
W8 = 64.0   # fp8 weight pre-scale (avoids e4m3 subnormals); undone at eviction

# window-aligned token chunks for dispatch A (2+2+2+1 windows)
_NSL = [(0, 392), (392, 784), (784, 1176), (1176, 1372)]
_NSL_LN = [(i * N, (i + 1) * N) for i in range(NW)]        # LN chunks, A
_NSL_B = [(0, 256), (256, 512), (512, 768), (768, 1024)]   # LN chunks, B
_FC_B = [(0, 512), (512, 1024)]                            # matmul chunks, B
_SUBS = [(0, 128), (128, 196)]                             # within-window subchunks

# augmented-key row layout per parity: (q_lo, relh_lo, relw_lo)
_EVEN = (0, 64, 96)    # q 0:64,  rel rows above
_ODD = (64, 0, 32)     # q 64:128, rel rows below


def _standardize(nc, tc, nsl_list, jc, load_chunk, xn, ones1, onesP1, rows_p,
                 cast_engine="gpsimd", bf16_in=False):
    """Pure LN standardize: xn[:, j, c] = (x - mu[c]) * rsig[c], bf16 out.

    load_chunk(ci, lo, hi) -> AP [128, jc, w] for that chunk (may DMA into a
    fresh tile or return a view of a resident one); fp32 unless bf16_in (then
    it is used directly, no cast).  Stats run as bf16 matmuls vs a ones
    vector; mu/rsig are kept as bf16 rows, broadcast across partitions via
    tiny bf16 matmuls, evicted to bf16 SBUF and applied with two DVE
    tensor_tensor ops (all-bf16, stride-1 -> DVE fast mode).
    ones1: [128,1] bf16 ones; onesP1: [1,128] bf16 ones; rows_p: pool for rows.
    """
    nch = len(nsl_list)
    dimn = jc * 128

    eng_cast = getattr(nc, cast_engine)

    epsr = rows_p.tile([1, 1], dt.float32, tag="epsr")
    nc.vector.memset(epsr[:], EPS)

    with tc.tile_pool(name="ln_xb", bufs=nch) as xbp, \
         tc.tile_pool(name="ln_sq", bufs=2) as sqp, \
         tc.tile_pool(name="ln_st", bufs=2, space="PSUM") as st_ps, \
         tc.tile_pool(name="ln_bc", bufs=2, space="PSUM") as bc_ps, \
         tc.tile_pool(name="ln_bcs", bufs=4) as bcs, \
         tc.tile_pool(name="ln_rows", bufs=nch + 1) as rp:
        # per-chunk stats emitted with the apply of the PREVIOUS chunk
        # interleaved (one-chunk lag): PE runs stats back to back while the
        # row math / broadcast / apply of the prior chunk drains on Act/DVE
        xbs, mus, rss = [], [], []

        def emit_stats(ci, lo, hi):
            w = hi - lo
            xt = load_chunk(ci, lo, hi)
            if bf16_in:
                xb = xt
            else:
                xb = xbp.tile([128, jc, w], dt.bfloat16, tag="xb",
                              name=f"xb{ci}")
                eng_cast.tensor_copy(xb[:], xt)
                xb = xb[:]
            xbs.append(xb)
            # per-token sums -> mu
            pmu = st_ps.tile([1, w], dt.float32, tag="st")
            for j in range(jc):
                nc.tensor.matmul(pmu[:], ones1[:], xb[:, j, :],
                                 start=(j == 0), stop=(j == jc - 1))
            mu = rp.tile([1, w], dt.bfloat16, tag="mu", name=f"mu{ci}")
            nc.scalar.activation(mu[:], pmu[:], AF.Copy, scale=1.0 / dimn)
            mus.append(mu)
            # per-token sum of squares -> E[x^2]
            sq = sqp.tile([128, jc, w], dt.bfloat16, tag="sq")
            nc.vector.tensor_tensor(out=sq[:], in0=xb, in1=xb, op=ALU.mult)
            pmq = st_ps.tile([1, w], dt.float32, tag="st")
            for j in range(jc):
                nc.tensor.matmul(pmq[:], ones1[:], sq[:, j, :],
                                 start=(j == 0), stop=(j == jc - 1))
            mq = rp.tile([1, w], dt.float32, tag="mq")
            nc.scalar.activation(mq[:], pmq[:], AF.Copy, scale=1.0 / dimn)
            # rsig = 1/sqrt(E[x^2] - mu^2 + eps)
            m2 = rp.tile([1, w], dt.float32, tag="m2")
            nc.vector.tensor_tensor(out=m2[:], in0=mu[:], in1=mu[:],
                                    op=ALU.mult)
            nc.vector.tensor_tensor(out=mq[:], in0=mq[:], in1=m2[:],
                                    op=ALU.subtract)
            sd = rp.tile([1, w], dt.float32, tag="sd")
            nc.scalar.activation(sd[:], mq[:], AF.Sqrt, bias=epsr[:])
            rsig = rp.tile([1, w], dt.bfloat16, tag="rs", name=f"rs{ci}")
            with nc.allow_low_precision(reason="bf16 rsig row; 0.4% rel err "
                                        "matches the bf16 matmul noise "
                                        "floor"):
                nc.vector.reciprocal(rsig[:], sd[:])
            rss.append(rsig)

        def emit_apply(ci, lo, hi):
            w = hi - lo
            xb, mu, rsig = xbs[ci], mus[ci], rss[ci]
            bmu_p = bc_ps.tile([128, w], dt.float32, tag="bc")
            nc.tensor.matmul(bmu_p[:], onesP1[:], mu[:], start=True, stop=True)
            brs_p = bc_ps.tile([128, w], dt.float32, tag="bc")
            nc.tensor.matmul(brs_p[:], onesP1[:], rsig[:], start=True,
                             stop=True)
            bmu = bcs.tile([128, w], dt.bfloat16, tag="bmu")
            nc.scalar.copy(bmu[:], bmu_p[:])
            brs = bcs.tile([128, w], dt.bfloat16, tag="brs")
            nc.scalar.copy(brs[:], brs_p[:])
            # xn = (x - mu) * rsig   (two all-bf16 DVE ops, j-broadcast)
            cen = sqp.tile([128, jc, w], dt.bfloat16, tag="cen")
            nc.vector.tensor_tensor(
                out=cen[:], in0=xb,
                in1=bmu[:].unsqueeze(1).to_broadcast([128, jc, w]),
                op=ALU.subtract)
            with nc.allow_low_precision(reason="xn storage dtype (bf16/fp8) "
                                        "is the matmul operand precision"):
                nc.vector.tensor_tensor(
                    out=xn[:, :, lo:hi], in0=cen[:],
                    in1=brs[:].unsqueeze(1).to_broadcast([128, jc, w]),
                    op=ALU.mult)

        # chunk 0's apply is emitted right after its stats so the first xn
        # chunk (the qk-phase gate) is produced as early as possible
        emit_stats(0, *nsl_list[0])
        emit_apply(0, *nsl_list[0])
        for ci in range(1, nch):
            emit_stats(ci, *nsl_list[ci])
        for ci in range(1, nch):
            emit_apply(ci, *nsl_list[ci])


def build_attn(with_vbias=True):
    """Dispatch A: LN1 + qkv + windowed attention (+rel-pos) + proj."""
    nc = bacc.Bacc("TRN2", target_bir_lowering=False, debug=False)
    f32, bf16 = dt.float32, dt.bfloat16

    xT = nc.dram_tensor("xT", [128, JC, T], bf16, kind="ExternalInput").ap()
    f8 = dt.float8e4
    qkW = nc.dram_tensor("qkW", [128, 12, JC * 128], f8, kind="ExternalInput").ap()
    wvT = nc.dram_tensor("wvT", [128, JC, DIM], f8, kind="ExternalInput").ap()
    wpT = nc.dram_tensor("wpT", [128, JC, JC, 128], f8, kind="ExternalInput").ap()
    bqT = nc.dram_tensor("bqT", [128, JC], f32, kind="ExternalInput").ap()
    bkT = nc.dram_tensor("bkT", [128, JC], f32, kind="ExternalInput").ap()
    bvT = nc.dram_tensor("bvT", [1, 2, 384], bf16, kind="ExternalInput").ap()
    pbT = nc.dram_tensor("pbT", [128, JC], f32, kind="ExternalInput").ap()
    RhT = nc.dram_tensor("RhT", [64, WS, WS], f8, kind="ExternalInput").ap()
    RwT = nc.dram_tensor("RwT", [64, WS, WS], f8, kind="ExternalInput").ap()
    EhT = nc.dram_tensor("EhT", [WS, JC * NW, N], f8, kind="ExternalInput").ap()
    EwT = nc.dram_tensor("EwT", [WS, JC * NW, N], f8, kind="ExternalInput").ap()
    xoT = nc.dram_tensor("xoT", [128, JC, T], f32, kind="ExternalOutput").ap()

    with tile.TileContext(nc) as tc, ExitStack() as ctx:
        const = ctx.enter_context(tc.tile_pool(name="const", bufs=1))
        big = ctx.enter_context(tc.tile_pool(name="big", bufs=1))
        lnrows = ctx.enter_context(tc.tile_pool(name="lnrows", bufs=1))

        # ---- big persistent tensors (declared first so memsets start at t=0)
        # qhat/khat live in a partition-paired fp8 layout [64, two, j, t]:
        # logical contraction row r maps to (r % 64, r // 64), so the score
        # matmuls run fp8 DoubleRow.  Plane assignment per parity:
        #   even head: q = plane 0, rel_h rows 0:14 / rel_w 32:46 on plane 1
        #   odd head:  q = plane 1, rel_h rows 0:14 / rel_w 32:46 on plane 0
        xn = big.tile([128, JC, TPAD], dt.float8e4)
        qhE = big.tile([64, 2, JC, TPAD], dt.float8e4)
        khE = big.tile([64, 2, JC, TPAD], dt.float8e4)
        qhB = big.tile([64, 2, JC, TPAD], dt.float8e4)
        khB = big.tile([64, 2, JC, TPAD], dt.float8e4)
        vtok = big.tile([128, NW, 2, HEADS, HD + 1], bf16)

        # ---- constants ----
        ones1 = const.tile([128, 1], bf16)
        nc.vector.memset(ones1[:], 1.0)
        onesP1 = const.tile([1, 128], bf16)
        nc.vector.memset(onesP1[:], 1.0)
        onesT = const.tile([1, 128], bf16)
        nc.vector.memset(onesT[:], 1.0)
        ident = const.tile([128, 128], bf16)
        make_identity(nc, ident[:])

        rh = const.tile([64, WS, WS], f8)
        nc.scalar.dma_start(rh[:], RhT)
        rw = const.tile([64, WS, WS], f8)
        nc.scalar.dma_start(rw[:], RwT)
        bq = const.tile([128, JC], f32)
        nc.scalar.dma_start(bq[:], bqT)
        bk = const.tile([128, JC], f32)
        nc.scalar.dma_start(bk[:], bkT)
        bv = const.tile([1, 2, 384], bf16)
        nc.scalar.dma_start(bv[:], bvT)
        pb = const.tile([128, JC], f32)
        nc.scalar.dma_start(pb[:], pbT)
        # wv/wp tiles are created here but their loads are issued on the sync
        # queue after the x/qk-weight DMAs so the global DMA device serves x
        # first (sync-queue program order == DMA device order).
        wv = const.tile([128, JC, DIM], f8)
        wp = const.tile([128, JC, JC, 128], f8)

        # ---- LN1 (pure standardize; affine absorbed into weights) ----
        # x arrives already bf16 (host cast) -> no on-device cast, half DMA;
        # window-sized chunks get the first xn out early for the qk start
        with tc.tile_pool(name="ln_x", bufs=4) as xp:
            def load_chunk(ci, lo, hi):
                xt = xp.tile([128, JC, hi - lo], bf16, tag="x", name=f"x{ci}")
                nc.sync.dma_start(xt[:], xT[:, :, lo:hi])
                return xt[:]
            _standardize(nc, tc, _NSL, JC, load_chunk, xn, ones1, onesP1,
                         lnrows, bf16_in=True)

        # zero the rel/one-hot halves (gaps must be exactly 0; rel rows and
        # one-hot rows overlay these ranges later).  Issued after the LN body
        # so the Pool queue serves the LN casts first; the Tile deps still
        # order these before the mask DMAs / rel evictions below.
        nc.gpsimd.memset(khE[:, 1, :, :], 0.0)
        nc.gpsimd.memset(khB[:, 0, :, :], 0.0)
        nc.gpsimd.memset(qhE[:, 1, :, :], 0.0)
        nc.gpsimd.memset(qhB[:, 0, :, :], 0.0)
        # ones column in vtok (AV matmul also yields the softmax denominator)
        nc.gpsimd.memset(
            vtok[:].rearrange("p w s h o -> p (w s h) o")[:, :, HD:HD + 1], 1.0)

        # ---- q/k (feature-major, split by parity) + v (token-major) ----
        # PE program order is tuned so the in-order PE queue never waits on
        # slow producers: qk c0,c1 | v w0,w1 | qk c2,c3 | rel matmuls |
        # v w2..w6 (covers the rel-eviction drain) | pipelined core.
        _WHALF = [(0, 4), (4, 7)]
        with tc.tile_pool(name="qk_w", bufs=12) as wqk_sb, \
             tc.tile_pool(name="qk_stg", bufs=2) as stg_sb, \
             tc.tile_pool(name="qk_ps", bufs=2, space="PSUM") as qk_ps, \
             tc.tile_pool(name="v_ps", bufs=2, space="PSUM") as v_ps, \
             tc.tile_pool(name="rel_ps", bufs=4, space="PSUM") as rel_ps:
            wms = []
            for m in range(12):
                wm = wqk_sb.tile([128, JC, 128], f8, tag="wqk", name=f"w{m}")
                nc.sync.dma_start(
                    wm[:].rearrange("p j c -> p (j c)"), qkW[:, m, :])
                wms.append(wm)
            nc.sync.dma_start(wv[:], wvT)
            nc.sync.dma_start(wp[:], wpT)
            # one-hot key-position masks into khat rel rows; issued last on
            # the sync queue so x/weight transfers win the DMA device first
            for (msrc, mdst, tw, r0) in ((EhT, khE, 1, 0), (EwT, khE, 1, 32),
                                         (EhT, khB, 0, 0), (EwT, khB, 0, 32)):
                nc.sync.dma_start(
                    mdst[r0:r0 + WS, tw, :, 0:T].rearrange(
                        "p j (w n) -> p j w n", n=N),
                    msrc.rearrange("p (j w) n -> p j w n", w=NW))

            def emit_qk_chunk(ci):
                lo, hi = _NSL[ci]
                w = hi - lo
                # full-height evicts into per-chunk staging tiles (the bias
                # AP is per-partition, so one op covers both parity halves);
                # per chunk just 4 SBUF->SBUF DMAs distribute the halves
                qstg = stg_sb.tile([128, JC, 392], dt.float8e4, tag="qstg")
                kstg = stg_sb.tile([128, JC, 392], dt.float8e4, tag="kstg")
                for m in range(12):
                    is_q = m < JC
                    e = m % JC
                    pt = qk_ps.tile([128, 392], f32, tag="qk")
                    for jp in range(JC // 2):
                        nc.tensor.matmul(pt[:, :w],
                                         wms[m][:, 2 * jp:2 * jp + 2, :],
                                         xn[:, 2 * jp:2 * jp + 2, lo:hi],
                                         start=(jp == 0),
                                         stop=(jp == JC // 2 - 1),
                                         perf_mode=mybir.MatmulPerfMode.DoubleRow)
                    if is_q:
                        nc.scalar.activation(qstg[:, e, :w], pt[:, :w],
                                             AF.Identity, bias=bq[:, m:m + 1],
                                             scale=SCALE ** 0.5 / W8)
                    else:
                        nc.vector.tensor_scalar(
                            out=kstg[:, e, :w], in0=pt[:, :w],
                            scalar1=SCALE ** 0.5 / W8, scalar2=bk[:, e:e + 1],
                            op0=ALU.mult, op1=ALU.add)
                nc.sync.dma_start(qhE[:, 0, :, lo:hi], qstg[0:64, :, :w])
                nc.sync.dma_start(qhB[:, 1, :, lo:hi], qstg[64:128, :, :w])
                nc.sync.dma_start(khE[:, 0, :, lo:hi], kstg[0:64, :, :w])
                nc.sync.dma_start(khB[:, 1, :, lo:hi], kstg[64:128, :, :w])

            def emit_v(win):
                for si, (slo, shi) in enumerate(_SUBS):
                    ssz = shi - slo
                    base = win * N + slo
                    for half in range(2):
                        pv = v_ps.tile([128, 384], f32, tag="v")
                        for jp in range(JC // 2):
                            nc.tensor.matmul(
                                pv[:ssz, :],
                                xn[:, 2 * jp:2 * jp + 2, base:base + ssz],
                                wv[:, 2 * jp:2 * jp + 2,
                                   half * 384:(half + 1) * 384],
                                start=(jp == 0),
                                stop=(not with_vbias
                                      and jp == JC // 2 - 1),
                                perf_mode=mybir.MatmulPerfMode.DoubleRow)
                        if with_vbias:
                            nc.tensor.matmul(
                                pv[:ssz, :], onesT[:, :ssz], bv[:, half, :],
                                start=False, stop=True)
                        if (si + half) % 2 == 0:
                            nc.scalar.activation(
                                vtok[0:ssz, win, si,
                                     6 * half:6 * half + 6, 0:HD],
                                pv[:ssz, :].rearrange("p (h d) -> p h d",
                                                      d=HD),
                                AF.Copy, scale=1.0 / W8)
                        else:
                            nc.vector.tensor_scalar(
                                out=vtok[0:ssz, win, si,
                                         6 * half:6 * half + 6, 0:HD],
                                in0=pv[:ssz, :].rearrange(
                                    "p (h d) -> p h d", d=HD),
                                scalar1=1.0 / W8, scalar2=None, op0=ALU.mult)

            def emit_rel():
                # rel-pos rows into qhat planes, batched per h (rel_h) / per
                # w (rel_w); window halves keep PSUM cols <= 336.  Both
                # parities read q from partitions 0:64 of their q plane and
                # write rel rows 0:14 (rel_h) / 32:46 (rel_w) of the other.
                ri = 0
                for (w0, w1) in _WHALF:   # window halves outermost: the core
                    # can start on windows 0..3 while half 4..7 still drains
                    for par in range(2):
                        qh = qhE if par == 0 else qhB
                        qtw = 0 if par == 0 else 1       # q plane
                        rtw = 1 - qtw                    # rel plane
                        for typ in range(2):
                            r0 = 0 if typ == 0 else 32
                            stat = rh if typ == 0 else rw
                            for hh in range(WS):
                                nwn = w1 - w0
                                if typ == 0:
                                    mov = qh[:, qtw, :, 0:T].rearrange(
                                        "p j (win n) -> p j win n", n=N)[
                                        :, :, w0:w1, hh * WS:(hh + 1) * WS]
                                else:
                                    mov = qh[:, qtw, :, 0:T].rearrange(
                                        "p j (win kh kw) -> p j win kh kw",
                                        kh=WS, kw=WS)[:, :, w0:w1, :, hh]
                                ncols = JC * nwn * WS
                                prel = rel_ps.tile([128, 336], f32, tag="rel")
                                # skip_group_check: sim-only guard; its
                                # flat-address region view aliases across
                                # banks for <128-partition outputs
                                nc.tensor.matmul(
                                    prel[r0:r0 + WS, :ncols],
                                    stat[0:64, hh, :], mov,
                                    start=True, stop=True,
                                    tile_position=(0, r0),
                                    skip_group_check=True)
                                if typ == 0:
                                    dst = qh[r0:r0 + WS, rtw, :, 0:T].rearrange(
                                        "p j (win n) -> p j win n", n=N)[
                                        :, :, w0:w1, hh * WS:(hh + 1) * WS]
                                else:
                                    dst = qh[r0:r0 + WS, rtw, :, 0:T].rearrange(
                                        "p j (win kh kw) -> p j win kh kw",
                                        kh=WS, kw=WS)[:, :, w0:w1, :, hh]
                                src = prel[r0:r0 + WS, :ncols].rearrange(
                                    "p (j win k) -> p j win k", j=JC, win=nwn)
                                with nc.allow_low_precision(
                                        reason="fp8 rel rows; absolute score "
                                        "error ~3e-3 vs budget 0.1"):
                                    if ri % 2 == 0:
                                        nc.scalar.activation(
                                            dst, src, AF.Copy, scale=1.0 / W8)
                                    else:
                                        nc.vector.tensor_scalar(
                                            out=dst, in0=src,
                                            scalar1=1.0 / W8, scalar2=None,
                                            op0=ALU.mult)
                                ri += 1

            emit_qk_chunk(0)
            emit_qk_chunk(1)
            emit_v(0)
            emit_v(1)
            emit_qk_chunk(2)
            emit_qk_chunk(3)
            emit_rel()
            for win in range(2, NW):
                emit_v(win)

        # ---- attention core + proj: software-pipelined across windows ----
        # per iteration: AV+normalize(w) | scores+exp(w+1) | transp+proj(w);
        # window w+1's score matmuls keep PE busy while w's softmax
        # normalization drains on DVE.
        with tc.tile_pool(name="s_ps", bufs=1, space="PSUM") as s_ps, \
             tc.tile_pool(name="av_ps", bufs=2, space="PSUM") as av_ps, \
             tc.tile_pool(name="t_ps", bufs=2, space="PSUM") as t_ps, \
             tc.tile_pool(name="pj_ps", bufs=2, space="PSUM") as pj_ps, \
             tc.tile_pool(name="pt_sb", bufs=26) as pt_sb, \
             tc.tile_pool(name="ao_sb", bufs=2) as ao_sb, \
             tc.tile_pool(name="at_sb", bufs=2) as at_sb, \
             tc.tile_pool(name="xo_sb", bufs=2) as xo_sb, \
             tc.tile_pool(name="r_sb", bufs=4) as r_sb:
            # two persistent score tiles, rotated manually: the exp reads the
            # full [128, 392] tile, so the region no matmul covers (rows
            # 68:128 of the second key chunk) is zeroed exactly once
            sbufs = []
            for i in range(2):
                st = s_ps.tile([128, 392], f32, tag=f"s{i}", name=f"s{i}")
                # partition start must be 32-aligned on PSUM; rows 64:68 are
                # re-written by every second score matmul afterwards
                nc.vector.memset(st[64:128, 196:392], 0.0)
                sbufs.append(st)
            state = {"hidx": 0}

            def emit_scores(win):
                pts = []
                for head in range(HEADS):
                    blk = head // 2
                    par = head % 2
                    qh = qhE if par == 0 else qhB
                    kh = khE if par == 0 else khB
                    ps_t = sbufs[state["hidx"] % 2]
                    state["hidx"] += 1
                    for si, (slo, shi) in enumerate(_SUBS):
                        ssz = shi - slo
                        nc.tensor.matmul(
                            ps_t[:ssz, si * N:si * N + N],
                            kh[:, :, blk, win * N + slo:win * N + shi],
                            qh[:, :, blk, win * N:(win + 1) * N],
                            start=True, stop=True,
                            perf_mode=mybir.MatmulPerfMode.DoubleRow)
                    ptile = pt_sb.tile([128, 392], bf16, tag="pt")
                    nc.scalar.activation(ptile[:], ps_t[:], AF.Exp)
                    pts.append(ptile)
                return pts

            def emit_av(win, pts):
                ao0 = ao_sb.tile([128, DIM], bf16, tag="ao0")
                ao1 = ao_sb.tile([68, DIM], bf16, tag="ao1")
                for qi, (qlo, qhi) in enumerate(_SUBS):
                    qsz = qhi - qlo
                    ao = ao0 if qi == 0 else ao1
                    for hg in range(2):
                        pav = av_ps.tile([128, 6, HD + 1], f32, tag="av")
                        for hl in range(6):
                            head = hg * 6 + hl
                            for si, (slo, shi) in enumerate(_SUBS):
                                ssz = shi - slo
                                nc.tensor.matmul(
                                    pav[:qsz, hl, :],
                                    pts[head][0:ssz, si * N + qlo:si * N + qhi],
                                    vtok[0:ssz, win, si, head, :],
                                    start=(si == 0), stop=(si == 1))
                        rec = r_sb.tile([128, 6], f32, tag="rec")
                        nc.vector.reciprocal(
                            rec[:qsz, :],
                            pav[:qsz, :, HD:HD + 1].rearrange(
                                "p h o -> p (h o)"))
                        nc.vector.tensor_tensor(
                            out=ao[0:qsz, hg * 384:(hg + 1) * 384].rearrange(
                                "p (h d) -> p h d", d=HD),
                            in0=pav[:qsz, :, 0:HD],
                            in1=rec[:qsz, :].unsqueeze(2).to_broadcast(
                                [qsz, 6, HD]),
                            op=ALU.mult)
                return ao0, ao1

            def emit_transp_proj(win, ao0, ao1):
                aT = at_sb.tile([128, JC, NPAD], dt.float8e4, tag="at")
                ti = 0
                for j in range(JC):
                    for qi, (qlo, qhi) in enumerate(_SUBS):
                        qsz = qhi - qlo
                        src = ao0 if qi == 0 else ao1
                        ptt = t_ps.tile([128, 128], bf16, tag="tp")
                        nc.tensor.transpose(ptt[:, :qsz],
                                            src[0:qsz, j * 128:(j + 1) * 128],
                                            ident[0:qsz, 0:qsz])
                        if ti % 3 == 0:
                            nc.scalar.copy(aT[:, j, qlo:qhi], ptt[:, :qsz])
                        else:
                            nc.vector.tensor_copy(aT[:, j, qlo:qhi],
                                                  ptt[:, :qsz])
                        ti += 1
                xo_t = xo_sb.tile([128, JC, N], f32, tag="xo")
                for m in range(JC):
                    pp = pj_ps.tile([128, N], f32, tag="pj")
                    for jp in range(JC // 2):
                        nc.tensor.matmul(
                            pp[:], wp[:, 2 * jp:2 * jp + 2, m, :],
                            aT[:, 2 * jp:2 * jp + 2, 0:N],
                            start=(jp == 0), stop=(jp == JC // 2 - 1),
                            perf_mode=mybir.MatmulPerfMode.DoubleRow)
                    nc.vector.tensor_scalar(
                        out=xo_t[:, m, :], in0=pp[:],
                        scalar1=1.0 / W8, scalar2=pb[:, m:m + 1],
                        op0=ALU.mult, op1=ALU.add)
                nc.sync.dma_start(
                    xoT.rearrange("p j (w n) -> p j w n", n=N)[:, :, win, :],
                    xo_t[:])

            pts = emit_scores(0)
            for win in range(NW):
                ao0, ao1 = emit_av(win, pts)
                if win + 1 < NW:
                    pts = emit_scores(win + 1)
                emit_transp_proj(win, ao0, ao1)
    nc.compile()
    return nc


def build_mlp():
    """Dispatch B: y = x + fc2(gelu(fc1(LN2(x)))), 1024 tokens/core.
    LN2 affine is absorbed into fc1 host-side."""
    nc = bacc.Bacc("TRN2", target_bir_lowering=False, debug=False)
    f32, bf16 = dt.float32, dt.bfloat16

    xT = nc.dram_tensor("xT", [128, JC, TB], bf16, kind="ExternalInput").ap()
    f8 = dt.float8e4
    fc1W = nc.dram_tensor("fc1W", [128, HC, DIM], f8, kind="ExternalInput").ap()
    fc1S = nc.dram_tensor("fc1S", [128, HC], f32, kind="ExternalInput").ap()
    fc2W = nc.dram_tensor("fc2W", [128, HC, DIM], bf16, kind="ExternalInput").ap()
    fc1B = nc.dram_tensor("fc1B", [128, HC], f32, kind="ExternalInput").ap()
    fc2B = nc.dram_tensor("fc2B", [128, JC], f32, kind="ExternalInput").ap()
    yT = nc.dram_tensor("yT", [128, JC, TB], f32, kind="ExternalOutput").ap()

    with tile.TileContext(nc) as tc, ExitStack() as ctx:
        const = ctx.enter_context(tc.tile_pool(name="const", bufs=1))
        big = ctx.enter_context(tc.tile_pool(name="big", bufs=1))
        lnrows = ctx.enter_context(tc.tile_pool(name="lnrows", bufs=1))

        ones1 = const.tile([128, 1], bf16)
        nc.vector.memset(ones1[:], 1.0)
        onesP1 = const.tile([1, 128], bf16)
        nc.vector.memset(onesP1[:], 1.0)
        b1t = const.tile([128, HC], f32)
        nc.scalar.dma_start(b1t[:], fc1B)
        s1t = const.tile([128, HC], f32)
        nc.scalar.dma_start(s1t[:], fc1S)
        b2t = const.tile([128, JC], f32)
        nc.scalar.dma_start(b2t[:], fc2B)

        xtiles = big.tile([128, JC, TB], bf16)
        xn = big.tile([128, JC, TB], dt.float8e4)
        h = big.tile([128, HC, TB], bf16)

        with tc.tile_pool(name="w1_sb", bufs=HC) as w1p, \
             tc.tile_pool(name="w2_sb", bufs=1) as w2p:
            # x chunks first on the sync queue -> served first by the DMA
            # device; weights follow in need order (w1 chunks, then w2)
            for lo, hi in _NSL_B:
                nc.sync.dma_start(xtiles[:, :, lo:hi], xT[:, :, lo:hi])
            w1s = []
            for m in range(HC):
                w1m = w1p.tile([128, JC, 128], f8, tag="w1", name=f"w1_{m}")
                nc.sync.dma_start(
                    w1m[:].rearrange("p j c -> p (j c)"), fc1W[:, m, :])
                w1s.append(w1m)
            w2t = w2p.tile([128, HC, DIM], bf16)
            nc.sync.dma_start(w2t[:], fc2W)

            def load_chunk(ci, lo, hi):
                return xtiles[:, :, lo:hi]

            _standardize(nc, tc, _NSL_B, JC, load_chunk, xn, ones1, onesP1,
                         lnrows, bf16_in=True)

            with tc.tile_pool(name="f1_ps", bufs=4, space="PSUM") as f1_ps, \
                 tc.tile_pool(name="f2_ps", bufs=3, space="PSUM") as f2_ps, \
                 tc.tile_pool(name="out_sb", bufs=3) as out_sb:
                for ci, (lo, hi) in enumerate(_FC_B):
                    w = hi - lo
                    for m in range(HC):
                        pt = f1_ps.tile([128, w], f32, tag="f1")
                        for jp in range(JC // 2):
                            nc.tensor.matmul(
                                pt[:], w1s[m][:, 2 * jp:2 * jp + 2, :],
                                xn[:, 2 * jp:2 * jp + 2, lo:hi],
                                start=(jp == 0), stop=(jp == JC // 2 - 1),
                                perf_mode=mybir.MatmulPerfMode.DoubleRow)
                        # per-out-channel fp8 descale via the activation
                        # scale AP; bias applies after the scale
                        nc.scalar.activation(h[:, m, lo:hi], pt[:], AF.Gelu,
                                             bias=b1t[:, m:m + 1],
                                             scale=s1t[:, m:m + 1])
                for ci, (lo, hi) in enumerate(_FC_B):
                    for m in range(JC):
                        last = (ci == len(_FC_B) - 1 and m == JC - 1)
                        parts = ([(lo, (lo + hi) // 2), ((lo + hi) // 2, hi)]
                                 if last else [(lo, hi)])
                        for plo, phi in parts:
                            w = phi - plo
                            pt = f2_ps.tile([128, 512], f32, tag="f2")
                            for j in range(HC):
                                nc.tensor.matmul(
                                    pt[:, :w], w2t[:, j, m * 128:(m + 1) * 128],
                                    h[:, j, plo:phi],
                                    start=(j == 0), stop=(j == HC - 1))
                            ot = out_sb.tile([128, 512], f32, tag="out")
                            # ot = (psum + fc2_b) + x   in one DVE pass
                            nc.vector.scalar_tensor_tensor(
                                out=ot[:, :w], in0=pt[:, :w],
                                scalar=b2t[:, m:m + 1],
                                in1=xtiles[:, m, plo:phi],
                                op0=ALU.add, op1=ALU.add)
                            nc.sync.dma_start(yT[:, m, plo:phi], ot[:, :w])
    nc.compile()
    return nc


# ---------------- host glue ----------------

_CACHE = {}


def _get(name, builder):
    if name not in _CACHE:
        _CACHE[name] = builder()
    return _CACHE[name]


def _featmajor(a):
    """(T, 768) fp32 -> [128, 6, T]"""
    Tn = a.shape[0]
    return np.ascontiguousarray(a.T.reshape(JC, 128, Tn).transpose(1, 0, 2))


def _unfeat(aT):
    """[128, 6, T] -> (T, 768)"""
    return np.asarray(aT).transpose(1, 0, 2).reshape(DIM, -1).T


def _chunkvec(v):
    """(c*128,) -> [128, c] fp32"""
    v = np.asarray(v, np.float32)
    return np.ascontiguousarray(v.reshape(-1, 128).T)


def _wchunk(w, nchunk, dtype=BF16):
    """(768, nchunk*128) weight -> [128, nchunk, 768] (m-major chunks:
    out[p, m, j*128 + c] = w[j*128 + p, m*128 + c])."""
    w = np.asarray(w, np.float32)
    kin = w.shape[0] // 128
    out = w.reshape(kin, 128, nchunk, 128).transpose(1, 2, 0, 3)
    return np.ascontiguousarray(out.reshape(128, nchunk, kin * 128)).astype(dtype)


def _bf16(a):
    return np.asarray(a, dtype=BF16)


def _build_rel(rel_pos, ws=WS):
    """[64, 14, 14] fp8: out[c, h, k] = rel_pos[idx[h,k], c] * W8 / sqrt(SCALE)
    (qhat holds sqrt(SCALE)*q; W8 prescale is undone at the rel eviction)."""
    idx = np.arange(ws)[:, None] - np.arange(ws)[None, :] + (ws - 1)
    R = np.asarray(rel_pos, np.float32)[idx] * (W8 / SCALE ** 0.5)
    return R.transpose(2, 0, 1).astype(F8)


def _build_onehots():
    """Eh[r, :, k] = 1 if k//14 == r;  Ew[r, :, k] = 1 if k%14 == r,
    pre-expanded over the (j, win) axis for big contiguous DMA runs."""
    k = np.arange(N)
    Eh = (k[None, :] // WS == np.arange(WS)[:, None]).astype(np.float32)
    Ew = (k[None, :] % WS == np.arange(WS)[:, None]).astype(np.float32)
    Eh = np.ascontiguousarray(np.broadcast_to(Eh[:, None, :], (WS, JC * NW, N)))
    Ew = np.ascontiguousarray(np.broadcast_to(Ew[:, None, :], (WS, JC * NW, N)))
    return Eh.astype(F8), Ew.astype(F8)


kernel_last_perf = {}

try:
    from antenv.axon_hooks import get_axon_ntff_profile_hook as _hook  # noqa: F401
    _HAVE_TRACE = True
except ImportError:
    _HAVE_TRACE = False
    import os as _os
    _os.environ["BASS_NEVER_TRACE"] = "1"   # bass_utils re-reads BASS_TRACE


def window_x(x):
    """(2, 64, 64, 768) -> (56, 196, 768) padded window tokens."""
    B, H, W, C = x.shape
    xp = np.zeros((B, 70, 70, C), np.float32)
    xp[:, :64, :64] = x
    xw = xp.reshape(B, 5, WS, 5, WS, C).transpose(0, 1, 3, 2, 4, 5).reshape(50, N, C)
    xall = np.zeros((56, N, C), np.float32)
    xall[:50] = xw
    return xall


def attn_consts(norm1_w, norm1_b, qkv_w, qkv_b, proj_w, proj_b,
                rel_pos_h, rel_pos_w):
    """Host-side constant tensors for dispatch A (LN1 affine absorbed)."""
    n1w = np.asarray(norm1_w, np.float32)
    n1b = np.asarray(norm1_b, np.float32)
    qkvw = np.asarray(qkv_w, np.float32)
    qkvb = np.asarray(qkv_b, np.float32)
    Wq = n1w[:, None] * qkvw                 # (768, 2304)
    bfull = n1b @ qkvw + qkvb                # (2304,)
    Eh, Ew = _build_onehots()
    return {
        "qkW": _wchunk(Wq[:, 0:2 * DIM] * W8, 12, F8),
        "wvT": np.ascontiguousarray(
            Wq[:, 2 * DIM:].reshape(JC, 128, DIM).transpose(1, 0, 2)
            * W8).astype(F8),
        "wpT": np.ascontiguousarray(
            np.asarray(proj_w, np.float32).reshape(JC, 128, JC, 128)
            .transpose(1, 0, 2, 3) * W8).astype(F8),
        "bqT": _chunkvec(bfull[0:DIM] * SCALE ** 0.5),
        "bkT": _chunkvec(bfull[DIM:2 * DIM] * SCALE ** 0.5),
        "bvT": _bf16(bfull[2 * DIM:].reshape(1, 2, 384) * W8),
        "pbT": _chunkvec(proj_b),
        "RhT": _build_rel(rel_pos_h),
        "RwT": _build_rel(rel_pos_w),
        "EhT": Eh,
        "EwT": Ew,
    }


def mlp_consts(norm2_w, norm2_b, fc1_w, fc1_b, fc2_w, fc2_b):
    """Host-side constant tensors for dispatch B (LN2 affine absorbed)."""
    n2w = np.asarray(norm2_w, np.float32)
    n2b = np.asarray(norm2_b, np.float32)
    f1w = np.asarray(fc1_w, np.float32)
    W1 = n2w[:, None] * f1w                  # (768, 3072)
    b1 = n2b @ f1w + np.asarray(fc1_b, np.float32)
    # per-out-channel power-of-2 fp8 scaling for fc1 (exactly undone by the
    # gelu activation's per-partition scale AP)
    colmax = np.abs(W1).max(axis=0)                        # (3072,)
    sexp = np.clip(np.floor(np.log2(224.0 / np.maximum(colmax, 1e-30))),
                   -20, 20)
    wscale = np.exp2(sexp)                                 # (3072,)
    return {
        "fc1W": _wchunk(W1 * wscale[None, :], HC, F8),
        "fc1S": _chunkvec(1.0 / wscale),
        "fc2W": _bf16(np.ascontiguousarray(
            np.asarray(fc2_w, np.float32).reshape(HC, 128, DIM)
            .transpose(1, 0, 2))),
        "fc1B": _chunkvec(b1),
        "fc2B": _chunkvec(fc2_b),
    }


def kernel(x, norm1_w, norm1_b, qkv_w, qkv_b, proj_w, proj_b,
           rel_pos_h, rel_pos_w, norm2_w, norm2_b,
           fc1_w, fc1_b, fc2_w, fc2_b):
    import os
    trace = bool(os.environ.get("BASS_TRACE")) and _HAVE_TRACE
    x = np.asarray(x, np.float32)
    B, H, W, C = x.shape
    assert (B, H, W, C) == (2, 64, 64, DIM)

    # ---- dispatch A: windowed attention ----
    xall = window_x(x)
    consts_a = attn_consts(norm1_w, norm1_b, qkv_w, qkv_b, proj_w, proj_b,
                           rel_pos_h, rel_pos_w)
    with_vbias = bool(np.any(np.asarray(consts_a["bvT"], np.float32)))
    nc_a = _get(f"attn{int(with_vbias)}",
                lambda: build_attn(with_vbias=with_vbias))
    in_maps = []
    for c in range(NCORES):
        m = dict(consts_a)
        m["xT"] = _featmajor(
            xall[c * NW:(c + 1) * NW].reshape(T, C)).astype(BF16)
        in_maps.append(m)
    res_a = run_bass_kernel_spmd(nc_a, in_maps, core_ids=list(range(NCORES)),
                                 trace=trace)
    kernel_last_perf["attn"] = res_a.exec_time_ns
    xo_all = np.stack([_unfeat(res_a.results[c]["xoT"]) for c in range(NCORES)])
    xo = xo_all.reshape(56, N, C)[:50]
    xo = xo.reshape(B, 5, 5, WS, WS, C).transpose(0, 1, 3, 2, 4, 5).reshape(B, 70, 70, C)
    x2 = x + xo[:, :64, :64]

    # ---- dispatch B: MLP ----
    nc_b = _get("mlp", build_mlp)
    consts_b = mlp_consts(norm2_w, norm2_b, fc1_w, fc1_b, fc2_w, fc2_b)
    x2f = np.ascontiguousarray(x2.reshape(B * H * W, C))
    in_maps = []
    for c in range(NCORES):
        m = dict(consts_b)
        m["xT"] = _featmajor(x2f[c * TB:(c + 1) * TB]).astype(BF16)
        in_maps.append(m)
    res_b = run_bass_kernel_spmd(nc_b, in_maps, core_ids=list(range(NCORES)),
                                 trace=trace)
    kernel_last_perf["mlp"] = res_b.exec_time_ns
    y = np.concatenate([_unfeat(res_b.results[c]["yT"]) for c in range(NCORES)])
    return y.reshape(B, H, W, C).astype(np.float32)


# revision 62
# speedup vs baseline: 1.1653x; 1.0065x over previous
"""Trainium2 Bass kernel for a SAM/ViTDet-style windowed-attention transformer
block (DIM=768, 12 heads, window 14, decomposed rel-pos bias, exact-gelu MLP).

Contract: kernel(**inputs) takes the FULL unsharded inputs from
reference.setup_inputs() and returns the FULL (2, 64, 64, 768) float32 output.

Strategy (8 NeuronCores, SPMD, data-parallel):
  Dispatch A (attention): shard the 50 real windows (padded to 56) as 7
    windows/core. Per core: LN1 -> qkv -> windowed attention with the
    decomposed rel-pos bias folded into an augmented-key matmul -> proj.
  Host: window-unpartition, crop, residual add.
  Dispatch B (MLP): shard the 8192 tokens as 1024/core. Per core:
    LN2 -> fc1 -> exact GELU -> fc2 -> residual.

Perf notes (v2):
  * The LN affine (w, b) is absorbed host-side into the following matmul
    weights/biases, so on-device LN is a pure standardize: bf16 stats
    matmuls + bf16 broadcast tiles + two 4x-rate DVE tensor_tensor ops.
  * Rel-pos rows are produced by per-(h or w) batched matmuls (112 instead
    of 392) whose PSUM outputs land at partition bases 0/32/64/96, cutting
    eviction traffic.
  * The two score matmuls of one (window, head) share a [128, 392] PSUM
    tile -> a single exp instruction per head.
  * AV outputs are 6-head-batched in PSUM; the softmax normalization is one
    DVE multiply with a stride-0 (broadcast) reciprocal operand.
  * Weight DMAs are chunked and issued up front so compute starts ~10us in.

Augmented-key rel-pos layout (q/k head-pair blocks, per j-chunk):
  even head: q 0:64,  rel_h 64:78,  zeros 78:96, rel_w 96:110, zeros 110:128
  odd head:  rel_h 0:14, zeros 14:32, rel_w 32:46, zeros 46:64, q 64:128
khat holds k values in the q rows and one-hot key-position masks in the rel
rows; zero gaps make the extra contraction rows inert. S^T = khat^T qhat then
includes the decomposed bias exactly.
"""

import sys

sys.path.insert(0, "/opt/trn_rl_repo")

from contextlib import ExitStack

import numpy as np
import ml_dtypes

import concourse.bacc as bacc
import concourse.mybir as mybir
import concourse.tile as tile
from concourse.bass_utils import run_bass_kernel_spmd
from concourse.masks import make_identity

dt = mybir.dt
AF = mybir.ActivationFunctionType
ALU = mybir.AluOpType

DIM = 768
HEADS = 12
HD = 64
WS = 14
N = WS * WS          # 196 tokens / window
NW = 7               # windows per core
T = NW * N           # 1372 token slots per core (dispatch A)
TB = 1024            # tokens per core (dispatch B)
MLP = 3072
NCORES = 8
JC = DIM // 128      # 6 feature chunks
HC = MLP // 128      # 24 hidden chunks
EPS = 1e-5
SCALE = HD ** -0.5   # 0.125
TPAD = 1376          # T rounded up so fp8 DoubleRow pair strides are 16B-aligned
NPAD = 208           # N rounded up likewise (aT)
BF16 = ml_dtypes.bfloat16
F8 = ml_dtypes.float8_e4m3
W8 = 64.0   # fp8 weight pre-scale (avoids e4m3 subnormals); undone at eviction

# window-aligned token chunks for dispatch A (2+2+2+1 windows)
_NSL = [(0, 392), (392, 784), (784, 1176), (1176, 1372)]
_NSL_LN = [(i * N, (i + 1) * N) for i in range(NW)]        # LN chunks, A
_NSL_B = [(0, 256), (256, 512), (512, 768), (768, 1024)]   # LN chunks, B
_FC_B = [(0, 512), (512, 1024)]                            # matmul chunks, B
_SUBS = [(0, 128), (128, 196)]                             # within-window subchunks

# augmented-key row layout per parity: (q_lo, relh_lo, relw_lo)
_EVEN = (0, 64, 96)    # q 0:64,  rel rows above
_ODD = (64, 0, 32)     # q 64:128, rel rows below


def _standardize(nc, tc, nsl_list, jc, load_chunk, xn, ones1, onesP1, rows_p,
                 cast_engine="gpsimd", bf16_in=False):
    """Pure LN standardize: xn[:, j, c] = (x - mu[c]) * rsig[c], bf16 out.

    load_chunk(ci, lo, hi) -> AP [128, jc, w] for that chunk (may DMA into a
    fresh tile or return a view of a resident one); fp32 unless bf16_in (then
    it is used directly, no cast).  Stats run as bf16 matmuls vs a ones
    vector; mu/rsig are kept as bf16 rows, broadcast across partitions via
    tiny bf16 matmuls, evicted to bf16 SBUF and applied with two DVE
    tensor_tensor ops (all-bf16, stride-1 -> DVE fast mode).
    ones1: [128,1] bf16 ones; onesP1: [1,128] bf16 ones; rows_p: pool for rows.
    """
    nch = len(nsl_list)
    dimn = jc * 128

    eng_cast = getattr(nc, cast_engine)

    epsr = rows_p.tile([1, 1], dt.float32, tag="epsr")
    nc.vector.memset(epsr[:], EPS)

    with tc.tile_pool(name="ln_xb", bufs=nch) as xbp, \
         tc.tile_pool(name="ln_sq", bufs=2) as sqp, \
         tc.tile_pool(name="ln_st", bufs=2, space="PSUM") as st_ps, \
         tc.tile_pool(name="ln_bc", bufs=2, space="PSUM") as bc_ps, \
         tc.tile_pool(name="ln_bcs", bufs=4) as bcs, \
         tc.tile_pool(name="ln_rows", bufs=nch + 1) as rp:
        # per-chunk stats emitted with the apply of the PREVIOUS chunk
        # interleaved (one-chunk lag): PE runs stats back to back while the
        # row math / broadcast / apply of the prior chunk drains on Act/DVE
        xbs, mus, rss = [], [], []

        def emit_stats(ci, lo, hi):
            w = hi - lo
            xt = load_chunk(ci, lo, hi)
            if bf16_in:
                xb = xt
            else:
                xb = xbp.tile([128, jc, w], dt.bfloat16, tag="xb",
                              name=f"xb{ci}")
                eng_cast.tensor_copy(xb[:], xt)
                xb = xb[:]
            xbs.append(xb)
            # per-token sums -> mu
            pmu = st_ps.tile([1, w], dt.float32, tag="st")
            for j in range(jc):
                nc.tensor.matmul(pmu[:], ones1[:], xb[:, j, :],
                                 start=(j == 0), stop=(j == jc - 1))
            mu = rp.tile([1, w], dt.bfloat16, tag="mu", name=f"mu{ci}")
            nc.scalar.activation(mu[:], pmu[:], AF.Copy, scale=1.0 / dimn)
            mus.append(mu)
            # per-token sum of squares -> E[x^2]
            sq = sqp.tile([128, jc, w], dt.bfloat16, tag="sq")
            nc.vector.tensor_tensor(out=sq[:], in0=xb, in1=xb, op=ALU.mult)
            pmq = st_ps.tile([1, w], dt.float32, tag="st")
            for j in range(jc):
                nc.tensor.matmul(pmq[:], ones1[:], sq[:, j, :],
                                 start=(j == 0), stop=(j == jc - 1))
            mq = rp.tile([1, w], dt.float32, tag="mq")
            nc.scalar.activation(mq[:], pmq[:], AF.Copy, scale=1.0 / dimn)
            # rsig = 1/sqrt(E[x^2] - mu^2 + eps)
            m2 = rp.tile([1, w], dt.float32, tag="m2")
            nc.vector.tensor_tensor(out=m2[:], in0=mu[:], in1=mu[:],
                                    op=ALU.mult)
            nc.vector.tensor_tensor(out=mq[:], in0=mq[:], in1=m2[:],
                                    op=ALU.subtract)
            sd = rp.tile([1, w], dt.float32, tag="sd")
            nc.scalar.activation(sd[:], mq[:], AF.Sqrt, bias=epsr[:])
            rsig = rp.tile([1, w], dt.bfloat16, tag="rs", name=f"rs{ci}")
            with nc.allow_low_precision(reason="bf16 rsig row; 0.4% rel err "
                                        "matches the bf16 matmul noise "
                                        "floor"):
                nc.vector.reciprocal(rsig[:], sd[:])
            rss.append(rsig)

        def emit_apply(ci, lo, hi):
            w = hi - lo
            xb, mu, rsig = xbs[ci], mus[ci], rss[ci]
            bmu_p = bc_ps.tile([128, w], dt.float32, tag="bc")
            nc.tensor.matmul(bmu_p[:], onesP1[:], mu[:], start=True, stop=True)
            brs_p = bc_ps.tile([128, w], dt.float32, tag="bc")
            nc.tensor.matmul(brs_p[:], onesP1[:], rsig[:], start=True,
                             stop=True)
            bmu = bcs.tile([128, w], dt.bfloat16, tag="bmu")
            nc.scalar.copy(bmu[:], bmu_p[:])
            brs = bcs.tile([128, w], dt.bfloat16, tag="brs")
            nc.scalar.copy(brs[:], brs_p[:])
            # xn = (x - mu) * rsig   (two all-bf16 DVE ops, j-broadcast)
            cen = sqp.tile([128, jc, w], dt.bfloat16, tag="cen")
            nc.vector.tensor_tensor(
                out=cen[:], in0=xb,
                in1=bmu[:].unsqueeze(1).to_broadcast([128, jc, w]),
                op=ALU.subtract)
            with nc.allow_low_precision(reason="xn storage dtype (bf16/fp8) "
                                        "is the matmul operand precision"):
                nc.vector.tensor_tensor(
                    out=xn[:, :, lo:hi], in0=cen[:],
                    in1=brs[:].unsqueeze(1).to_broadcast([128, jc, w]),
                    op=ALU.mult)

        # chunk 0's apply is emitted right after its stats so the first xn
        # chunk (the qk-phase gate) is produced as early as possible
        emit_stats(0, *nsl_list[0])
        emit_apply(0, *nsl_list[0])
        for ci in range(1, nch):
            emit_stats(ci, *nsl_list[ci])
        for ci in range(1, nch):
            emit_apply(ci, *nsl_list[ci])


def build_attn(with_vbias=True):
    """Dispatch A: LN1 + qkv + windowed attention (+rel-pos) + proj."""
    nc = bacc.Bacc("TRN2", target_bir_lowering=False, debug=False)
    f32, bf16 = dt.float32, dt.bfloat16

    xT = nc.dram_tensor("xT", [128, JC, T], bf16, kind="ExternalInput").ap()
    f8 = dt.float8e4
    qkW = nc.dram_tensor("qkW", [128, 12, JC * 128], f8, kind="ExternalInput").ap()
    wvT = nc.dram_tensor("wvT", [128, JC, DIM], f8, kind="ExternalInput").ap()
    wpT = nc.dram_tensor("wpT", [128, JC, JC, 128], f8, kind="ExternalInput").ap()
    bqT = nc.dram_tensor("bqT", [128, JC], f32, kind="ExternalInput").ap()
    bkT = nc.dram_tensor("bkT", [128, JC], f32, kind="ExternalInput").ap()
    bvT = nc.dram_tensor("bvT", [1, 2, 384], bf16, kind="ExternalInput").ap()
    pbT = nc.dram_tensor("pbT", [128, JC], f32, kind="ExternalInput").ap()
    RhT = nc.dram_tensor("RhT", [64, WS, WS], f8, kind="ExternalInput").ap()
    RwT = nc.dram_tensor("RwT", [64, WS, WS], f8, kind="ExternalInput").ap()
    EhT = nc.dram_tensor("EhT", [WS, JC * NW, N], f8, kind="ExternalInput").ap()
    EwT = nc.dram_tensor("EwT", [WS, JC * NW, N], f8, kind="ExternalInput").ap()
    xoT = nc.dram_tensor("xoT", [128, JC, T], f32, kind="ExternalOutput").ap()

    with tile.TileContext(nc) as tc, ExitStack() as ctx:
        const = ctx.enter_context(tc.tile_pool(name="const", bufs=1))
        big = ctx.enter_context(tc.tile_pool(name="big", bufs=1))
        lnrows = ctx.enter_context(tc.tile_pool(name="lnrows", bufs=1))

        # ---- big persistent tensors (declared first so memsets start at t=0)
        # qhat/khat live in a partition-paired fp8 layout [64, two, j, t]:
        # logical contraction row r maps to (r % 64, r // 64), so the score
        # matmuls run fp8 DoubleRow.  Plane assignment per parity:
        #   even head: q = plane 0, rel_h rows 0:14 / rel_w 32:46 on plane 1
        #   odd head:  q = plane 1, rel_h rows 0:14 / rel_w 32:46 on plane 0
        xn = big.tile([128, JC, TPAD], dt.float8e4)
        qhE = big.tile([64, 2, JC, TPAD], dt.float8e4)
        khE = big.tile([64, 2, JC, TPAD], dt.float8e4)
        qhB = big.tile([64, 2, JC, TPAD], dt.float8e4)
        khB = big.tile([64, 2, JC, TPAD], dt.float8e4)
        vtok = big.tile([128, NW, 2, HEADS, HD + 1], bf16)

        # ---- constants ----
        ones1 = const.tile([128, 1], bf16)
        nc.vector.memset(ones1[:], 1.0)
        onesP1 = const.tile([1, 128], bf16)
        nc.vector.memset(onesP1[:], 1.0)
        onesT = const.tile([1, 128], bf16)
        nc.vector.memset(onesT[:], 1.0)
        ident = const.tile([128, 128], bf16)
        make_identity(nc, ident[:])

        rh = const.tile([64, WS, WS], f8)
        nc.scalar.dma_start(rh[:], RhT)
        rw = const.tile([64, WS, WS], f8)
        nc.scalar.dma_start(rw[:], RwT)
        bq = const.tile([128, JC], f32)
        nc.scalar.dma_start(bq[:], bqT)
        bk = const.tile([128, JC], f32)
        nc.scalar.dma_start(bk[:], bkT)
        bv = const.tile([1, 2, 384], bf16)
        nc.scalar.dma_start(bv[:], bvT)
        pb = const.tile([128, JC], f32)
        nc.scalar.dma_start(pb[:], pbT)
        # wv/wp tiles are created here but their loads are issued on the sync
        # queue after the x/qk-weight DMAs so the global DMA device serves x
        # first (sync-queue program order == DMA device order).
        wv = const.tile([128, JC, DIM], f8)
        wp = const.tile([128, JC, JC, 128], f8)

        # ---- LN1 (pure standardize; affine absorbed into weights) ----
        # x arrives already bf16 (host cast) -> no on-device cast, half DMA;
        # window-sized chunks get the first xn out early for the qk start
        with tc.tile_pool(name="ln_x", bufs=4) as xp:
            def load_chunk(ci, lo, hi):
                xt = xp.tile([128, JC, hi - lo], bf16, tag="x", name=f"x{ci}")
                nc.sync.dma_start(xt[:], xT[:, :, lo:hi])
                return xt[:]
            _standardize(nc, tc, _NSL, JC, load_chunk, xn, ones1, onesP1,
                         lnrows, bf16_in=True)

        # zero the rel/one-hot halves (gaps must be exactly 0; rel rows and
        # one-hot rows overlay these ranges later).  Issued after the LN body
        # so the Pool queue serves the LN casts first; the Tile deps still
        # order these before the mask DMAs / rel evictions below.
        nc.gpsimd.memset(khE[:, 1, :, :], 0.0)
        nc.gpsimd.memset(khB[:, 0, :, :], 0.0)
        nc.gpsimd.memset(qhE[:, 1, :, :], 0.0)
        nc.gpsimd.memset(qhB[:, 0, :, :], 0.0)
        # ones column in vtok (AV matmul also yields the softmax denominator)
        nc.gpsimd.memset(
            vtok[:].rearrange("p w s h o -> p (w s h) o")[:, :, HD:HD + 1], 1.0)

        # ---- q/k (feature-major, split by parity) + v (token-major) ----
        # PE program order is tuned so the in-order PE queue never waits on
        # slow producers: qk c0,c1 | v w0,w1 | qk c2,c3 | rel matmuls |
        # v w2..w6 (covers the rel-eviction drain) | pipelined core.
        _WHALF = [(0, 4), (4, 7)]
        with tc.tile_pool(name="qk_w", bufs=12) as wqk_sb, \
             tc.tile_pool(name="qk_stg", bufs=2) as stg_sb, \
             tc.tile_pool(name="qk_ps", bufs=2, space="PSUM") as qk_ps, \
             tc.tile_pool(name="v_ps", bufs=2, space="PSUM") as v_ps, \
             tc.tile_pool(name="rel_ps", bufs=4, space="PSUM") as rel_ps:
            wms = []
            for m in range(12):
                wm = wqk_sb.tile([128, JC, 128], f8, tag="wqk", name=f"w{m}")
                nc.sync.dma_start(
                    wm[:].rearrange("p j c -> p (j c)"), qkW[:, m, :])
                wms.append(wm)
            nc.sync.dma_start(wv[:], wvT)
            nc.sync.dma_start(wp[:], wpT)
            # one-hot key-position masks into khat rel rows; issued last on
            # the sync queue so x/weight transfers win the DMA device first
            for (msrc, mdst, tw, r0) in ((EhT, khE, 1, 0), (EwT, khE, 1, 32),
                                         (EhT, khB, 0, 0), (EwT, khB, 0, 32)):
                nc.sync.dma_start(
                    mdst[r0:r0 + WS, tw, :, 0:T].rearrange(
                        "p j (w n) -> p j w n", n=N),
                    msrc.rearrange("p (j w) n -> p j w n", w=NW))

            def emit_qk_chunk(ci):
                lo, hi = _NSL[ci]
                w = hi - lo
                # full-height evicts into per-chunk staging tiles (the bias
                # AP is per-partition, so one op covers both parity halves);
                # per chunk just 4 SBUF->SBUF DMAs distribute the halves
                qstg = stg_sb.tile([128, JC, 392], dt.float8e4, tag="qstg")
                kstg = stg_sb.tile([128, JC, 392], dt.float8e4, tag="kstg")
                for m in range(12):
                    is_q = m < JC
                    e = m % JC
                    pt = qk_ps.tile([128, 392], f32, tag="qk")
                    for jp in range(JC // 2):
                        nc.tensor.matmul(pt[:, :w],
                                         wms[m][:, 2 * jp:2 * jp + 2, :],
                                         xn[:, 2 * jp:2 * jp + 2, lo:hi],
                                         start=(jp == 0),
                                         stop=(jp == JC // 2 - 1),
                                         perf_mode=mybir.MatmulPerfMode.DoubleRow)
                    if is_q:
                        nc.scalar.activation(qstg[:, e, :w], pt[:, :w],
                                             AF.Identity, bias=bq[:, m:m + 1],
                                             scale=SCALE ** 0.5 / W8)
                    else:
                        nc.vector.tensor_scalar(
                            out=kstg[:, e, :w], in0=pt[:, :w],
                            scalar1=SCALE ** 0.5 / W8, scalar2=bk[:, e:e + 1],
                            op0=ALU.mult, op1=ALU.add)
                nc.sync.dma_start(qhE[:, 0, :, lo:hi], qstg[0:64, :, :w])
                nc.sync.dma_start(qhB[:, 1, :, lo:hi], qstg[64:128, :, :w])
                nc.sync.dma_start(khE[:, 0, :, lo:hi], kstg[0:64, :, :w])
                nc.sync.dma_start(khB[:, 1, :, lo:hi], kstg[64:128, :, :w])

            def emit_v(win):
                for si, (slo, shi) in enumerate(_SUBS):
                    ssz = shi - slo
                    base = win * N + slo
                    for half in range(2):
                        pv = v_ps.tile([128, 384], f32, tag="v")
                        for jp in range(JC // 2):
                            nc.tensor.matmul(
                                pv[:ssz, :],
                                xn[:, 2 * jp:2 * jp + 2, base:base + ssz],
                                wv[:, 2 * jp:2 * jp + 2,
                                   half * 384:(half + 1) * 384],
                                start=(jp == 0),
                                stop=(not with_vbias
                                      and jp == JC // 2 - 1),
                                perf_mode=mybir.MatmulPerfMode.DoubleRow)
                        if with_vbias:
                            nc.tensor.matmul(
                                pv[:ssz, :], onesT[:, :ssz], bv[:, half, :],
                                start=False, stop=True)
                        if (si + half) % 2 == 0:
                            nc.scalar.activation(
                                vtok[0:ssz, win, si,
                                     6 * half:6 * half + 6, 0:HD],
                                pv[:ssz, :].rearrange("p (h d) -> p h d",
                                                      d=HD),
                                AF.Copy, scale=1.0 / W8)
                        else:
                            nc.vector.tensor_scalar(
                                out=vtok[0:ssz, win, si,
                                         6 * half:6 * half + 6, 0:HD],
                                in0=pv[:ssz, :].rearrange(
                                    "p (h d) -> p h d", d=HD),
                                scalar1=1.0 / W8, scalar2=None, op0=ALU.mult)

            def emit_rel():
                # rel-pos rows into qhat planes, batched per h (rel_h) / per
                # w (rel_w); window halves keep PSUM cols <= 336.  Both
                # parities read q from partitions 0:64 of their q plane and
                # write rel rows 0:14 (rel_h) / 32:46 (rel_w) of the other.
                ri = 0
                for (w0, w1) in _WHALF:   # window halves outermost: the core
                    # can start on windows 0..3 while half 4..7 still drains
                    for par in range(2):
                        qh = qhE if par == 0 else qhB
                        qtw = 0 if par == 0 else 1       # q plane
                        rtw = 1 - qtw                    # rel plane
                        for typ in range(2):
                            r0 = 0 if typ == 0 else 32
                            stat = rh if typ == 0 else rw
                            for hh in range(WS):
                                nwn = w1 - w0
                                if typ == 0:
                                    mov = qh[:, qtw, :, 0:T].rearrange(
                                        "p j (win n) -> p j win n", n=N)[
                                        :, :, w0:w1, hh * WS:(hh + 1) * WS]
                                else:
                                    mov = qh[:, qtw, :, 0:T].rearrange(
                                        "p j (win kh kw) -> p j win kh kw",
                                        kh=WS, kw=WS)[:, :, w0:w1, :, hh]
                                ncols = JC * nwn * WS
                                prel = rel_ps.tile([128, 336], f32, tag="rel")
                                # skip_group_check: sim-only guard; its
                                # flat-address region view aliases across
                                # banks for <128-partition outputs
                                nc.tensor.matmul(
                                    prel[r0:r0 + WS, :ncols],
                                    stat[0:64, hh, :], mov,
                                    start=True, stop=True,
                                    tile_position=(0, r0),
                                    skip_group_check=True)
                                if typ == 0:
                                    dst = qh[r0:r0 + WS, rtw, :, 0:T].rearrange(
                                        "p j (win n) -> p j win n", n=N)[
                                        :, :, w0:w1, hh * WS:(hh + 1) * WS]
                                else:
                                    dst = qh[r0:r0 + WS, rtw, :, 0:T].rearrange(
                                        "p j (win kh kw) -> p j win kh kw",
                                        kh=WS, kw=WS)[:, :, w0:w1, :, hh]
                                src = prel[r0:r0 + WS, :ncols].rearrange(
                                    "p (j win k) -> p j win k", j=JC, win=nwn)
                                with nc.allow_low_precision(
                                        reason="fp8 rel rows; absolute score "
                                        "error ~3e-3 vs budget 0.1"):
                                    if ri % 2 == 0:
                                        nc.scalar.activation(
                                            dst, src, AF.Copy, scale=1.0 / W8)
                                    else:
                                        nc.vector.tensor_scalar(
                                            out=dst, in0=src,
                                            scalar1=1.0 / W8, scalar2=None,
                                            op0=ALU.mult)
                                ri += 1

            emit_qk_chunk(0)
            emit_qk_chunk(1)
            emit_v(0)
            emit_v(1)
            emit_qk_chunk(2)
            emit_qk_chunk(3)
            emit_rel()
            for win in range(2, NW):
                emit_v(win)

        # ---- attention core + proj: software-pipelined across windows ----
        # per iteration: AV+normalize(w) | scores+exp(w+1) | transp+proj(w);
        # window w+1's score matmuls keep PE busy while w's softmax
        # normalization drains on DVE.
        with tc.tile_pool(name="s_ps", bufs=1, space="PSUM") as s_ps, \
             tc.tile_pool(name="av_ps", bufs=2, space="PSUM") as av_ps, \
             tc.tile_pool(name="t_ps", bufs=2, space="PSUM") as t_ps, \
             tc.tile_pool(name="pj_ps", bufs=2, space="PSUM") as pj_ps, \
             tc.tile_pool(name="pt_sb", bufs=26) as pt_sb, \
             tc.tile_pool(name="ao_sb", bufs=2) as ao_sb, \
             tc.tile_pool(name="at_sb", bufs=2) as at_sb, \
             tc.tile_pool(name="xo_sb", bufs=2) as xo_sb, \
             tc.tile_pool(name="r_sb", bufs=4) as r_sb:
            # two persistent score tiles, rotated manually: the exp reads the
            # full [128, 392] tile, so the region no matmul covers (rows
            # 68:128 of the second key chunk) is zeroed exactly once
            sbufs = []
            for i in range(2):
                st = s_ps.tile([128, 392], f32, tag=f"s{i}", name=f"s{i}")
                # partition start must be 32-aligned on PSUM; rows 64:68 are
                # re-written by every second score matmul afterwards
                nc.vector.memset(st[64:128, 196:392], 0.0)
                sbufs.append(st)
            state = {"hidx": 0}

            def emit_scores(win):
                pts = []
                for head in range(HEADS):
                    blk = head // 2
                    par = head % 2
                    qh = qhE if par == 0 else qhB
                    kh = khE if par == 0 else khB
                    ps_t = sbufs[state["hidx"] % 2]
                    state["hidx"] += 1
                    for si, (slo, shi) in enumerate(_SUBS):
                        ssz = shi - slo
                        nc.tensor.matmul(
                            ps_t[:ssz, si * N:si * N + N],
                            kh[:, :, blk, win * N + slo:win * N + shi],
                            qh[:, :, blk, win * N:(win + 1) * N],
                            start=True, stop=True,
                            perf_mode=mybir.MatmulPerfMode.DoubleRow)
                    ptile = pt_sb.tile([128, 392], bf16, tag="pt")
                    nc.scalar.activation(ptile[:], ps_t[:], AF.Exp)
                    pts.append(ptile)
                return pts

            def emit_av(win, pts):
                ao0 = ao_sb.tile([128, DIM], bf16, tag="ao0")
                ao1 = ao_sb.tile([68, DIM], bf16, tag="ao1")
                for qi, (qlo, qhi) in enumerate(_SUBS):
                    qsz = qhi - qlo
                    ao = ao0 if qi == 0 else ao1
                    for hg in range(2):
                        pav = av_ps.tile([128, 6, HD + 1], f32, tag="av")
                        for hl in range(6):
                            head = hg * 6 + hl
                            for si, (slo, shi) in enumerate(_SUBS):
                                ssz = shi - slo
                                nc.tensor.matmul(
                                    pav[:qsz, hl, :],
                                    pts[head][0:ssz, si * N + qlo:si * N + qhi],
                                    vtok[0:ssz, win, si, head, :],
                                    start=(si == 0), stop=(si == 1))
                        rec = r_sb.tile([128, 6], f32, tag="rec")
                        nc.vector.reciprocal(
                            rec[:qsz, :],
                            pav[:qsz, :, HD:HD + 1].rearrange(
                                "p h o -> p (h o)"))
                        nc.vector.tensor_tensor(
                            out=ao[0:qsz, hg * 384:(hg + 1) * 384].rearrange(
                                "p (h d) -> p h d", d=HD),
                            in0=pav[:qsz, :, 0:HD],
                            in1=rec[:qsz, :].unsqueeze(2).to_broadcast(
                                [qsz, 6, HD]),
                            op=ALU.mult)
                return ao0, ao1

            def emit_transp_proj(win, ao0, ao1):
                aT = at_sb.tile([128, JC, NPAD], dt.float8e4, tag="at")
                ti = 0
                for j in range(JC):
                    for qi, (qlo, qhi) in enumerate(_SUBS):
                        qsz = qhi - qlo
                        src = ao0 if qi == 0 else ao1
                        ptt = t_ps.tile([128, 128], bf16, tag="tp")
                        nc.tensor.transpose(ptt[:, :qsz],
                                            src[0:qsz, j * 128:(j + 1) * 128],
                                            ident[0:qsz, 0:qsz])
                        if False:
                            nc.scalar.copy(aT[:, j, qlo:qhi], ptt[:, :qsz])
                        else:
                            nc.vector.tensor_copy(aT[:, j, qlo:qhi],
                                                  ptt[:, :qsz])
                        ti += 1
                xo_t = xo_sb.tile([128, JC, N], f32, tag="xo")
                for m in range(JC):
                    pp = pj_ps.tile([128, N], f32, tag="pj")
                    for jp in range(JC // 2):
                        nc.tensor.matmul(
                            pp[:], wp[:, 2 * jp:2 * jp + 2, m, :],
                            aT[:, 2 * jp:2 * jp + 2, 0:N],
                            start=(jp == 0), stop=(jp == JC // 2 - 1),
                            perf_mode=mybir.MatmulPerfMode.DoubleRow)
                    nc.vector.tensor_scalar(
                        out=xo_t[:, m, :], in0=pp[:],
                        scalar1=1.0 / W8, scalar2=pb[:, m:m + 1],
                        op0=ALU.mult, op1=ALU.add)
                nc.sync.dma_start(
                    xoT.rearrange("p j (w n) -> p j w n", n=N)[:, :, win, :],
                    xo_t[:])

            pts = emit_scores(0)
            for win in range(NW):
                ao0, ao1 = emit_av(win, pts)
                if win + 1 < NW:
                    pts = emit_scores(win + 1)
                emit_transp_proj(win, ao0, ao1)
    nc.compile()
    return nc


def build_mlp():
    """Dispatch B: y = x + fc2(gelu(fc1(LN2(x)))), 1024 tokens/core.
    LN2 affine is absorbed into fc1 host-side."""
    nc = bacc.Bacc("TRN2", target_bir_lowering=False, debug=False)
    f32, bf16 = dt.float32, dt.bfloat16

    xT = nc.dram_tensor("xT", [128, JC, TB], bf16, kind="ExternalInput").ap()
    f8 = dt.float8e4
    fc1W = nc.dram_tensor("fc1W", [128, HC, DIM], f8, kind="ExternalInput").ap()
    fc1S = nc.dram_tensor("fc1S", [128, HC], f32, kind="ExternalInput").ap()
    fc2W = nc.dram_tensor("fc2W", [128, HC, DIM], bf16, kind="ExternalInput").ap()
    fc1B = nc.dram_tensor("fc1B", [128, HC], f32, kind="ExternalInput").ap()
    fc2B = nc.dram_tensor("fc2B", [128, JC], f32, kind="ExternalInput").ap()
    yT = nc.dram_tensor("yT", [128, JC, TB], f32, kind="ExternalOutput").ap()

    with tile.TileContext(nc) as tc, ExitStack() as ctx:
        const = ctx.enter_context(tc.tile_pool(name="const", bufs=1))
        big = ctx.enter_context(tc.tile_pool(name="big", bufs=1))
        lnrows = ctx.enter_context(tc.tile_pool(name="lnrows", bufs=1))

        ones1 = const.tile([128, 1], bf16)
        nc.vector.memset(ones1[:], 1.0)
        onesP1 = const.tile([1, 128], bf16)
        nc.vector.memset(onesP1[:], 1.0)
        b1t = const.tile([128, HC], f32)
        nc.scalar.dma_start(b1t[:], fc1B)
        s1t = const.tile([128, HC], f32)
        nc.scalar.dma_start(s1t[:], fc1S)
        b2t = const.tile([128, JC], f32)
        nc.scalar.dma_start(b2t[:], fc2B)

        xtiles = big.tile([128, JC, TB], bf16)
        xn = big.tile([128, JC, TB], dt.float8e4)
        h = big.tile([128, HC, TB], bf16)

        with tc.tile_pool(name="w1_sb", bufs=HC) as w1p, \
             tc.tile_pool(name="w2_sb", bufs=1) as w2p:
            # x chunks first on the sync queue -> served first by the DMA
            # device; weights follow in need order (w1 chunks, then w2)
            for lo, hi in _NSL_B:
                nc.sync.dma_start(xtiles[:, :, lo:hi], xT[:, :, lo:hi])
            w1s = []
            for m in range(HC):
                w1m = w1p.tile([128, JC, 128], f8, tag="w1", name=f"w1_{m}")
                nc.sync.dma_start(
                    w1m[:].rearrange("p j c -> p (j c)"), fc1W[:, m, :])
                w1s.append(w1m)
            w2t = w2p.tile([128, HC, DIM], bf16)
            nc.sync.dma_start(w2t[:], fc2W)

            def load_chunk(ci, lo, hi):
                return xtiles[:, :, lo:hi]

            _standardize(nc, tc, _NSL_B, JC, load_chunk, xn, ones1, onesP1,
                         lnrows, bf16_in=True)

            with tc.tile_pool(name="f1_ps", bufs=4, space="PSUM") as f1_ps, \
                 tc.tile_pool(name="f2_ps", bufs=3, space="PSUM") as f2_ps, \
                 tc.tile_pool(name="out_sb", bufs=3) as out_sb:
                for ci, (lo, hi) in enumerate(_FC_B):
                    w = hi - lo
                    for m in range(HC):
                        pt = f1_ps.tile([128, w], f32, tag="f1")
                        for jp in range(JC // 2):
                            nc.tensor.matmul(
                                pt[:], w1s[m][:, 2 * jp:2 * jp + 2, :],
                                xn[:, 2 * jp:2 * jp + 2, lo:hi],
                                start=(jp == 0), stop=(jp == JC // 2 - 1),
                                perf_mode=mybir.MatmulPerfMode.DoubleRow)
                        # per-out-channel fp8 descale via the activation
                        # scale AP; bias applies after the scale
                        nc.scalar.activation(h[:, m, lo:hi], pt[:], AF.Gelu,
                                             bias=b1t[:, m:m + 1],
                                             scale=s1t[:, m:m + 1])
                for ci, (lo, hi) in enumerate(_FC_B):
                    for m in range(JC):
                        last = (ci == len(_FC_B) - 1 and m == JC - 1)
                        parts = ([(lo, (lo + hi) // 2), ((lo + hi) // 2, hi)]
                                 if last else [(lo, hi)])
                        for plo, phi in parts:
                            w = phi - plo
                            pt = f2_ps.tile([128, 512], f32, tag="f2")
                            for j in range(HC):
                                nc.tensor.matmul(
                                    pt[:, :w], w2t[:, j, m * 128:(m + 1) * 128],
                                    h[:, j, plo:phi],
                                    start=(j == 0), stop=(j == HC - 1))
                            ot = out_sb.tile([128, 512], f32, tag="out")
                            # ot = (psum + fc2_b) + x   in one DVE pass
                            nc.vector.scalar_tensor_tensor(
                                out=ot[:, :w], in0=pt[:, :w],
                                scalar=b2t[:, m:m + 1],
                                in1=xtiles[:, m, plo:phi],
                                op0=ALU.add, op1=ALU.add)
                            nc.sync.dma_start(yT[:, m, plo:phi], ot[:, :w])
    nc.compile()
    return nc


# ---------------- host glue ----------------

_CACHE = {}


def _get(name, builder):
    if name not in _CACHE:
        _CACHE[name] = builder()
    return _CACHE[name]


def _featmajor(a):
    """(T, 768) fp32 -> [128, 6, T]"""
    Tn = a.shape[0]
    return np.ascontiguousarray(a.T.reshape(JC, 128, Tn).transpose(1, 0, 2))


def _unfeat(aT):
    """[128, 6, T] -> (T, 768)"""
    return np.asarray(aT).transpose(1, 0, 2).reshape(DIM, -1).T


def _chunkvec(v):
    """(c*128,) -> [128, c] fp32"""
    v = np.asarray(v, np.float32)
    return np.ascontiguousarray(v.reshape(-1, 128).T)


def _wchunk(w, nchunk, dtype=BF16):
    """(768, nchunk*128) weight -> [128, nchunk, 768] (m-major chunks:
    out[p, m, j*128 + c] = w[j*128 + p, m*128 + c])."""
    w = np.asarray(w, np.float32)
    kin = w.shape[0] // 128
    out = w.reshape(kin, 128, nchunk, 128).transpose(1, 2, 0, 3)
    return np.ascontiguousarray(out.reshape(128, nchunk, kin * 128)).astype(dtype)


def _bf16(a):
    return np.asarray(a, dtype=BF16)


def _build_rel(rel_pos, ws=WS):
    """[64, 14, 14] fp8: out[c, h, k] = rel_pos[idx[h,k], c] * W8 / sqrt(SCALE)
    (qhat holds sqrt(SCALE)*q; W8 prescale is undone at the rel eviction)."""
    idx = np.arange(ws)[:, None] - np.arange(ws)[None, :] + (ws - 1)
    R = np.asarray(rel_pos, np.float32)[idx] * (W8 / SCALE ** 0.5)
    return R.transpose(2, 0, 1).astype(F8)


def _build_onehots():
    """Eh[r, :, k] = 1 if k//14 == r;  Ew[r, :, k] = 1 if k%14 == r,
    pre-expanded over the (j, win) axis for big contiguous DMA runs."""
    k = np.arange(N)
    Eh = (k[None, :] // WS == np.arange(WS)[:, None]).astype(np.float32)
    Ew = (k[None, :] % WS == np.arange(WS)[:, None]).astype(np.float32)
    Eh = np.ascontiguousarray(np.broadcast_to(Eh[:, None, :], (WS, JC * NW, N)))
    Ew = np.ascontiguousarray(np.broadcast_to(Ew[:, None, :], (WS, JC * NW, N)))
    return Eh.astype(F8), Ew.astype(F8)


kernel_last_perf = {}

try:
    from antenv.axon_hooks import get_axon_ntff_profile_hook as _hook  # noqa: F401
    _HAVE_TRACE = True
except ImportError:
    _HAVE_TRACE = False
    import os as _os
    _os.environ["BASS_NEVER_TRACE"] = "1"   # bass_utils re-reads BASS_TRACE


def window_x(x):
    """(2, 64, 64, 768) -> (56, 196, 768) padded window tokens."""
    B, H, W, C = x.shape
    xp = np.zeros((B, 70, 70, C), np.float32)
    xp[:, :64, :64] = x
    xw = xp.reshape(B, 5, WS, 5, WS, C).transpose(0, 1, 3, 2, 4, 5).reshape(50, N, C)
    xall = np.zeros((56, N, C), np.float32)
    xall[:50] = xw
    return xall


def attn_consts(norm1_w, norm1_b, qkv_w, qkv_b, proj_w, proj_b,
                rel_pos_h, rel_pos_w):
    """Host-side constant tensors for dispatch A (LN1 affine absorbed)."""
    n1w = np.asarray(norm1_w, np.float32)
    n1b = np.asarray(norm1_b, np.float32)
    qkvw = np.asarray(qkv_w, np.float32)
    qkvb = np.asarray(qkv_b, np.float32)
    Wq = n1w[:, None] * qkvw                 # (768, 2304)
    bfull = n1b @ qkvw + qkvb                # (2304,)
    Eh, Ew = _build_onehots()
    return {
        "qkW": _wchunk(Wq[:, 0:2 * DIM] * W8, 12, F8),
        "wvT": np.ascontiguousarray(
            Wq[:, 2 * DIM:].reshape(JC, 128, DIM).transpose(1, 0, 2)
            * W8).astype(F8),
        "wpT": np.ascontiguousarray(
            np.asarray(proj_w, np.float32).reshape(JC, 128, JC, 128)
            .transpose(1, 0, 2, 3) * W8).astype(F8),
        "bqT": _chunkvec(bfull[0:DIM] * SCALE ** 0.5),
        "bkT": _chunkvec(bfull[DIM:2 * DIM] * SCALE ** 0.5),
        "bvT": _bf16(bfull[2 * DIM:].reshape(1, 2, 384) * W8),
        "pbT": _chunkvec(proj_b),
        "RhT": _build_rel(rel_pos_h),
        "RwT": _build_rel(rel_pos_w),
        "EhT": Eh,
        "EwT": Ew,
    }


def mlp_consts(norm2_w, norm2_b, fc1_w, fc1_b, fc2_w, fc2_b):
    """Host-side constant tensors for dispatch B (LN2 affine absorbed)."""
    n2w = np.asarray(norm2_w, np.float32)
    n2b = np.asarray(norm2_b, np.float32)
    f1w = np.asarray(fc1_w, np.float32)
    W1 = n2w[:, None] * f1w                  # (768, 3072)
    b1 = n2b @ f1w + np.asarray(fc1_b, np.float32)
    # per-out-channel power-of-2 fp8 scaling for fc1 (exactly undone by the
    # gelu activation's per-partition scale AP)
    colmax = np.abs(W1).max(axis=0)                        # (3072,)
    sexp = np.clip(np.floor(np.log2(224.0 / np.maximum(colmax, 1e-30))),
                   -20, 20)
    wscale = np.exp2(sexp)                                 # (3072,)
    return {
        "fc1W": _wchunk(W1 * wscale[None, :], HC, F8),
        "fc1S": _chunkvec(1.0 / wscale),
        "fc2W": _bf16(np.ascontiguousarray(
            np.asarray(fc2_w, np.float32).reshape(HC, 128, DIM)
            .transpose(1, 0, 2))),
        "fc1B": _chunkvec(b1),
        "fc2B": _chunkvec(fc2_b),
    }


def kernel(x, norm1_w, norm1_b, qkv_w, qkv_b, proj_w, proj_b,
           rel_pos_h, rel_pos_w, norm2_w, norm2_b,
           fc1_w, fc1_b, fc2_w, fc2_b):
    import os
    trace = bool(os.environ.get("BASS_TRACE")) and _HAVE_TRACE
    x = np.asarray(x, np.float32)
    B, H, W, C = x.shape
    assert (B, H, W, C) == (2, 64, 64, DIM)

    # ---- dispatch A: windowed attention ----
    xall = window_x(x)
    consts_a = attn_consts(norm1_w, norm1_b, qkv_w, qkv_b, proj_w, proj_b,
                           rel_pos_h, rel_pos_w)
    with_vbias = bool(np.any(np.asarray(consts_a["bvT"], np.float32)))
    nc_a = _get(f"attn{int(with_vbias)}",
                lambda: build_attn(with_vbias=with_vbias))
    in_maps = []
    for c in range(NCORES):
        m = dict(consts_a)
        m["xT"] = _featmajor(
            xall[c * NW:(c + 1) * NW].reshape(T, C)).astype(BF16)
        in_maps.append(m)
    res_a = run_bass_kernel_spmd(nc_a, in_maps, core_ids=list(range(NCORES)),
                                 trace=trace)
    kernel_last_perf["attn"] = res_a.exec_time_ns
    xo_all = np.stack([_unfeat(res_a.results[c]["xoT"]) for c in range(NCORES)])
    xo = xo_all.reshape(56, N, C)[:50]
    xo = xo.reshape(B, 5, 5, WS, WS, C).transpose(0, 1, 3, 2, 4, 5).reshape(B, 70, 70, C)
    x2 = x + xo[:, :64, :64]

    # ---- dispatch B: MLP ----
    nc_b = _get("mlp", build_mlp)
    consts_b = mlp_consts(norm2_w, norm2_b, fc1_w, fc1_b, fc2_w, fc2_b)
    x2f = np.ascontiguousarray(x2.reshape(B * H * W, C))
    in_maps = []
    for c in range(NCORES):
        m = dict(consts_b)
        m["xT"] = _featmajor(x2f[c * TB:(c + 1) * TB]).astype(BF16)
        in_maps.append(m)
    res_b = run_bass_kernel_spmd(nc_b, in_maps, core_ids=list(range(NCORES)),
                                 trace=trace)
    kernel_last_perf["mlp"] = res_b.exec_time_ns
    y = np.concatenate([_unfeat(res_b.results[c]["yT"]) for c in range(NCORES)])
    return y.reshape(B, H, W, C).astype(np.float32)


# revision 65
# speedup vs baseline: 1.1845x; 1.0165x over previous
"""Trainium2 Bass kernel for a SAM/ViTDet-style windowed-attention transformer
block (DIM=768, 12 heads, window 14, decomposed rel-pos bias, exact-gelu MLP).

Contract: kernel(**inputs) takes the FULL unsharded inputs from
reference.setup_inputs() and returns the FULL (2, 64, 64, 768) float32 output.

Strategy (8 NeuronCores, SPMD, data-parallel):
  Dispatch A (attention): shard the 50 real windows (padded to 56) as 7
    windows/core. Per core: LN1 -> qkv -> windowed attention with the
    decomposed rel-pos bias folded into an augmented-key matmul -> proj.
  Host: window-unpartition, crop, residual add.
  Dispatch B (MLP): shard the 8192 tokens as 1024/core. Per core:
    LN2 -> fc1 -> exact GELU -> fc2 -> residual.

Perf notes (v2):
  * The LN affine (w, b) is absorbed host-side into the following matmul
    weights/biases, so on-device LN is a pure standardize: bf16 stats
    matmuls + bf16 broadcast tiles + two 4x-rate DVE tensor_tensor ops.
  * Rel-pos rows are produced by per-(h or w) batched matmuls (112 instead
    of 392) whose PSUM outputs land at partition bases 0/32/64/96, cutting
    eviction traffic.
  * The two score matmuls of one (window, head) share a [128, 392] PSUM
    tile -> a single exp instruction per head.
  * AV outputs are 6-head-batched in PSUM; the softmax normalization is one
    DVE multiply with a stride-0 (broadcast) reciprocal operand.
  * Weight DMAs are chunked and issued up front so compute starts ~10us in.

Augmented-key rel-pos layout (q/k head-pair blocks, per j-chunk):
  even head: q 0:64,  rel_h 64:78,  zeros 78:96, rel_w 96:110, zeros 110:128
  odd head:  rel_h 0:14, zeros 14:32, rel_w 32:46, zeros 46:64, q 64:128
khat holds k values in the q rows and one-hot key-position masks in the rel
rows; zero gaps make the extra contraction rows inert. S^T = khat^T qhat then
includes the decomposed bias exactly.
"""

import sys

sys.path.insert(0, "/opt/trn_rl_repo")

from contextlib import ExitStack

import numpy as np
import ml_dtypes

import concourse.bacc as bacc
import concourse.mybir as mybir
import concourse.tile as tile
from concourse.bass_utils import run_bass_kernel_spmd
from concourse.masks import make_identity

dt = mybir.dt
AF = mybir.ActivationFunctionType
ALU = mybir.AluOpType

DIM = 768
HEADS = 12
HD = 64
WS = 14
N = WS * WS          # 196 tokens / window
NW = 7               # windows per core
T = NW * N           # 1372 token slots per core (dispatch A)
TB = 1024            # tokens per core (dispatch B)
MLP = 3072
NCORES = 8
JC = DIM // 128      # 6 feature chunks
HC = MLP // 128      # 24 hidden chunks
EPS = 1e-5
SCALE = HD ** -0.5   # 0.125
TPAD = 1376          # T rounded up so fp8 DoubleRow pair strides are 16B-aligned
NPAD = 208           # N rounded up likewise (aT)
BF16 = ml_dtypes.bfloat16
F8 = ml_dtypes.float8_e4m3
W8 = 64.0   # fp8 weight pre-scale (avoids e4m3 subnormals); undone at eviction

# window-aligned token chunks for dispatch A (2+2+2+1 windows)
_NSL = [(0, 392), (392, 784), (784, 1176), (1176, 1372)]
_NSL_LN = [(i * N, (i + 1) * N) for i in range(NW)]        # LN chunks, A
_NSL_B = [(0, 256), (256, 512), (512, 768), (768, 1024)]   # LN chunks, B
_FC_B = [(0, 512), (512, 1024)]                            # matmul chunks, B
_SUBS = [(0, 128), (128, 196)]                             # within-window subchunks

# augmented-key row layout per parity: (q_lo, relh_lo, relw_lo)
_EVEN = (0, 64, 96)    # q 0:64,  rel rows above
_ODD = (64, 0, 32)     # q 64:128, rel rows below


def _standardize(nc, tc, nsl_list, jc, load_chunk, xn, ones1, onesP1, rows_p,
                 cast_engine="gpsimd", bf16_in=False):
    """Pure LN standardize: xn[:, j, c] = (x - mu[c]) * rsig[c], bf16 out.

    load_chunk(ci, lo, hi) -> AP [128, jc, w] for that chunk (may DMA into a
    fresh tile or return a view of a resident one); fp32 unless bf16_in (then
    it is used directly, no cast).  Stats run as bf16 matmuls vs a ones
    vector; mu/rsig are kept as bf16 rows, broadcast across partitions via
    tiny bf16 matmuls, evicted to bf16 SBUF and applied with two DVE
    tensor_tensor ops (all-bf16, stride-1 -> DVE fast mode).
    ones1: [128,1] bf16 ones; onesP1: [1,128] bf16 ones; rows_p: pool for rows.
    """
    nch = len(nsl_list)
    dimn = jc * 128

    eng_cast = getattr(nc, cast_engine)

    epsr = rows_p.tile([1, 1], dt.float32, tag="epsr")
    nc.vector.memset(epsr[:], EPS)

    with tc.tile_pool(name="ln_xb", bufs=nch) as xbp, \
         tc.tile_pool(name="ln_sq", bufs=2) as sqp, \
         tc.tile_pool(name="ln_st", bufs=2, space="PSUM") as st_ps, \
         tc.tile_pool(name="ln_bc", bufs=2, space="PSUM") as bc_ps, \
         tc.tile_pool(name="ln_bcs", bufs=4) as bcs, \
         tc.tile_pool(name="ln_rows", bufs=nch + 1) as rp:
        # per-chunk stats emitted with the apply of the PREVIOUS chunk
        # interleaved (one-chunk lag): PE runs stats back to back while the
        # row math / broadcast / apply of the prior chunk drains on Act/DVE
        xbs, mus, rss = [], [], []

        def emit_stats(ci, lo, hi):
            w = hi - lo
            xt = load_chunk(ci, lo, hi)
            if bf16_in:
                xb = xt
            else:
                xb = xbp.tile([128, jc, w], dt.bfloat16, tag="xb",
                              name=f"xb{ci}")
                eng_cast.tensor_copy(xb[:], xt)
                xb = xb[:]
            xbs.append(xb)
            # per-token sums -> mu
            pmu = st_ps.tile([1, w], dt.float32, tag="st")
            for j in range(jc):
                nc.tensor.matmul(pmu[:], ones1[:], xb[:, j, :],
                                 start=(j == 0), stop=(j == jc - 1))
            mu = rp.tile([1, w], dt.bfloat16, tag="mu", name=f"mu{ci}")
            nc.scalar.activation(mu[:], pmu[:], AF.Copy, scale=1.0 / dimn)
            mus.append(mu)
            # per-token sum of squares -> E[x^2]
            sq = sqp.tile([128, jc, w], dt.bfloat16, tag="sq")
            nc.vector.tensor_tensor(out=sq[:], in0=xb, in1=xb, op=ALU.mult)
            pmq = st_ps.tile([1, w], dt.float32, tag="st")
            for j in range(jc):
                nc.tensor.matmul(pmq[:], ones1[:], sq[:, j, :],
                                 start=(j == 0), stop=(j == jc - 1))
            mq = rp.tile([1, w], dt.float32, tag="mq")
            nc.scalar.activation(mq[:], pmq[:], AF.Copy, scale=1.0 / dimn)
            # rsig = 1/sqrt(E[x^2] - mu^2 + eps)
            m2 = rp.tile([1, w], dt.float32, tag="m2")
            nc.vector.tensor_tensor(out=m2[:], in0=mu[:], in1=mu[:],
                                    op=ALU.mult)
            nc.vector.tensor_tensor(out=mq[:], in0=mq[:], in1=m2[:],
                                    op=ALU.subtract)
            sd = rp.tile([1, w], dt.float32, tag="sd")
            nc.scalar.activation(sd[:], mq[:], AF.Sqrt, bias=epsr[:])
            rsig = rp.tile([1, w], dt.bfloat16, tag="rs", name=f"rs{ci}")
            with nc.allow_low_precision(reason="bf16 rsig row; 0.4% rel err "
                                        "matches the bf16 matmul noise "
                                        "floor"):
                nc.vector.reciprocal(rsig[:], sd[:])
            rss.append(rsig)

        def emit_apply(ci, lo, hi):
            w = hi - lo
            xb, mu, rsig = xbs[ci], mus[ci], rss[ci]
            bmu_p = bc_ps.tile([128, w], dt.float32, tag="bc")
            nc.tensor.matmul(bmu_p[:], onesP1[:], mu[:], start=True, stop=True)
            brs_p = bc_ps.tile([128, w], dt.float32, tag="bc")
            nc.tensor.matmul(brs_p[:], onesP1[:], rsig[:], start=True,
                             stop=True)
            bmu = bcs.tile([128, w], dt.bfloat16, tag="bmu")
            nc.scalar.copy(bmu[:], bmu_p[:])
            brs = bcs.tile([128, w], dt.bfloat16, tag="brs")
            nc.scalar.copy(brs[:], brs_p[:])
            # xn = (x - mu) * rsig   (two all-bf16 DVE ops, j-broadcast)
            cen = sqp.tile([128, jc, w], dt.bfloat16, tag="cen")
            nc.vector.tensor_tensor(
                out=cen[:], in0=xb,
                in1=bmu[:].unsqueeze(1).to_broadcast([128, jc, w]),
                op=ALU.subtract)
            with nc.allow_low_precision(reason="xn storage dtype (bf16/fp8) "
                                        "is the matmul operand precision"):
                nc.vector.tensor_tensor(
                    out=xn[:, :, lo:hi], in0=cen[:],
                    in1=brs[:].unsqueeze(1).to_broadcast([128, jc, w]),
                    op=ALU.mult)

        # chunk 0's apply is emitted right after its stats so the first xn
        # chunk (the qk-phase gate) is produced as early as possible
        emit_stats(0, *nsl_list[0])
        emit_apply(0, *nsl_list[0])
        for ci in range(1, nch):
            emit_stats(ci, *nsl_list[ci])
        for ci in range(1, nch):
            emit_apply(ci, *nsl_list[ci])


def build_attn(with_vbias=True):
    """Dispatch A: LN1 + qkv + windowed attention (+rel-pos) + proj."""
    nc = bacc.Bacc("TRN2", target_bir_lowering=False, debug=False)
    f32, bf16 = dt.float32, dt.bfloat16

    xT = nc.dram_tensor("xT", [128, JC, T], bf16, kind="ExternalInput").ap()
    f8 = dt.float8e4
    qkW = nc.dram_tensor("qkW", [128, 12, JC * 128], f8, kind="ExternalInput").ap()
    wvT = nc.dram_tensor("wvT", [128, JC, DIM], f8, kind="ExternalInput").ap()
    wpT = nc.dram_tensor("wpT", [128, JC, JC, 128], f8, kind="ExternalInput").ap()
    bqT = nc.dram_tensor("bqT", [128, JC], f32, kind="ExternalInput").ap()
    bkT = nc.dram_tensor("bkT", [128, JC], f32, kind="ExternalInput").ap()
    bvT = nc.dram_tensor("bvT", [1, 2, 384], bf16, kind="ExternalInput").ap()
    pbT = nc.dram_tensor("pbT", [128, JC], f32, kind="ExternalInput").ap()
    RhT = nc.dram_tensor("RhT", [64, WS, WS], f8, kind="ExternalInput").ap()
    RwT = nc.dram_tensor("RwT", [64, WS, WS], f8, kind="ExternalInput").ap()
    EhT = nc.dram_tensor("EhT", [WS, JC * NW, N], f8, kind="ExternalInput").ap()
    EwT = nc.dram_tensor("EwT", [WS, JC * NW, N], f8, kind="ExternalInput").ap()
    xoT = nc.dram_tensor("xoT", [128, JC, T], f32, kind="ExternalOutput").ap()

    with tile.TileContext(nc) as tc, ExitStack() as ctx:
        const = ctx.enter_context(tc.tile_pool(name="const", bufs=1))
        big = ctx.enter_context(tc.tile_pool(name="big", bufs=1))
        lnrows = ctx.enter_context(tc.tile_pool(name="lnrows", bufs=1))

        # ---- big persistent tensors (declared first so memsets start at t=0)
        # qhat/khat live in a partition-paired fp8 layout [64, two, j, t]:
        # logical contraction row r maps to (r % 64, r // 64), so the score
        # matmuls run fp8 DoubleRow.  Plane assignment per parity:
        #   even head: q = plane 0, rel_h rows 0:14 / rel_w 32:46 on plane 1
        #   odd head:  q = plane 1, rel_h rows 0:14 / rel_w 32:46 on plane 0
        xn = big.tile([128, JC, TPAD], dt.float8e4)
        qhE = big.tile([64, 2, JC, TPAD], dt.float8e4)
        khE = big.tile([64, 2, JC, TPAD], dt.float8e4)
        qhB = big.tile([64, 2, JC, TPAD], dt.float8e4)
        khB = big.tile([64, 2, JC, TPAD], dt.float8e4)
        vtok = big.tile([128, NW, 2, HEADS, HD + 1], bf16)

        # ---- constants ----
        ones1 = const.tile([128, 1], bf16)
        nc.vector.memset(ones1[:], 1.0)
        onesP1 = const.tile([1, 128], bf16)
        nc.vector.memset(onesP1[:], 1.0)
        onesT = const.tile([1, 128], bf16)
        nc.vector.memset(onesT[:], 1.0)
        ident = const.tile([128, 128], bf16)
        make_identity(nc, ident[:])

        rh = const.tile([64, WS, WS], f8)
        nc.scalar.dma_start(rh[:], RhT)
        rw = const.tile([64, WS, WS], f8)
        nc.scalar.dma_start(rw[:], RwT)
        bq = const.tile([128, JC], f32)
        nc.scalar.dma_start(bq[:], bqT)
        bk = const.tile([128, JC], f32)
        nc.scalar.dma_start(bk[:], bkT)
        bv = const.tile([1, 2, 384], bf16)
        nc.scalar.dma_start(bv[:], bvT)
        pb = const.tile([128, JC], f32)
        nc.scalar.dma_start(pb[:], pbT)
        # wv/wp tiles are created here but their loads are issued on the sync
        # queue after the x/qk-weight DMAs so the global DMA device serves x
        # first (sync-queue program order == DMA device order).
        wv = const.tile([128, JC, DIM], f8)
        wp = const.tile([128, JC, JC, 128], f8)

        # ---- LN1 (pure standardize; affine absorbed into weights) ----
        # x arrives already bf16 (host cast) -> no on-device cast, half DMA;
        # window-sized chunks get the first xn out early for the qk start
        with tc.tile_pool(name="ln_x", bufs=4) as xp:
            def load_chunk(ci, lo, hi):
                xt = xp.tile([128, JC, hi - lo], bf16, tag="x", name=f"x{ci}")
                nc.sync.dma_start(xt[:], xT[:, :, lo:hi])
                return xt[:]
            _standardize(nc, tc, _NSL, JC, load_chunk, xn, ones1, onesP1,
                         lnrows, bf16_in=True)

        # dummy exp right after LN: the Act engine loads the exp activation
        # table here (post-LN idle) instead of serially at the core start
        dume = const.tile([1, 1], f32)
        nc.scalar.activation(dume[:], ones1[0:1, :], AF.Exp)

        # zero the rel/one-hot halves (gaps must be exactly 0; rel rows and
        # one-hot rows overlay these ranges later).  Issued after the LN body
        # so the Pool queue serves the LN casts first; the Tile deps still
        # order these before the mask DMAs / rel evictions below.
        nc.gpsimd.memset(khE[:, 1, :, :], 0.0)
        nc.gpsimd.memset(khB[:, 0, :, :], 0.0)
        nc.gpsimd.memset(qhE[:, 1, :, :], 0.0)
        nc.gpsimd.memset(qhB[:, 0, :, :], 0.0)
        # ones column in vtok (AV matmul also yields the softmax denominator)
        nc.gpsimd.memset(
            vtok[:].rearrange("p w s h o -> p (w s h) o")[:, :, HD:HD + 1], 1.0)

        # ---- q/k (feature-major, split by parity) + v (token-major) ----
        # PE program order is tuned so the in-order PE queue never waits on
        # slow producers: qk c0,c1 | v w0,w1 | qk c2,c3 | rel matmuls |
        # v w2..w6 (covers the rel-eviction drain) | pipelined core.
        _WHALF = [(0, 4), (4, 7)]
        with tc.tile_pool(name="qk_w", bufs=12) as wqk_sb, \
             tc.tile_pool(name="qk_stg", bufs=2) as stg_sb, \
             tc.tile_pool(name="qk_ps", bufs=2, space="PSUM") as qk_ps, \
             tc.tile_pool(name="v_ps", bufs=2, space="PSUM") as v_ps, \
             tc.tile_pool(name="rel_ps", bufs=4, space="PSUM") as rel_ps:
            wms = []
            for m in range(12):
                wm = wqk_sb.tile([128, JC, 128], f8, tag="wqk", name=f"w{m}")
                nc.sync.dma_start(
                    wm[:].rearrange("p j c -> p (j c)"), qkW[:, m, :])
                wms.append(wm)
            nc.sync.dma_start(wv[:], wvT)
            nc.sync.dma_start(wp[:], wpT)
            # one-hot key-position masks into khat rel rows; issued last on
            # the sync queue so x/weight transfers win the DMA device first
            for (msrc, mdst, tw, r0) in ((EhT, khE, 1, 0), (EwT, khE, 1, 32),
                                         (EhT, khB, 0, 0), (EwT, khB, 0, 32)):
                nc.sync.dma_start(
                    mdst[r0:r0 + WS, tw, :, 0:T].rearrange(
                        "p j (w n) -> p j w n", n=N),
                    msrc.rearrange("p (j w) n -> p j w n", w=NW))

            def emit_qk_chunk(ci):
                lo, hi = _NSL[ci]
                w = hi - lo
                # full-height evicts into per-chunk staging tiles (the bias
                # AP is per-partition, so one op covers both parity halves);
                # per chunk just 4 SBUF->SBUF DMAs distribute the halves
                qstg = stg_sb.tile([128, JC, 392], dt.float8e4, tag="qstg")
                kstg = stg_sb.tile([128, JC, 392], dt.float8e4, tag="kstg")
                for m in range(12):
                    is_q = m < JC
                    e = m % JC
                    pt = qk_ps.tile([128, 392], f32, tag="qk")
                    for jp in range(JC // 2):
                        nc.tensor.matmul(pt[:, :w],
                                         wms[m][:, 2 * jp:2 * jp + 2, :],
                                         xn[:, 2 * jp:2 * jp + 2, lo:hi],
                                         start=(jp == 0),
                                         stop=(jp == JC // 2 - 1),
                                         perf_mode=mybir.MatmulPerfMode.DoubleRow)
                    if is_q:
                        nc.scalar.activation(qstg[:, e, :w], pt[:, :w],
                                             AF.Identity, bias=bq[:, m:m + 1],
                                             scale=SCALE ** 0.5 / W8)
                    else:
                        nc.vector.tensor_scalar(
                            out=kstg[:, e, :w], in0=pt[:, :w],
                            scalar1=SCALE ** 0.5 / W8, scalar2=bk[:, e:e + 1],
                            op0=ALU.mult, op1=ALU.add)
                nc.sync.dma_start(qhE[:, 0, :, lo:hi], qstg[0:64, :, :w])
                nc.sync.dma_start(qhB[:, 1, :, lo:hi], qstg[64:128, :, :w])
                nc.sync.dma_start(khE[:, 0, :, lo:hi], kstg[0:64, :, :w])
                nc.sync.dma_start(khB[:, 1, :, lo:hi], kstg[64:128, :, :w])

            def emit_v(win):
                for si, (slo, shi) in enumerate(_SUBS):
                    ssz = shi - slo
                    base = win * N + slo
                    for half in range(2):
                        pv = v_ps.tile([128, 384], f32, tag="v")
                        for jp in range(JC // 2):
                            nc.tensor.matmul(
                                pv[:ssz, :],
                                xn[:, 2 * jp:2 * jp + 2, base:base + ssz],
                                wv[:, 2 * jp:2 * jp + 2,
                                   half * 384:(half + 1) * 384],
                                start=(jp == 0),
                                stop=(not with_vbias
                                      and jp == JC // 2 - 1),
                                perf_mode=mybir.MatmulPerfMode.DoubleRow)
                        if with_vbias:
                            nc.tensor.matmul(
                                pv[:ssz, :], onesT[:, :ssz], bv[:, half, :],
                                start=False, stop=True)
                        if (si + half) % 2 == 0:
                            nc.scalar.activation(
                                vtok[0:ssz, win, si,
                                     6 * half:6 * half + 6, 0:HD],
                                pv[:ssz, :].rearrange("p (h d) -> p h d",
                                                      d=HD),
                                AF.Copy, scale=1.0 / W8)
                        else:
                            nc.vector.tensor_scalar(
                                out=vtok[0:ssz, win, si,
                                         6 * half:6 * half + 6, 0:HD],
                                in0=pv[:ssz, :].rearrange(
                                    "p (h d) -> p h d", d=HD),
                                scalar1=1.0 / W8, scalar2=None, op0=ALU.mult)

            def emit_rel():
                # rel-pos rows into qhat planes, batched per h (rel_h) / per
                # w (rel_w); window halves keep PSUM cols <= 336.  Both
                # parities read q from partitions 0:64 of their q plane and
                # write rel rows 0:14 (rel_h) / 32:46 (rel_w) of the other.
                ri = 0
                for (w0, w1) in _WHALF:   # window halves outermost: the core
                    # can start on windows 0..3 while half 4..7 still drains
                    for par in range(2):
                        qh = qhE if par == 0 else qhB
                        qtw = 0 if par == 0 else 1       # q plane
                        rtw = 1 - qtw                    # rel plane
                        for typ in range(2):
                            r0 = 0 if typ == 0 else 32
                            stat = rh if typ == 0 else rw
                            for hh in range(WS):
                                nwn = w1 - w0
                                if typ == 0:
                                    mov = qh[:, qtw, :, 0:T].rearrange(
                                        "p j (win n) -> p j win n", n=N)[
                                        :, :, w0:w1, hh * WS:(hh + 1) * WS]
                                else:
                                    mov = qh[:, qtw, :, 0:T].rearrange(
                                        "p j (win kh kw) -> p j win kh kw",
                                        kh=WS, kw=WS)[:, :, w0:w1, :, hh]
                                ncols = JC * nwn * WS
                                prel = rel_ps.tile([128, 336], f32, tag="rel")
                                # skip_group_check: sim-only guard; its
                                # flat-address region view aliases across
                                # banks for <128-partition outputs
                                nc.tensor.matmul(
                                    prel[r0:r0 + WS, :ncols],
                                    stat[0:64, hh, :], mov,
                                    start=True, stop=True,
                                    tile_position=(0, r0),
                                    skip_group_check=True)
                                if typ == 0:
                                    dst = qh[r0:r0 + WS, rtw, :, 0:T].rearrange(
                                        "p j (win n) -> p j win n", n=N)[
                                        :, :, w0:w1, hh * WS:(hh + 1) * WS]
                                else:
                                    dst = qh[r0:r0 + WS, rtw, :, 0:T].rearrange(
                                        "p j (win kh kw) -> p j win kh kw",
                                        kh=WS, kw=WS)[:, :, w0:w1, :, hh]
                                src = prel[r0:r0 + WS, :ncols].rearrange(
                                    "p (j win k) -> p j win k", j=JC, win=nwn)
                                with nc.allow_low_precision(
                                        reason="fp8 rel rows; absolute score "
                                        "error ~3e-3 vs budget 0.1"):
                                    if ri % 2 == 0:
                                        nc.scalar.activation(
                                            dst, src, AF.Copy, scale=1.0 / W8)
                                    else:
                                        nc.vector.tensor_scalar(
                                            out=dst, in0=src,
                                            scalar1=1.0 / W8, scalar2=None,
                                            op0=ALU.mult)
                                ri += 1

            emit_qk_chunk(0)
            emit_qk_chunk(1)
            emit_v(0)
            emit_v(1)
            emit_qk_chunk(2)
            emit_qk_chunk(3)
            emit_rel()
            for win in range(2, NW):
                emit_v(win)

        # ---- attention core + proj: software-pipelined across windows ----
        # per iteration: AV+normalize(w) | scores+exp(w+1) | transp+proj(w);
        # window w+1's score matmuls keep PE busy while w's softmax
        # normalization drains on DVE.
        with tc.tile_pool(name="s_ps", bufs=1, space="PSUM") as s_ps, \
             tc.tile_pool(name="av_ps", bufs=2, space="PSUM") as av_ps, \
             tc.tile_pool(name="t_ps", bufs=2, space="PSUM") as t_ps, \
             tc.tile_pool(name="pj_ps", bufs=2, space="PSUM") as pj_ps, \
             tc.tile_pool(name="pt_sb", bufs=26) as pt_sb, \
             tc.tile_pool(name="ao_sb", bufs=2) as ao_sb, \
             tc.tile_pool(name="at_sb", bufs=2) as at_sb, \
             tc.tile_pool(name="xo_sb", bufs=2) as xo_sb, \
             tc.tile_pool(name="r_sb", bufs=4) as r_sb:
            # two persistent score tiles, rotated manually: the exp reads the
            # full [128, 392] tile, so the region no matmul covers (rows
            # 68:128 of the second key chunk) is zeroed exactly once
            sbufs = []
            for i in range(2):
                st = s_ps.tile([128, 392], f32, tag=f"s{i}", name=f"s{i}")
                # partition start must be 32-aligned on PSUM; rows 64:68 are
                # re-written by every second score matmul afterwards
                nc.vector.memset(st[64:128, 196:392], 0.0)
                sbufs.append(st)
            state = {"hidx": 0}

            def emit_scores(win):
                pts = []
                for head in range(HEADS):
                    blk = head // 2
                    par = head % 2
                    qh = qhE if par == 0 else qhB
                    kh = khE if par == 0 else khB
                    ps_t = sbufs[state["hidx"] % 2]
                    state["hidx"] += 1
                    for si, (slo, shi) in enumerate(_SUBS):
                        ssz = shi - slo
                        nc.tensor.matmul(
                            ps_t[:ssz, si * N:si * N + N],
                            kh[:, :, blk, win * N + slo:win * N + shi],
                            qh[:, :, blk, win * N:(win + 1) * N],
                            start=True, stop=True,
                            perf_mode=mybir.MatmulPerfMode.DoubleRow)
                    ptile = pt_sb.tile([128, 392], bf16, tag="pt")
                    nc.scalar.activation(ptile[:], ps_t[:], AF.Exp)
                    pts.append(ptile)
                return pts

            def emit_av(win, pts):
                ao0 = ao_sb.tile([128, DIM], bf16, tag="ao0")
                ao1 = ao_sb.tile([68, DIM], bf16, tag="ao1")
                for qi, (qlo, qhi) in enumerate(_SUBS):
                    qsz = qhi - qlo
                    ao = ao0 if qi == 0 else ao1
                    for hg in range(2):
                        pav = av_ps.tile([128, 6, HD + 1], f32, tag="av")
                        for hl in range(6):
                            head = hg * 6 + hl
                            for si, (slo, shi) in enumerate(_SUBS):
                                ssz = shi - slo
                                nc.tensor.matmul(
                                    pav[:qsz, hl, :],
                                    pts[head][0:ssz, si * N + qlo:si * N + qhi],
                                    vtok[0:ssz, win, si, head, :],
                                    start=(si == 0), stop=(si == 1))
                        rec = r_sb.tile([128, 6], f32, tag="rec")
                        nc.vector.reciprocal(
                            rec[:qsz, :],
                            pav[:qsz, :, HD:HD + 1].rearrange(
                                "p h o -> p (h o)"))
                        nc.vector.tensor_tensor(
                            out=ao[0:qsz, hg * 384:(hg + 1) * 384].rearrange(
                                "p (h d) -> p h d", d=HD),
                            in0=pav[:qsz, :, 0:HD],
                            in1=rec[:qsz, :].unsqueeze(2).to_broadcast(
                                [qsz, 6, HD]),
                            op=ALU.mult)
                return ao0, ao1

            def emit_transp_proj(win, ao0, ao1):
                aT = at_sb.tile([128, JC, NPAD], dt.float8e4, tag="at")
                ti = 0
                for j in range(JC):
                    for qi, (qlo, qhi) in enumerate(_SUBS):
                        qsz = qhi - qlo
                        src = ao0 if qi == 0 else ao1
                        ptt = t_ps.tile([128, 128], bf16, tag="tp")
                        nc.tensor.transpose(ptt[:, :qsz],
                                            src[0:qsz, j * 128:(j + 1) * 128],
                                            ident[0:qsz, 0:qsz])
                        if False:
                            nc.scalar.copy(aT[:, j, qlo:qhi], ptt[:, :qsz])
                        else:
                            nc.vector.tensor_copy(aT[:, j, qlo:qhi],
                                                  ptt[:, :qsz])
                        ti += 1
                xo_t = xo_sb.tile([128, JC, N], f32, tag="xo")
                for m in range(JC):
                    pp = pj_ps.tile([128, N], f32, tag="pj")
                    for jp in range(JC // 2):
                        nc.tensor.matmul(
                            pp[:], wp[:, 2 * jp:2 * jp + 2, m, :],
                            aT[:, 2 * jp:2 * jp + 2, 0:N],
                            start=(jp == 0), stop=(jp == JC // 2 - 1),
                            perf_mode=mybir.MatmulPerfMode.DoubleRow)
                    if m % 2 == 0:
                        nc.scalar.activation(xo_t[:, m, :], pp[:],
                                             AF.Identity,
                                             bias=pb[:, m:m + 1],
                                             scale=1.0 / W8)
                    else:
                        nc.vector.tensor_scalar(
                            out=xo_t[:, m, :], in0=pp[:],
                            scalar1=1.0 / W8, scalar2=pb[:, m:m + 1],
                            op0=ALU.mult, op1=ALU.add)
                nc.sync.dma_start(
                    xoT.rearrange("p j (w n) -> p j w n", n=N)[:, :, win, :],
                    xo_t[:])

            pts = emit_scores(0)
            for win in range(NW):
                ao0, ao1 = emit_av(win, pts)
                if win + 1 < NW:
                    pts = emit_scores(win + 1)
                emit_transp_proj(win, ao0, ao1)
    nc.compile()
    return nc


def build_mlp():
    """Dispatch B: y = x + fc2(gelu(fc1(LN2(x)))), 1024 tokens/core.
    LN2 affine is absorbed into fc1 host-side."""
    nc = bacc.Bacc("TRN2", target_bir_lowering=False, debug=False)
    f32, bf16 = dt.float32, dt.bfloat16

    xT = nc.dram_tensor("xT", [128, JC, TB], bf16, kind="ExternalInput").ap()
    f8 = dt.float8e4
    fc1W = nc.dram_tensor("fc1W", [128, HC, DIM], f8, kind="ExternalInput").ap()
    fc1S = nc.dram_tensor("fc1S", [128, HC], f32, kind="ExternalInput").ap()
    fc2W = nc.dram_tensor("fc2W", [128, HC, DIM], bf16, kind="ExternalInput").ap()
    fc1B = nc.dram_tensor("fc1B", [128, HC], f32, kind="ExternalInput").ap()
    fc2B = nc.dram_tensor("fc2B", [128, JC], f32, kind="ExternalInput").ap()
    yT = nc.dram_tensor("yT", [128, JC, TB], f32, kind="ExternalOutput").ap()

    with tile.TileContext(nc) as tc, ExitStack() as ctx:
        const = ctx.enter_context(tc.tile_pool(name="const", bufs=1))
        big = ctx.enter_context(tc.tile_pool(name="big", bufs=1))
        lnrows = ctx.enter_context(tc.tile_pool(name="lnrows", bufs=1))

        ones1 = const.tile([128, 1], bf16)
        nc.vector.memset(ones1[:], 1.0)
        onesP1 = const.tile([1, 128], bf16)
        nc.vector.memset(onesP1[:], 1.0)
        b1t = const.tile([128, HC], f32)
        nc.scalar.dma_start(b1t[:], fc1B)
        s1t = const.tile([128, HC], f32)
        nc.scalar.dma_start(s1t[:], fc1S)
        b2t = const.tile([128, JC], f32)
        nc.scalar.dma_start(b2t[:], fc2B)

        xtiles = big.tile([128, JC, TB], bf16)
        xn = big.tile([128, JC, TB], dt.float8e4)
        h = big.tile([128, HC, TB], bf16)

        with tc.tile_pool(name="w1_sb", bufs=HC) as w1p, \
             tc.tile_pool(name="w2_sb", bufs=1) as w2p:
            # x chunks first on the sync queue -> served first by the DMA
            # device; weights follow in need order (w1 chunks, then w2)
            for lo, hi in _NSL_B:
                nc.sync.dma_start(xtiles[:, :, lo:hi], xT[:, :, lo:hi])
            w1s = []
            for m in range(HC):
                w1m = w1p.tile([128, JC, 128], f8, tag="w1", name=f"w1_{m}")
                nc.sync.dma_start(
                    w1m[:].rearrange("p j c -> p (j c)"), fc1W[:, m, :])
                w1s.append(w1m)
            w2t = w2p.tile([128, HC, DIM], bf16)
            nc.sync.dma_start(w2t[:], fc2W)

            def load_chunk(ci, lo, hi):
                return xtiles[:, :, lo:hi]

            _standardize(nc, tc, _NSL_B, JC, load_chunk, xn, ones1, onesP1,
                         lnrows, bf16_in=True)
            # dummy gelu: load the gelu activation table during LN-tail idle
            # instead of serially before the first fc1 eviction
            dumg = const.tile([1, 1], f32)
            nc.scalar.activation(dumg[:], ones1[0:1, :], AF.Gelu)

            with tc.tile_pool(name="f1_ps", bufs=4, space="PSUM") as f1_ps, \
                 tc.tile_pool(name="f2_ps", bufs=3, space="PSUM") as f2_ps, \
                 tc.tile_pool(name="out_sb", bufs=3) as out_sb:
                for ci, (lo, hi) in enumerate(_FC_B):
                    w = hi - lo
                    for m in range(HC):
                        pt = f1_ps.tile([128, w], f32, tag="f1")
                        for jp in range(JC // 2):
                            nc.tensor.matmul(
                                pt[:], w1s[m][:, 2 * jp:2 * jp + 2, :],
                                xn[:, 2 * jp:2 * jp + 2, lo:hi],
                                start=(jp == 0), stop=(jp == JC // 2 - 1),
                                perf_mode=mybir.MatmulPerfMode.DoubleRow)
                        # per-out-channel fp8 descale via the activation
                        # scale AP; bias applies after the scale
                        nc.scalar.activation(h[:, m, lo:hi], pt[:], AF.Gelu,
                                             bias=b1t[:, m:m + 1],
                                             scale=s1t[:, m:m + 1])
                for ci, (lo, hi) in enumerate(_FC_B):
                    for m in range(JC):
                        last = (ci == len(_FC_B) - 1 and m == JC - 1)
                        parts = ([(lo, (lo + hi) // 2), ((lo + hi) // 2, hi)]
                                 if last else [(lo, hi)])
                        for plo, phi in parts:
                            w = phi - plo
                            pt = f2_ps.tile([128, 512], f32, tag="f2")
                            for j in range(HC):
                                nc.tensor.matmul(
                                    pt[:, :w], w2t[:, j, m * 128:(m + 1) * 128],
                                    h[:, j, plo:phi],
                                    start=(j == 0), stop=(j == HC - 1))
                            ot = out_sb.tile([128, 512], f32, tag="out")
                            # ot = (psum + fc2_b) + x   in one DVE pass
                            nc.vector.scalar_tensor_tensor(
                                out=ot[:, :w], in0=pt[:, :w],
                                scalar=b2t[:, m:m + 1],
                                in1=xtiles[:, m, plo:phi],
                                op0=ALU.add, op1=ALU.add)
                            nc.sync.dma_start(yT[:, m, plo:phi], ot[:, :w])
    nc.compile()
    return nc


# ---------------- host glue ----------------

_CACHE = {}


def _get(name, builder):
    if name not in _CACHE:
        _CACHE[name] = builder()
    return _CACHE[name]


def _featmajor(a):
    """(T, 768) fp32 -> [128, 6, T]"""
    Tn = a.shape[0]
    return np.ascontiguousarray(a.T.reshape(JC, 128, Tn).transpose(1, 0, 2))


def _unfeat(aT):
    """[128, 6, T] -> (T, 768)"""
    return np.asarray(aT).transpose(1, 0, 2).reshape(DIM, -1).T


def _chunkvec(v):
    """(c*128,) -> [128, c] fp32"""
    v = np.asarray(v, np.float32)
    return np.ascontiguousarray(v.reshape(-1, 128).T)


def _wchunk(w, nchunk, dtype=BF16):
    """(768, nchunk*128) weight -> [128, nchunk, 768] (m-major chunks:
    out[p, m, j*128 + c] = w[j*128 + p, m*128 + c])."""
    w = np.asarray(w, np.float32)
    kin = w.shape[0] // 128
    out = w.reshape(kin, 128, nchunk, 128).transpose(1, 2, 0, 3)
    return np.ascontiguousarray(out.reshape(128, nchunk, kin * 128)).astype(dtype)


def _bf16(a):
    return np.asarray(a, dtype=BF16)


def _build_rel(rel_pos, ws=WS):
    """[64, 14, 14] fp8: out[c, h, k] = rel_pos[idx[h,k], c] * W8 / sqrt(SCALE)
    (qhat holds sqrt(SCALE)*q; W8 prescale is undone at the rel eviction)."""
    idx = np.arange(ws)[:, None] - np.arange(ws)[None, :] + (ws - 1)
    R = np.asarray(rel_pos, np.float32)[idx] * (W8 / SCALE ** 0.5)
    return R.transpose(2, 0, 1).astype(F8)


def _build_onehots():
    """Eh[r, :, k] = 1 if k//14 == r;  Ew[r, :, k] = 1 if k%14 == r,
    pre-expanded over the (j, win) axis for big contiguous DMA runs."""
    k = np.arange(N)
    Eh = (k[None, :] // WS == np.arange(WS)[:, None]).astype(np.float32)
    Ew = (k[None, :] % WS == np.arange(WS)[:, None]).astype(np.float32)
    Eh = np.ascontiguousarray(np.broadcast_to(Eh[:, None, :], (WS, JC * NW, N)))
    Ew = np.ascontiguousarray(np.broadcast_to(Ew[:, None, :], (WS, JC * NW, N)))
    return Eh.astype(F8), Ew.astype(F8)


kernel_last_perf = {}

try:
    from antenv.axon_hooks import get_axon_ntff_profile_hook as _hook  # noqa: F401
    _HAVE_TRACE = True
except ImportError:
    _HAVE_TRACE = False
    import os as _os
    _os.environ["BASS_NEVER_TRACE"] = "1"   # bass_utils re-reads BASS_TRACE


def window_x(x):
    """(2, 64, 64, 768) -> (56, 196, 768) padded window tokens."""
    B, H, W, C = x.shape
    xp = np.zeros((B, 70, 70, C), np.float32)
    xp[:, :64, :64] = x
    xw = xp.reshape(B, 5, WS, 5, WS, C).transpose(0, 1, 3, 2, 4, 5).reshape(50, N, C)
    xall = np.zeros((56, N, C), np.float32)
    xall[:50] = xw
    return xall


def attn_consts(norm1_w, norm1_b, qkv_w, qkv_b, proj_w, proj_b,
                rel_pos_h, rel_pos_w):
    """Host-side constant tensors for dispatch A (LN1 affine absorbed)."""
    n1w = np.asarray(norm1_w, np.float32)
    n1b = np.asarray(norm1_b, np.float32)
    qkvw = np.asarray(qkv_w, np.float32)
    qkvb = np.asarray(qkv_b, np.float32)
    Wq = n1w[:, None] * qkvw                 # (768, 2304)
    bfull = n1b @ qkvw + qkvb                # (2304,)
    Eh, Ew = _build_onehots()
    return {
        "qkW": _wchunk(Wq[:, 0:2 * DIM] * W8, 12, F8),
        "wvT": np.ascontiguousarray(
            Wq[:, 2 * DIM:].reshape(JC, 128, DIM).transpose(1, 0, 2)
            * W8).astype(F8),
        "wpT": np.ascontiguousarray(
            np.asarray(proj_w, np.float32).reshape(JC, 128, JC, 128)
            .transpose(1, 0, 2, 3) * W8).astype(F8),
        "bqT": _chunkvec(bfull[0:DIM] * SCALE ** 0.5),
        "bkT": _chunkvec(bfull[DIM:2 * DIM] * SCALE ** 0.5),
        "bvT": _bf16(bfull[2 * DIM:].reshape(1, 2, 384) * W8),
        "pbT": _chunkvec(proj_b),
        "RhT": _build_rel(rel_pos_h),
        "RwT": _build_rel(rel_pos_w),
        "EhT": Eh,
        "EwT": Ew,
    }


def mlp_consts(norm2_w, norm2_b, fc1_w, fc1_b, fc2_w, fc2_b):
    """Host-side constant tensors for dispatch B (LN2 affine absorbed)."""
    n2w = np.asarray(norm2_w, np.float32)
    n2b = np.asarray(norm2_b, np.float32)
    f1w = np.asarray(fc1_w, np.float32)
    W1 = n2w[:, None] * f1w                  # (768, 3072)
    b1 = n2b @ f1w + np.asarray(fc1_b, np.float32)
    # per-out-channel power-of-2 fp8 scaling for fc1 (exactly undone by the
    # gelu activation's per-partition scale AP)
    colmax = np.abs(W1).max(axis=0)                        # (3072,)
    sexp = np.clip(np.floor(np.log2(224.0 / np.maximum(colmax, 1e-30))),
                   -20, 20)
    wscale = np.exp2(sexp)                                 # (3072,)
    return {
        "fc1W": _wchunk(W1 * wscale[None, :], HC, F8),
        "fc1S": _chunkvec(1.0 / wscale),
        "fc2W": _bf16(np.ascontiguousarray(
            np.asarray(fc2_w, np.float32).reshape(HC, 128, DIM)
            .transpose(1, 0, 2))),
        "fc1B": _chunkvec(b1),
        "fc2B": _chunkvec(fc2_b),
    }


def kernel(x, norm1_w, norm1_b, qkv_w, qkv_b, proj_w, proj_b,
           rel_pos_h, rel_pos_w, norm2_w, norm2_b,
           fc1_w, fc1_b, fc2_w, fc2_b):
    import os
    trace = bool(os.environ.get("BASS_TRACE")) and _HAVE_TRACE
    x = np.asarray(x, np.float32)
    B, H, W, C = x.shape
    assert (B, H, W, C) == (2, 64, 64, DIM)

    # ---- dispatch A: windowed attention ----
    xall = window_x(x)
    consts_a = attn_consts(norm1_w, norm1_b, qkv_w, qkv_b, proj_w, proj_b,
                           rel_pos_h, rel_pos_w)
    with_vbias = bool(np.any(np.asarray(consts_a["bvT"], np.float32)))
    nc_a = _get(f"attn{int(with_vbias)}",
                lambda: build_attn(with_vbias=with_vbias))
    in_maps = []
    for c in range(NCORES):
        m = dict(consts_a)
        m["xT"] = _featmajor(
            xall[c * NW:(c + 1) * NW].reshape(T, C)).astype(BF16)
        in_maps.append(m)
    res_a = run_bass_kernel_spmd(nc_a, in_maps, core_ids=list(range(NCORES)),
                                 trace=trace)
    kernel_last_perf["attn"] = res_a.exec_time_ns
    xo_all = np.stack([_unfeat(res_a.results[c]["xoT"]) for c in range(NCORES)])
    xo = xo_all.reshape(56, N, C)[:50]
    xo = xo.reshape(B, 5, 5, WS, WS, C).transpose(0, 1, 3, 2, 4, 5).reshape(B, 70, 70, C)
    x2 = x + xo[:, :64, :64]

    # ---- dispatch B: MLP ----
    nc_b = _get("mlp", build_mlp)
    consts_b = mlp_consts(norm2_w, norm2_b, fc1_w, fc1_b, fc2_w, fc2_b)
    x2f = np.ascontiguousarray(x2.reshape(B * H * W, C))
    in_maps = []
    for c in range(NCORES):
        m = dict(consts_b)
        m["xT"] = _featmajor(x2f[c * TB:(c + 1) * TB]).astype(BF16)
        in_maps.append(m)
    res_b = run_bass_kernel_spmd(nc_b, in_maps, core_ids=list(range(NCORES)),
                                 trace=trace)
    kernel_last_perf["mlp"] = res_b.exec_time_ns
    y = np.concatenate([_unfeat(res_b.results[c]["yT"]) for c in range(NCORES)])
    return y.reshape(B, H, W, C).astype(np.float32)


# revision 74
# speedup vs baseline: 1.1963x; 1.0100x over previous
"""Trainium2 Bass kernel for a SAM/ViTDet-style windowed-attention transformer
block (DIM=768, 12 heads, window 14, decomposed rel-pos bias, exact-gelu MLP).

Contract: kernel(**inputs) takes the FULL unsharded inputs from
reference.setup_inputs() and returns the FULL (2, 64, 64, 768) float32 output.

Strategy (8 NeuronCores, SPMD, data-parallel):
  Dispatch A (attention): shard the 50 real windows (padded to 56) as 7
    windows/core. Per core: LN1 -> qkv -> windowed attention with the
    decomposed rel-pos bias folded into an augmented-key matmul -> proj.
  Host: window-unpartition, crop, residual add.
  Dispatch B (MLP): shard the 8192 tokens as 1024/core. Per core:
    LN2 -> fc1 -> exact GELU -> fc2 -> residual.

Perf notes (v2):
  * The LN affine (w, b) is absorbed host-side into the following matmul
    weights/biases, so on-device LN is a pure standardize: bf16 stats
    matmuls + bf16 broadcast tiles + two 4x-rate DVE tensor_tensor ops.
  * Rel-pos rows are produced by per-(h or w) batched matmuls (112 instead
    of 392) whose PSUM outputs land at partition bases 0/32/64/96, cutting
    eviction traffic.
  * The two score matmuls of one (window, head) share a [128, 392] PSUM
    tile -> a single exp instruction per head.
  * AV outputs are 6-head-batched in PSUM; the softmax normalization is one
    DVE multiply with a stride-0 (broadcast) reciprocal operand.
  * Weight DMAs are chunked and issued up front so compute starts ~10us in.

Augmented-key rel-pos layout (q/k head-pair blocks, per j-chunk):
  even head: q 0:64,  rel_h 64:78,  zeros 78:96, rel_w 96:110, zeros 110:128
  odd head:  rel_h 0:14, zeros 14:32, rel_w 32:46, zeros 46:64, q 64:128
khat holds k values in the q rows and one-hot key-position masks in the rel
rows; zero gaps make the extra contraction rows inert. S^T = khat^T qhat then
includes the decomposed bias exactly.
"""

import sys

sys.path.insert(0, "/opt/trn_rl_repo")

from contextlib import ExitStack

import numpy as np
import ml_dtypes

import concourse.bacc as bacc
import concourse.mybir as mybir
import concourse.tile as tile
from concourse.bass_utils import run_bass_kernel_spmd
from concourse.masks import make_identity

dt = mybir.dt
AF = mybir.ActivationFunctionType
ALU = mybir.AluOpType

DIM = 768
HEADS = 12
HD = 64
WS = 14
N = WS * WS          # 196 tokens / window
NW = 7               # windows per core
T = NW * N           # 1372 token slots per core (dispatch A)
TB = 1024            # tokens per core (dispatch B)
MLP = 3072
NCORES = 8
JC = DIM // 128      # 6 feature chunks
HC = MLP // 128      # 24 hidden chunks
EPS = 1e-5
SCALE = HD ** -0.5   # 0.125
TPAD = 1376          # T rounded up so fp8 DoubleRow pair strides are 16B-aligned
NPAD = 208           # N rounded up likewise (aT)
BF16 = ml_dtypes.bfloat16
F8 = ml_dtypes.float8_e4m3
W8 = 64.0   # fp8 weight pre-scale (avoids e4m3 subnormals); undone at eviction

# window-aligned token chunks for dispatch A (2+2+2+1 windows)
_NSL = [(0, 392), (392, 784), (784, 1176), (1176, 1372)]
_NSL_LN = [(i * N, (i + 1) * N) for i in range(NW)]        # LN chunks, A
_NSL_B = [(0, 256), (256, 512), (512, 768), (768, 1024)]   # LN chunks, B
_FC_B = [(0, 512), (512, 1024)]                            # matmul chunks, B
_SUBS = [(0, 128), (128, 196)]                             # within-window subchunks

# augmented-key row layout per parity: (q_lo, relh_lo, relw_lo)
_EVEN = (0, 64, 96)    # q 0:64,  rel rows above
_ODD = (64, 0, 32)     # q 64:128, rel rows below


def _standardize(nc, tc, nsl_list, jc, load_chunk, xn, ones1, onesP1, rows_p,
                 cast_engine="gpsimd", bf16_in=False):
    """Pure LN standardize: xn[:, j, c] = (x - mu[c]) * rsig[c], bf16 out.

    load_chunk(ci, lo, hi) -> AP [128, jc, w] for that chunk (may DMA into a
    fresh tile or return a view of a resident one); fp32 unless bf16_in (then
    it is used directly, no cast).  Stats run as bf16 matmuls vs a ones
    vector; mu/rsig are kept as bf16 rows, broadcast across partitions via
    tiny bf16 matmuls, evicted to bf16 SBUF and applied with two DVE
    tensor_tensor ops (all-bf16, stride-1 -> DVE fast mode).
    ones1: [128,1] bf16 ones; onesP1: [1,128] bf16 ones; rows_p: pool for rows.
    """
    nch = len(nsl_list)
    dimn = jc * 128

    eng_cast = getattr(nc, cast_engine)

    epsr = rows_p.tile([1, 1], dt.float32, tag="epsr")
    nc.vector.memset(epsr[:], EPS)

    with tc.tile_pool(name="ln_xb", bufs=nch) as xbp, \
         tc.tile_pool(name="ln_sq", bufs=2) as sqp, \
         tc.tile_pool(name="ln_st", bufs=2, space="PSUM") as st_ps, \
         tc.tile_pool(name="ln_bc", bufs=2, space="PSUM") as bc_ps, \
         tc.tile_pool(name="ln_bcs", bufs=4) as bcs, \
         tc.tile_pool(name="ln_rows", bufs=nch + 1) as rp:
        # per-chunk stats emitted with the apply of the PREVIOUS chunk
        # interleaved (one-chunk lag): PE runs stats back to back while the
        # row math / broadcast / apply of the prior chunk drains on Act/DVE
        xbs, mus, rss = [], [], []

        def emit_stats(ci, lo, hi):
            w = hi - lo
            xt = load_chunk(ci, lo, hi)
            if bf16_in:
                xb = xt
            else:
                xb = xbp.tile([128, jc, w], dt.bfloat16, tag="xb",
                              name=f"xb{ci}")
                eng_cast.tensor_copy(xb[:], xt)
                xb = xb[:]
            xbs.append(xb)
            # per-token sums -> mu
            pmu = st_ps.tile([1, w], dt.float32, tag="st")
            for j in range(jc):
                nc.tensor.matmul(pmu[:], ones1[:], xb[:, j, :],
                                 start=(j == 0), stop=(j == jc - 1))
            mu = rp.tile([1, w], dt.bfloat16, tag="mu", name=f"mu{ci}")
            nc.scalar.activation(mu[:], pmu[:], AF.Copy, scale=1.0 / dimn)
            mus.append(mu)
            # per-token sum of squares -> E[x^2]
            sq = sqp.tile([128, jc, w], dt.bfloat16, tag="sq")
            nc.vector.tensor_tensor(out=sq[:], in0=xb, in1=xb, op=ALU.mult)
            pmq = st_ps.tile([1, w], dt.float32, tag="st")
            for j in range(jc):
                nc.tensor.matmul(pmq[:], ones1[:], sq[:, j, :],
                                 start=(j == 0), stop=(j == jc - 1))
            mq = rp.tile([1, w], dt.float32, tag="mq")
            nc.scalar.activation(mq[:], pmq[:], AF.Copy, scale=1.0 / dimn)
            # rsig = 1/sqrt(E[x^2] - mu^2 + eps)
            m2 = rp.tile([1, w], dt.float32, tag="m2")
            nc.vector.tensor_tensor(out=m2[:], in0=mu[:], in1=mu[:],
                                    op=ALU.mult)
            nc.vector.tensor_tensor(out=mq[:], in0=mq[:], in1=m2[:],
                                    op=ALU.subtract)
            sd = rp.tile([1, w], dt.float32, tag="sd")
            nc.scalar.activation(sd[:], mq[:], AF.Sqrt, bias=epsr[:])
            rsig = rp.tile([1, w], dt.bfloat16, tag="rs", name=f"rs{ci}")
            with nc.allow_low_precision(reason="bf16 rsig row; 0.4% rel err "
                                        "matches the bf16 matmul noise "
                                        "floor"):
                nc.vector.reciprocal(rsig[:], sd[:])
            rss.append(rsig)

        def emit_apply(ci, lo, hi):
            w = hi - lo
            xb, mu, rsig = xbs[ci], mus[ci], rss[ci]
            bmu_p = bc_ps.tile([128, w], dt.float32, tag="bc")
            nc.tensor.matmul(bmu_p[:], onesP1[:], mu[:], start=True, stop=True)
            brs_p = bc_ps.tile([128, w], dt.float32, tag="bc")
            nc.tensor.matmul(brs_p[:], onesP1[:], rsig[:], start=True,
                             stop=True)
            bmu = bcs.tile([128, w], dt.bfloat16, tag="bmu")
            nc.scalar.copy(bmu[:], bmu_p[:])
            brs = bcs.tile([128, w], dt.bfloat16, tag="brs")
            nc.scalar.copy(brs[:], brs_p[:])
            # xn = (x - mu) * rsig   (two all-bf16 DVE ops, j-broadcast)
            cen = sqp.tile([128, jc, w], dt.bfloat16, tag="cen")
            nc.vector.tensor_tensor(
                out=cen[:], in0=xb,
                in1=bmu[:].unsqueeze(1).to_broadcast([128, jc, w]),
                op=ALU.subtract)
            with nc.allow_low_precision(reason="xn storage dtype (bf16/fp8) "
                                        "is the matmul operand precision"):
                nc.vector.tensor_tensor(
                    out=xn[:, :, lo:hi], in0=cen[:],
                    in1=brs[:].unsqueeze(1).to_broadcast([128, jc, w]),
                    op=ALU.mult)

        # chunk 0's apply is emitted right after its stats so the first xn
        # chunk (the qk-phase gate) is produced as early as possible
        emit_stats(0, *nsl_list[0])
        emit_apply(0, *nsl_list[0])
        for ci in range(1, nch):
            emit_stats(ci, *nsl_list[ci])
        for ci in range(1, nch):
            emit_apply(ci, *nsl_list[ci])


def build_attn(with_vbias=True):
    """Dispatch A: LN1 + qkv + windowed attention (+rel-pos) + proj."""
    nc = bacc.Bacc("TRN2", target_bir_lowering=False, debug=False)
    f32, bf16 = dt.float32, dt.bfloat16

    xT = nc.dram_tensor("xT", [128, JC, T], bf16, kind="ExternalInput").ap()
    f8 = dt.float8e4
    qkW = nc.dram_tensor("qkW", [128, 12, JC * 128], f8, kind="ExternalInput").ap()
    wvT = nc.dram_tensor("wvT", [128, JC, DIM], f8, kind="ExternalInput").ap()
    wpT = nc.dram_tensor("wpT", [128, JC, JC, 128], f8, kind="ExternalInput").ap()
    bqT = nc.dram_tensor("bqT", [128, JC], f32, kind="ExternalInput").ap()
    bkT = nc.dram_tensor("bkT", [128, JC], f32, kind="ExternalInput").ap()
    bvT = nc.dram_tensor("bvT", [1, 2, 384], bf16, kind="ExternalInput").ap()
    pbT = nc.dram_tensor("pbT", [128, JC], f32, kind="ExternalInput").ap()
    RhT = nc.dram_tensor("RhT", [64, WS, WS], f8, kind="ExternalInput").ap()
    RwT = nc.dram_tensor("RwT", [64, WS, WS], f8, kind="ExternalInput").ap()
    EhT = nc.dram_tensor("EhT", [WS, JC * NW, N], f8, kind="ExternalInput").ap()
    EwT = nc.dram_tensor("EwT", [WS, JC * NW, N], f8, kind="ExternalInput").ap()
    xoT = nc.dram_tensor("xoT", [128, JC, T], f32, kind="ExternalOutput").ap()

    with tile.TileContext(nc) as tc, ExitStack() as ctx:
        const = ctx.enter_context(tc.tile_pool(name="const", bufs=1))
        big = ctx.enter_context(tc.tile_pool(name="big", bufs=1))
        lnrows = ctx.enter_context(tc.tile_pool(name="lnrows", bufs=1))

        # ---- big persistent tensors (declared first so memsets start at t=0)
        # qhat/khat live in a partition-paired fp8 layout [64, two, j, t]:
        # logical contraction row r maps to (r % 64, r // 64), so the score
        # matmuls run fp8 DoubleRow.  Plane assignment per parity:
        #   even head: q = plane 0, rel_h rows 0:14 / rel_w 32:46 on plane 1
        #   odd head:  q = plane 1, rel_h rows 0:14 / rel_w 32:46 on plane 0
        xn = big.tile([128, JC, TPAD], dt.float8e4)
        qhE = big.tile([64, 2, JC, TPAD], dt.float8e4)
        khE = big.tile([64, 2, JC, TPAD], dt.float8e4)
        qhB = big.tile([64, 2, JC, TPAD], dt.float8e4)
        khB = big.tile([64, 2, JC, TPAD], dt.float8e4)
        vtok = big.tile([128, NW, 2, HEADS, HD + 1], bf16)

        # ---- constants ----
        ones1 = const.tile([128, 1], bf16)
        nc.vector.memset(ones1[:], 1.0)
        onesP1 = const.tile([1, 128], bf16)
        nc.vector.memset(onesP1[:], 1.0)
        onesT = const.tile([1, 128], bf16)
        nc.vector.memset(onesT[:], 1.0)
        ident = const.tile([128, 128], bf16)
        make_identity(nc, ident[:])

        rh = const.tile([64, WS, WS], f8)
        nc.scalar.dma_start(rh[:], RhT)
        rw = const.tile([64, WS, WS], f8)
        nc.scalar.dma_start(rw[:], RwT)
        bq = const.tile([128, JC], f32)
        nc.scalar.dma_start(bq[:], bqT)
        bk = const.tile([128, JC], f32)
        nc.scalar.dma_start(bk[:], bkT)
        bv = const.tile([1, 2, 384], bf16)
        nc.scalar.dma_start(bv[:], bvT)
        pb = const.tile([128, JC], f32)
        nc.scalar.dma_start(pb[:], pbT)
        # wv/wp tiles are created here but their loads are issued on the sync
        # queue after the x/qk-weight DMAs so the global DMA device serves x
        # first (sync-queue program order == DMA device order).
        wv = const.tile([128, JC, DIM], f8)
        wp = const.tile([128, JC, JC, 128], f8)

        # ---- LN1 (pure standardize; affine absorbed into weights) ----
        # x arrives already bf16 (host cast) -> no on-device cast, half DMA;
        # window-sized chunks get the first xn out early for the qk start
        with tc.tile_pool(name="ln_x", bufs=4) as xp:
            def load_chunk(ci, lo, hi):
                xt = xp.tile([128, JC, hi - lo], bf16, tag="x", name=f"x{ci}")
                nc.sync.dma_start(xt[:], xT[:, :, lo:hi])
                return xt[:]
            _standardize(nc, tc, _NSL, JC, load_chunk, xn, ones1, onesP1,
                         lnrows, bf16_in=True)

        # dummy exp right after LN: the Act engine loads the exp activation
        # table here (post-LN idle) instead of serially at the core start
        dume = const.tile([1, 1], f32)
        nc.scalar.activation(dume[:], ones1[0:1, :], AF.Exp)

        # zero the rel/one-hot halves (gaps must be exactly 0; rel rows and
        # one-hot rows overlay these ranges later).  Issued after the LN body
        # so the Pool queue serves the LN casts first; the Tile deps still
        # order these before the mask DMAs / rel evictions below.
        nc.gpsimd.memset(khE[:, 1, :, :], 0.0)
        nc.gpsimd.memset(khB[:, 0, :, :], 0.0)
        nc.gpsimd.memset(qhE[:, 1, :, :], 0.0)
        nc.gpsimd.memset(qhB[:, 0, :, :], 0.0)
        # ones column in vtok (AV matmul also yields the softmax denominator)
        nc.gpsimd.memset(
            vtok[:].rearrange("p w s h o -> p (w s h) o")[:, :, HD:HD + 1], 1.0)

        # ---- q/k (feature-major, split by parity) + v (token-major) ----
        # PE program order is tuned so the in-order PE queue never waits on
        # slow producers: qk c0,c1 | v w0,w1 | qk c2,c3 | rel matmuls |
        # v w2..w6 (covers the rel-eviction drain) | pipelined core.
        _WHALF = [(0, 4), (4, 7)]
        with tc.tile_pool(name="qk_w", bufs=12) as wqk_sb, \
             tc.tile_pool(name="qk_stg", bufs=2) as stg_sb, \
             tc.tile_pool(name="qk_ps", bufs=2, space="PSUM") as qk_ps, \
             tc.tile_pool(name="v_ps", bufs=2, space="PSUM") as v_ps, \
             tc.tile_pool(name="rel_ps", bufs=4, space="PSUM") as rel_ps:
            wms = []
            for m in range(12):
                wm = wqk_sb.tile([128, JC, 128], f8, tag="wqk", name=f"w{m}")
                nc.sync.dma_start(
                    wm[:].rearrange("p j c -> p (j c)"), qkW[:, m, :])
                wms.append(wm)
            nc.sync.dma_start(wv[:], wvT)
            nc.sync.dma_start(wp[:], wpT)
            # one-hot key-position masks into khat rel rows; issued last on
            # the sync queue so x/weight transfers win the DMA device first
            for (msrc, mdst, tw, r0) in ((EhT, khE, 1, 0), (EwT, khE, 1, 32),
                                         (EhT, khB, 0, 0), (EwT, khB, 0, 32)):
                nc.sync.dma_start(
                    mdst[r0:r0 + WS, tw, :, 0:T].rearrange(
                        "p j (w n) -> p j w n", n=N),
                    msrc.rearrange("p (j w) n -> p j w n", w=NW))

            def emit_qk_chunk(ci):
                lo, hi = _NSL[ci]
                w = hi - lo
                # full-height evicts into per-chunk staging tiles (the bias
                # AP is per-partition, so one op covers both parity halves);
                # per chunk just 4 SBUF->SBUF DMAs distribute the halves
                qstg = stg_sb.tile([128, JC, 392], dt.float8e4, tag="qstg")
                kstg = stg_sb.tile([128, JC, 392], dt.float8e4, tag="kstg")
                for m in range(12):
                    is_q = m < JC
                    e = m % JC
                    pt = qk_ps.tile([128, 392], f32, tag="qk")
                    for jp in range(JC // 2):
                        nc.tensor.matmul(pt[:, :w],
                                         wms[m][:, 2 * jp:2 * jp + 2, :],
                                         xn[:, 2 * jp:2 * jp + 2, lo:hi],
                                         start=(jp == 0),
                                         stop=(jp == JC // 2 - 1),
                                         perf_mode=mybir.MatmulPerfMode.DoubleRow)
                    if is_q:
                        nc.scalar.activation(qstg[:, e, :w], pt[:, :w],
                                             AF.Identity, bias=bq[:, m:m + 1],
                                             scale=SCALE ** 0.5 / W8)
                    else:
                        nc.vector.tensor_scalar(
                            out=kstg[:, e, :w], in0=pt[:, :w],
                            scalar1=SCALE ** 0.5 / W8, scalar2=bk[:, e:e + 1],
                            op0=ALU.mult, op1=ALU.add)
                nc.sync.dma_start(qhE[:, 0, :, lo:hi], qstg[0:64, :, :w])
                nc.sync.dma_start(qhB[:, 1, :, lo:hi], qstg[64:128, :, :w])
                nc.sync.dma_start(khE[:, 0, :, lo:hi], kstg[0:64, :, :w])
                nc.sync.dma_start(khB[:, 1, :, lo:hi], kstg[64:128, :, :w])

            def emit_v(win):
                for si, (slo, shi) in enumerate(_SUBS):
                    ssz = shi - slo
                    base = win * N + slo
                    for half in range(2):
                        pv = v_ps.tile([128, 384], f32, tag="v")
                        for jp in range(JC // 2):
                            nc.tensor.matmul(
                                pv[:ssz, :],
                                xn[:, 2 * jp:2 * jp + 2, base:base + ssz],
                                wv[:, 2 * jp:2 * jp + 2,
                                   half * 384:(half + 1) * 384],
                                start=(jp == 0),
                                stop=(not with_vbias
                                      and jp == JC // 2 - 1),
                                perf_mode=mybir.MatmulPerfMode.DoubleRow)
                        if with_vbias:
                            nc.tensor.matmul(
                                pv[:ssz, :], onesT[:, :ssz], bv[:, half, :],
                                start=False, stop=True)
                        if True:
                            nc.scalar.activation(
                                vtok[0:ssz, win, si,
                                     6 * half:6 * half + 6, 0:HD],
                                pv[:ssz, :].rearrange("p (h d) -> p h d",
                                                      d=HD),
                                AF.Copy, scale=1.0 / W8)
                        else:
                            nc.vector.tensor_scalar(
                                out=vtok[0:ssz, win, si,
                                         6 * half:6 * half + 6, 0:HD],
                                in0=pv[:ssz, :].rearrange(
                                    "p (h d) -> p h d", d=HD),
                                scalar1=1.0 / W8, scalar2=None, op0=ALU.mult)

            def emit_rel():
                # rel-pos rows into qhat planes, batched per h (rel_h) / per
                # w (rel_w); window halves keep PSUM cols <= 336.  Both
                # parities read q from partitions 0:64 of their q plane and
                # write rel rows 0:14 (rel_h) / 32:46 (rel_w) of the other.
                ri = 0
                for (w0, w1) in _WHALF:   # window halves outermost: the core
                    # can start on windows 0..3 while half 4..7 still drains
                    for par in range(2):
                        qh = qhE if par == 0 else qhB
                        qtw = 0 if par == 0 else 1       # q plane
                        rtw = 1 - qtw                    # rel plane
                        for typ in range(2):
                            r0 = 0 if typ == 0 else 32
                            stat = rh if typ == 0 else rw
                            for hh in range(WS):
                                nwn = w1 - w0
                                if typ == 0:
                                    mov = qh[:, qtw, :, 0:T].rearrange(
                                        "p j (win n) -> p j win n", n=N)[
                                        :, :, w0:w1, hh * WS:(hh + 1) * WS]
                                else:
                                    mov = qh[:, qtw, :, 0:T].rearrange(
                                        "p j (win kh kw) -> p j win kh kw",
                                        kh=WS, kw=WS)[:, :, w0:w1, :, hh]
                                ncols = JC * nwn * WS
                                prel = rel_ps.tile([128, 336], f32, tag="rel")
                                # skip_group_check: sim-only guard; its
                                # flat-address region view aliases across
                                # banks for <128-partition outputs
                                nc.tensor.matmul(
                                    prel[r0:r0 + WS, :ncols],
                                    stat[0:64, hh, :], mov,
                                    start=True, stop=True,
                                    tile_position=(0, r0),
                                    skip_group_check=True)
                                if typ == 0:
                                    dst = qh[r0:r0 + WS, rtw, :, 0:T].rearrange(
                                        "p j (win n) -> p j win n", n=N)[
                                        :, :, w0:w1, hh * WS:(hh + 1) * WS]
                                else:
                                    dst = qh[r0:r0 + WS, rtw, :, 0:T].rearrange(
                                        "p j (win kh kw) -> p j win kh kw",
                                        kh=WS, kw=WS)[:, :, w0:w1, :, hh]
                                src = prel[r0:r0 + WS, :ncols].rearrange(
                                    "p (j win k) -> p j win k", j=JC, win=nwn)
                                with nc.allow_low_precision(
                                        reason="fp8 rel rows; absolute score "
                                        "error ~3e-3 vs budget 0.1"):
                                    if ri % 2 == 0:
                                        nc.scalar.activation(
                                            dst, src, AF.Copy, scale=1.0 / W8)
                                    else:
                                        nc.vector.tensor_scalar(
                                            out=dst, in0=src,
                                            scalar1=1.0 / W8, scalar2=None,
                                            op0=ALU.mult)
                                ri += 1

            emit_qk_chunk(0)
            emit_qk_chunk(1)
            emit_v(0)
            emit_v(1)
            emit_qk_chunk(2)
            emit_qk_chunk(3)
            emit_rel()
            for win in range(2, NW):
                emit_v(win)

        # ---- attention core + proj: software-pipelined across windows ----
        # per iteration: AV+normalize(w) | scores+exp(w+1) | transp+proj(w);
        # window w+1's score matmuls keep PE busy while w's softmax
        # normalization drains on DVE.
        with tc.tile_pool(name="s_ps", bufs=1, space="PSUM") as s_ps, \
             tc.tile_pool(name="av_ps", bufs=2, space="PSUM") as av_ps, \
             tc.tile_pool(name="t_ps", bufs=2, space="PSUM") as t_ps, \
             tc.tile_pool(name="pj_ps", bufs=2, space="PSUM") as pj_ps, \
             tc.tile_pool(name="pt_sb", bufs=26) as pt_sb, \
             tc.tile_pool(name="ao_sb", bufs=2) as ao_sb, \
             tc.tile_pool(name="at_sb", bufs=2) as at_sb, \
             tc.tile_pool(name="xo_sb", bufs=2) as xo_sb, \
             tc.tile_pool(name="r_sb", bufs=4) as r_sb:
            # two persistent score tiles, rotated manually: the exp reads the
            # full [128, 392] tile, so the region no matmul covers (rows
            # 68:128 of the second key chunk) is zeroed exactly once
            sbufs = []
            for i in range(2):
                st = s_ps.tile([128, 392], f32, tag=f"s{i}", name=f"s{i}")
                # partition start must be 32-aligned on PSUM; rows 64:68 are
                # re-written by every second score matmul afterwards
                nc.vector.memset(st[64:128, 196:392], 0.0)
                sbufs.append(st)
            state = {"hidx": 0}

            def emit_scores(win):
                pts = []
                for head in range(HEADS):
                    blk = head // 2
                    par = head % 2
                    qh = qhE if par == 0 else qhB
                    kh = khE if par == 0 else khB
                    ps_t = sbufs[state["hidx"] % 2]
                    state["hidx"] += 1
                    for si, (slo, shi) in enumerate(_SUBS):
                        ssz = shi - slo
                        nc.tensor.matmul(
                            ps_t[:ssz, si * N:si * N + N],
                            kh[:, :, blk, win * N + slo:win * N + shi],
                            qh[:, :, blk, win * N:(win + 1) * N],
                            start=True, stop=True,
                            perf_mode=mybir.MatmulPerfMode.DoubleRow)
                    ptile = pt_sb.tile([128, 392], bf16, tag="pt")
                    nc.scalar.activation(ptile[:], ps_t[:], AF.Exp)
                    pts.append(ptile)
                return pts

            def emit_av(win, pts):
                ao0 = ao_sb.tile([128, DIM], bf16, tag="ao0")
                ao1 = ao_sb.tile([68, DIM], bf16, tag="ao1")
                for qi, (qlo, qhi) in enumerate(_SUBS):
                    qsz = qhi - qlo
                    ao = ao0 if qi == 0 else ao1
                    for hg in range(2):
                        pav = av_ps.tile([128, 6, HD + 1], f32, tag="av")
                        for hl in range(6):
                            head = hg * 6 + hl
                            for si, (slo, shi) in enumerate(_SUBS):
                                ssz = shi - slo
                                nc.tensor.matmul(
                                    pav[:qsz, hl, :],
                                    pts[head][0:ssz, si * N + qlo:si * N + qhi],
                                    vtok[0:ssz, win, si, head, :],
                                    start=(si == 0), stop=(si == 1))
                        rec = r_sb.tile([128, 6], f32, tag="rec")
                        nc.vector.reciprocal(
                            rec[:qsz, :],
                            pav[:qsz, :, HD:HD + 1].rearrange(
                                "p h o -> p (h o)"))
                        nc.vector.tensor_tensor(
                            out=ao[0:qsz, hg * 384:(hg + 1) * 384].rearrange(
                                "p (h d) -> p h d", d=HD),
                            in0=pav[:qsz, :, 0:HD],
                            in1=rec[:qsz, :].unsqueeze(2).to_broadcast(
                                [qsz, 6, HD]),
                            op=ALU.mult)
                return ao0, ao1

            def emit_transp_proj(win, ao0, ao1):
                aT = at_sb.tile([128, JC, NPAD], dt.float8e4, tag="at")
                ti = 0
                for j in range(JC):
                    for qi, (qlo, qhi) in enumerate(_SUBS):
                        qsz = qhi - qlo
                        src = ao0 if qi == 0 else ao1
                        ptt = t_ps.tile([128, 128], bf16, tag="tp")
                        nc.tensor.transpose(ptt[:, :qsz],
                                            src[0:qsz, j * 128:(j + 1) * 128],
                                            ident[0:qsz, 0:qsz])
                        if False:
                            nc.scalar.copy(aT[:, j, qlo:qhi], ptt[:, :qsz])
                        else:
                            nc.vector.tensor_copy(aT[:, j, qlo:qhi],
                                                  ptt[:, :qsz])
                        ti += 1
                xo_t = xo_sb.tile([128, JC, N], f32, tag="xo")
                for m in range(JC):
                    pp = pj_ps.tile([128, N], f32, tag="pj")
                    for jp in range(JC // 2):
                        nc.tensor.matmul(
                            pp[:], wp[:, 2 * jp:2 * jp + 2, m, :],
                            aT[:, 2 * jp:2 * jp + 2, 0:N],
                            start=(jp == 0), stop=(jp == JC // 2 - 1),
                            perf_mode=mybir.MatmulPerfMode.DoubleRow)
                    if m % 2 == 0:
                        nc.scalar.activation(xo_t[:, m, :], pp[:],
                                             AF.Identity,
                                             bias=pb[:, m:m + 1],
                                             scale=1.0 / W8)
                    else:
                        nc.vector.tensor_scalar(
                            out=xo_t[:, m, :], in0=pp[:],
                            scalar1=1.0 / W8, scalar2=pb[:, m:m + 1],
                            op0=ALU.mult, op1=ALU.add)
                nc.sync.dma_start(
                    xoT.rearrange("p j (w n) -> p j w n", n=N)[:, :, win, :],
                    xo_t[:])

            pts = emit_scores(0)
            for win in range(NW):
                ao0, ao1 = emit_av(win, pts)
                if win + 1 < NW:
                    pts = emit_scores(win + 1)
                emit_transp_proj(win, ao0, ao1)
    nc.compile()
    return nc


def build_mlp():
    """Dispatch B: y = x + fc2(gelu(fc1(LN2(x)))), 1024 tokens/core.
    LN2 affine is absorbed into fc1 host-side."""
    nc = bacc.Bacc("TRN2", target_bir_lowering=False, debug=False)
    f32, bf16 = dt.float32, dt.bfloat16

    xT = nc.dram_tensor("xT", [128, JC, TB], bf16, kind="ExternalInput").ap()
    f8 = dt.float8e4
    fc1W = nc.dram_tensor("fc1W", [128, HC, DIM], f8, kind="ExternalInput").ap()
    fc1S = nc.dram_tensor("fc1S", [128, HC], f32, kind="ExternalInput").ap()
    fc2W = nc.dram_tensor("fc2W", [128, HC, DIM], bf16, kind="ExternalInput").ap()
    fc1B = nc.dram_tensor("fc1B", [128, HC], f32, kind="ExternalInput").ap()
    fc2B = nc.dram_tensor("fc2B", [128, JC], f32, kind="ExternalInput").ap()
    yT = nc.dram_tensor("yT", [128, JC, TB], f32, kind="ExternalOutput").ap()

    with tile.TileContext(nc) as tc, ExitStack() as ctx:
        const = ctx.enter_context(tc.tile_pool(name="const", bufs=1))
        big = ctx.enter_context(tc.tile_pool(name="big", bufs=1))
        lnrows = ctx.enter_context(tc.tile_pool(name="lnrows", bufs=1))

        ones1 = const.tile([128, 1], bf16)
        nc.vector.memset(ones1[:], 1.0)
        onesP1 = const.tile([1, 128], bf16)
        nc.vector.memset(onesP1[:], 1.0)
        b1t = const.tile([128, HC], f32)
        nc.scalar.dma_start(b1t[:], fc1B)
        s1t = const.tile([128, HC], f32)
        nc.scalar.dma_start(s1t[:], fc1S)
        b2t = const.tile([128, JC], f32)
        nc.scalar.dma_start(b2t[:], fc2B)

        xtiles = big.tile([128, JC, TB], bf16)
        xn = big.tile([128, JC, TB], dt.float8e4)
        h = big.tile([128, HC, TB], bf16)

        with tc.tile_pool(name="w1_sb", bufs=HC) as w1p, \
             tc.tile_pool(name="w2_sb", bufs=1) as w2p:
            # x chunks first on the sync queue -> served first by the DMA
            # device; weights follow in need order (w1 chunks, then w2)
            for lo, hi in _NSL_B:
                nc.sync.dma_start(xtiles[:, :, lo:hi], xT[:, :, lo:hi])
            w1s = []
            for m in range(HC):
                w1m = w1p.tile([128, JC, 128], f8, tag="w1", name=f"w1_{m}")
                nc.sync.dma_start(
                    w1m[:].rearrange("p j c -> p (j c)"), fc1W[:, m, :])
                w1s.append(w1m)
            w2t = w2p.tile([128, HC, DIM], bf16)
            nc.sync.dma_start(w2t[:], fc2W)

            def load_chunk(ci, lo, hi):
                return xtiles[:, :, lo:hi]

            _standardize(nc, tc, _NSL_B, JC, load_chunk, xn, ones1, onesP1,
                         lnrows, bf16_in=True)
            # dummy gelu: load the gelu activation table during LN-tail idle
            # instead of serially before the first fc1 eviction
            dumg = const.tile([1, 1], f32)
            nc.scalar.activation(dumg[:], ones1[0:1, :], AF.Gelu)

            with tc.tile_pool(name="f1_ps", bufs=4, space="PSUM") as f1_ps, \
                 tc.tile_pool(name="f2_ps", bufs=3, space="PSUM") as f2_ps, \
                 tc.tile_pool(name="out_sb", bufs=3) as out_sb:
                for ci, (lo, hi) in enumerate(_FC_B):
                    w = hi - lo
                    for m in range(HC):
                        pt = f1_ps.tile([128, w], f32, tag="f1")
                        for jp in range(JC // 2):
                            nc.tensor.matmul(
                                pt[:], w1s[m][:, 2 * jp:2 * jp + 2, :],
                                xn[:, 2 * jp:2 * jp + 2, lo:hi],
                                start=(jp == 0), stop=(jp == JC // 2 - 1),
                                perf_mode=mybir.MatmulPerfMode.DoubleRow)
                        # per-out-channel fp8 descale via the activation
                        # scale AP; bias applies after the scale
                        nc.scalar.activation(h[:, m, lo:hi], pt[:], AF.Gelu,
                                             bias=b1t[:, m:m + 1],
                                             scale=s1t[:, m:m + 1])
                for ci, (lo, hi) in enumerate(_FC_B):
                    for m in range(JC):
                        last = (ci == len(_FC_B) - 1 and m == JC - 1)
                        parts = ([(lo, (lo + hi) // 2), ((lo + hi) // 2, hi)]
                                 if last else [(lo, hi)])
                        for plo, phi in parts:
                            w = phi - plo
                            pt = f2_ps.tile([128, 512], f32, tag="f2")
                            for j in range(HC):
                                nc.tensor.matmul(
                                    pt[:, :w], w2t[:, j, m * 128:(m + 1) * 128],
                                    h[:, j, plo:phi],
                                    start=(j == 0), stop=(j == HC - 1))
                            ot = out_sb.tile([128, 512], f32, tag="out")
                            # ot = (psum + fc2_b) + x   in one DVE pass
                            nc.vector.scalar_tensor_tensor(
                                out=ot[:, :w], in0=pt[:, :w],
                                scalar=b2t[:, m:m + 1],
                                in1=xtiles[:, m, plo:phi],
                                op0=ALU.add, op1=ALU.add)
                            nc.sync.dma_start(yT[:, m, plo:phi], ot[:, :w])
    nc.compile()
    return nc


# ---------------- host glue ----------------

_CACHE = {}


def _get(name, builder):
    if name not in _CACHE:
        _CACHE[name] = builder()
    return _CACHE[name]


def _featmajor(a):
    """(T, 768) fp32 -> [128, 6, T]"""
    Tn = a.shape[0]
    return np.ascontiguousarray(a.T.reshape(JC, 128, Tn).transpose(1, 0, 2))


def _unfeat(aT):
    """[128, 6, T] -> (T, 768)"""
    return np.asarray(aT).transpose(1, 0, 2).reshape(DIM, -1).T


def _chunkvec(v):
    """(c*128,) -> [128, c] fp32"""
    v = np.asarray(v, np.float32)
    return np.ascontiguousarray(v.reshape(-1, 128).T)


def _wchunk(w, nchunk, dtype=BF16):
    """(768, nchunk*128) weight -> [128, nchunk, 768] (m-major chunks:
    out[p, m, j*128 + c] = w[j*128 + p, m*128 + c])."""
    w = np.asarray(w, np.float32)
    kin = w.shape[0] // 128
    out = w.reshape(kin, 128, nchunk, 128).transpose(1, 2, 0, 3)
    return np.ascontiguousarray(out.reshape(128, nchunk, kin * 128)).astype(dtype)


def _bf16(a):
    return np.asarray(a, dtype=BF16)


def _build_rel(rel_pos, ws=WS):
    """[64, 14, 14] fp8: out[c, h, k] = rel_pos[idx[h,k], c] * W8 / sqrt(SCALE)
    (qhat holds sqrt(SCALE)*q; W8 prescale is undone at the rel eviction)."""
    idx = np.arange(ws)[:, None] - np.arange(ws)[None, :] + (ws - 1)
    R = np.asarray(rel_pos, np.float32)[idx] * (W8 / SCALE ** 0.5)
    return R.transpose(2, 0, 1).astype(F8)


def _build_onehots():
    """Eh[r, :, k] = 1 if k//14 == r;  Ew[r, :, k] = 1 if k%14 == r,
    pre-expanded over the (j, win) axis for big contiguous DMA runs."""
    k = np.arange(N)
    Eh = (k[None, :] // WS == np.arange(WS)[:, None]).astype(np.float32)
    Ew = (k[None, :] % WS == np.arange(WS)[:, None]).astype(np.float32)
    Eh = np.ascontiguousarray(np.broadcast_to(Eh[:, None, :], (WS, JC * NW, N)))
    Ew = np.ascontiguousarray(np.broadcast_to(Ew[:, None, :], (WS, JC * NW, N)))
    return Eh.astype(F8), Ew.astype(F8)


kernel_last_perf = {}

try:
    from antenv.axon_hooks import get_axon_ntff_profile_hook as _hook  # noqa: F401
    _HAVE_TRACE = True
except ImportError:
    _HAVE_TRACE = False
    import os as _os
    _os.environ["BASS_NEVER_TRACE"] = "1"   # bass_utils re-reads BASS_TRACE


def window_x(x):
    """(2, 64, 64, 768) -> (56, 196, 768) padded window tokens."""
    B, H, W, C = x.shape
    xp = np.zeros((B, 70, 70, C), np.float32)
    xp[:, :64, :64] = x
    xw = xp.reshape(B, 5, WS, 5, WS, C).transpose(0, 1, 3, 2, 4, 5).reshape(50, N, C)
    xall = np.zeros((56, N, C), np.float32)
    xall[:50] = xw
    return xall


def attn_consts(norm1_w, norm1_b, qkv_w, qkv_b, proj_w, proj_b,
                rel_pos_h, rel_pos_w):
    """Host-side constant tensors for dispatch A (LN1 affine absorbed)."""
    n1w = np.asarray(norm1_w, np.float32)
    n1b = np.asarray(norm1_b, np.float32)
    qkvw = np.asarray(qkv_w, np.float32)
    qkvb = np.asarray(qkv_b, np.float32)
    Wq = n1w[:, None] * qkvw                 # (768, 2304)
    bfull = n1b @ qkvw + qkvb                # (2304,)
    Eh, Ew = _build_onehots()
    return {
        "qkW": _wchunk(Wq[:, 0:2 * DIM] * W8, 12, F8),
        "wvT": np.ascontiguousarray(
            Wq[:, 2 * DIM:].reshape(JC, 128, DIM).transpose(1, 0, 2)
            * W8).astype(F8),
        "wpT": np.ascontiguousarray(
            np.asarray(proj_w, np.float32).reshape(JC, 128, JC, 128)
            .transpose(1, 0, 2, 3) * W8).astype(F8),
        "bqT": _chunkvec(bfull[0:DIM] * SCALE ** 0.5),
        "bkT": _chunkvec(bfull[DIM:2 * DIM] * SCALE ** 0.5),
        "bvT": _bf16(bfull[2 * DIM:].reshape(1, 2, 384) * W8),
        "pbT": _chunkvec(proj_b),
        "RhT": _build_rel(rel_pos_h),
        "RwT": _build_rel(rel_pos_w),
        "EhT": Eh,
        "EwT": Ew,
    }


def mlp_consts(norm2_w, norm2_b, fc1_w, fc1_b, fc2_w, fc2_b):
    """Host-side constant tensors for dispatch B (LN2 affine absorbed)."""
    n2w = np.asarray(norm2_w, np.float32)
    n2b = np.asarray(norm2_b, np.float32)
    f1w = np.asarray(fc1_w, np.float32)
    W1 = n2w[:, None] * f1w                  # (768, 3072)
    b1 = n2b @ f1w + np.asarray(fc1_b, np.float32)
    # per-out-channel power-of-2 fp8 scaling for fc1 (exactly undone by the
    # gelu activation's per-partition scale AP)
    colmax = np.abs(W1).max(axis=0)                        # (3072,)
    sexp = np.clip(np.floor(np.log2(224.0 / np.maximum(colmax, 1e-30))),
                   -20, 20)
    wscale = np.exp2(sexp)                                 # (3072,)
    return {
        "fc1W": _wchunk(W1 * wscale[None, :], HC, F8),
        "fc1S": _chunkvec(1.0 / wscale),
        "fc2W": _bf16(np.ascontiguousarray(
            np.asarray(fc2_w, np.float32).reshape(HC, 128, DIM)
            .transpose(1, 0, 2))),
        "fc1B": _chunkvec(b1),
        "fc2B": _chunkvec(fc2_b),
    }


def kernel(x, norm1_w, norm1_b, qkv_w, qkv_b, proj_w, proj_b,
           rel_pos_h, rel_pos_w, norm2_w, norm2_b,
           fc1_w, fc1_b, fc2_w, fc2_b):
    import os
    trace = bool(os.environ.get("BASS_TRACE")) and _HAVE_TRACE
    x = np.asarray(x, np.float32)
    B, H, W, C = x.shape
    assert (B, H, W, C) == (2, 64, 64, DIM)

    # ---- dispatch A: windowed attention ----
    xall = window_x(x)
    consts_a = attn_consts(norm1_w, norm1_b, qkv_w, qkv_b, proj_w, proj_b,
                           rel_pos_h, rel_pos_w)
    with_vbias = bool(np.any(np.asarray(consts_a["bvT"], np.float32)))
    nc_a = _get(f"attn{int(with_vbias)}",
                lambda: build_attn(with_vbias=with_vbias))
    in_maps = []
    for c in range(NCORES):
        m = dict(consts_a)
        m["xT"] = _featmajor(
            xall[c * NW:(c + 1) * NW].reshape(T, C)).astype(BF16)
        in_maps.append(m)
    res_a = run_bass_kernel_spmd(nc_a, in_maps, core_ids=list(range(NCORES)),
                                 trace=trace)
    kernel_last_perf["attn"] = res_a.exec_time_ns
    xo_all = np.stack([_unfeat(res_a.results[c]["xoT"]) for c in range(NCORES)])
    xo = xo_all.reshape(56, N, C)[:50]
    xo = xo.reshape(B, 5, 5, WS, WS, C).transpose(0, 1, 3, 2, 4, 5).reshape(B, 70, 70, C)
    x2 = x + xo[:, :64, :64]

    # ---- dispatch B: MLP ----
    nc_b = _get("mlp", build_mlp)
    consts_b = mlp_consts(norm2_w, norm2_b, fc1_w, fc1_b, fc2_w, fc2_b)
    x2f = np.ascontiguousarray(x2.reshape(B * H * W, C))
    in_maps = []
    for c in range(NCORES):
        m = dict(consts_b)
        m["xT"] = _featmajor(x2f[c * TB:(c + 1) * TB]).astype(BF16)
        in_maps.append(m)
    res_b = run_bass_kernel_spmd(nc_b, in_maps, core_ids=list(range(NCORES)),
                                 trace=trace)
    kernel_last_perf["mlp"] = res_b.exec_time_ns
    y = np.concatenate([_unfeat(res_b.results[c]["yT"]) for c in range(NCORES)])
    return y.reshape(B, H, W, C).astype(np.float32)


# revision 76
# speedup vs baseline: 1.1980x; 1.0014x over previous
"""Trainium2 Bass kernel for a SAM/ViTDet-style windowed-attention transformer
block (DIM=768, 12 heads, window 14, decomposed rel-pos bias, exact-gelu MLP).

Contract: kernel(**inputs) takes the FULL unsharded inputs from
reference.setup_inputs() and returns the FULL (2, 64, 64, 768) float32 output.

Strategy (8 NeuronCores, SPMD, data-parallel):
  Dispatch A (attention): shard the 50 real windows (padded to 56) as 7
    windows/core. Per core: LN1 -> qkv -> windowed attention with the
    decomposed rel-pos bias folded into an augmented-key matmul -> proj.
  Host: window-unpartition, crop, residual add.
  Dispatch B (MLP): shard the 8192 tokens as 1024/core. Per core:
    LN2 -> fc1 -> exact GELU -> fc2 -> residual.

Perf notes (v2):
  * The LN affine (w, b) is absorbed host-side into the following matmul
    weights/biases, so on-device LN is a pure standardize: bf16 stats
    matmuls + bf16 broadcast tiles + two 4x-rate DVE tensor_tensor ops.
  * Rel-pos rows are produced by per-(h or w) batched matmuls (112 instead
    of 392) whose PSUM outputs land at partition bases 0/32/64/96, cutting
    eviction traffic.
  * The two score matmuls of one (window, head) share a [128, 392] PSUM
    tile -> a single exp instruction per head.
  * AV outputs are 6-head-batched in PSUM; the softmax normalization is one
    DVE multiply with a stride-0 (broadcast) reciprocal operand.
  * Weight DMAs are chunked and issued up front so compute starts ~10us in.

Augmented-key rel-pos layout (q/k head-pair blocks, per j-chunk):
  even head: q 0:64,  rel_h 64:78,  zeros 78:96, rel_w 96:110, zeros 110:128
  odd head:  rel_h 0:14, zeros 14:32, rel_w 32:46, zeros 46:64, q 64:128
khat holds k values in the q rows and one-hot key-position masks in the rel
rows; zero gaps make the extra contraction rows inert. S^T = khat^T qhat then
includes the decomposed bias exactly.
"""

import sys

sys.path.insert(0, "/opt/trn_rl_repo")

from contextlib import ExitStack

import numpy as np
import ml_dtypes

import concourse.bacc as bacc
import concourse.mybir as mybir
import concourse.tile as tile
from concourse.bass_utils import run_bass_kernel_spmd
from concourse.masks import make_identity

dt = mybir.dt
AF = mybir.ActivationFunctionType
ALU = mybir.AluOpType

DIM = 768
HEADS = 12
HD = 64
WS = 14
N = WS * WS          # 196 tokens / window
NW = 7               # windows per core
T = NW * N           # 1372 token slots per core (dispatch A)
TB = 1024            # tokens per core (dispatch B)
MLP = 3072
NCORES = 8
JC = DIM // 128      # 6 feature chunks
HC = MLP // 128      # 24 hidden chunks
EPS = 1e-5
SCALE = HD ** -0.5   # 0.125
TPAD = 1376          # T rounded up so fp8 DoubleRow pair strides are 16B-aligned
NPAD = 208           # N rounded up likewise (aT)
BF16 = ml_dtypes.bfloat16
F8 = ml_dtypes.float8_e4m3
W8 = 64.0   # fp8 weight pre-scale (avoids e4m3 subnormals); undone at eviction

# window-aligned token chunks for dispatch A (2+2+2+1 windows)
_NSL = [(0, 392), (392, 784), (784, 1176), (1176, 1372)]
_NSL_LN = [(i * N, (i + 1) * N) for i in range(NW)]        # LN chunks, A
_NSL_B = [(0, 256), (256, 512), (512, 768), (768, 1024)]   # LN chunks, B
_FC_B = [(0, 512), (512, 1024)]                            # matmul chunks, B
_SUBS = [(0, 128), (128, 196)]                             # within-window subchunks

# augmented-key row layout per parity: (q_lo, relh_lo, relw_lo)
_EVEN = (0, 64, 96)    # q 0:64,  rel rows above
_ODD = (64, 0, 32)     # q 64:128, rel rows below


def _standardize(nc, tc, nsl_list, jc, load_chunk, xn, ones1, onesP1, rows_p,
                 cast_engine="gpsimd", bf16_in=False):
    """Pure LN standardize: xn[:, j, c] = (x - mu[c]) * rsig[c], bf16 out.

    load_chunk(ci, lo, hi) -> AP [128, jc, w] for that chunk (may DMA into a
    fresh tile or return a view of a resident one); fp32 unless bf16_in (then
    it is used directly, no cast).  Stats run as bf16 matmuls vs a ones
    vector; mu/rsig are kept as bf16 rows, broadcast across partitions via
    tiny bf16 matmuls, evicted to bf16 SBUF and applied with two DVE
    tensor_tensor ops (all-bf16, stride-1 -> DVE fast mode).
    ones1: [128,1] bf16 ones; onesP1: [1,128] bf16 ones; rows_p: pool for rows.
    """
    nch = len(nsl_list)
    dimn = jc * 128

    eng_cast = getattr(nc, cast_engine)

    epsr = rows_p.tile([1, 1], dt.float32, tag="epsr")
    nc.vector.memset(epsr[:], EPS)

    with tc.tile_pool(name="ln_xb", bufs=nch) as xbp, \
         tc.tile_pool(name="ln_sq", bufs=2) as sqp, \
         tc.tile_pool(name="ln_st", bufs=2, space="PSUM") as st_ps, \
         tc.tile_pool(name="ln_bc", bufs=2, space="PSUM") as bc_ps, \
         tc.tile_pool(name="ln_bcs", bufs=4) as bcs, \
         tc.tile_pool(name="ln_rows", bufs=nch + 1) as rp:
        # per-chunk stats emitted with the apply of the PREVIOUS chunk
        # interleaved (one-chunk lag): PE runs stats back to back while the
        # row math / broadcast / apply of the prior chunk drains on Act/DVE
        xbs, mus, rss = [], [], []

        def emit_stats(ci, lo, hi):
            w = hi - lo
            xt = load_chunk(ci, lo, hi)
            if bf16_in:
                xb = xt
            else:
                xb = xbp.tile([128, jc, w], dt.bfloat16, tag="xb",
                              name=f"xb{ci}")
                eng_cast.tensor_copy(xb[:], xt)
                xb = xb[:]
            xbs.append(xb)
            # per-token sums -> mu
            pmu = st_ps.tile([1, w], dt.float32, tag="st")
            for j in range(jc):
                nc.tensor.matmul(pmu[:], ones1[:], xb[:, j, :],
                                 start=(j == 0), stop=(j == jc - 1))
            mu = rp.tile([1, w], dt.bfloat16, tag="mu", name=f"mu{ci}")
            nc.scalar.activation(mu[:], pmu[:], AF.Copy, scale=1.0 / dimn)
            mus.append(mu)
            # per-token sum of squares -> E[x^2]
            sq = sqp.tile([128, jc, w], dt.bfloat16, tag="sq")
            nc.vector.tensor_tensor(out=sq[:], in0=xb, in1=xb, op=ALU.mult)
            pmq = st_ps.tile([1, w], dt.float32, tag="st")
            for j in range(jc):
                nc.tensor.matmul(pmq[:], ones1[:], sq[:, j, :],
                                 start=(j == 0), stop=(j == jc - 1))
            mq = rp.tile([1, w], dt.float32, tag="mq")
            nc.scalar.activation(mq[:], pmq[:], AF.Copy, scale=1.0 / dimn)
            # rsig = 1/sqrt(E[x^2] - mu^2 + eps)
            m2 = rp.tile([1, w], dt.float32, tag="m2")
            nc.vector.tensor_tensor(out=m2[:], in0=mu[:], in1=mu[:],
                                    op=ALU.mult)
            nc.vector.tensor_tensor(out=mq[:], in0=mq[:], in1=m2[:],
                                    op=ALU.subtract)
            sd = rp.tile([1, w], dt.float32, tag="sd")
            nc.scalar.activation(sd[:], mq[:], AF.Sqrt, bias=epsr[:])
            rsig = rp.tile([1, w], dt.bfloat16, tag="rs", name=f"rs{ci}")
            with nc.allow_low_precision(reason="bf16 rsig row; 0.4% rel err "
                                        "matches the bf16 matmul noise "
                                        "floor"):
                nc.vector.reciprocal(rsig[:], sd[:])
            rss.append(rsig)

        def emit_apply(ci, lo, hi):
            w = hi - lo
            xb, mu, rsig = xbs[ci], mus[ci], rss[ci]
            bmu_p = bc_ps.tile([128, w], dt.float32, tag="bc")
            nc.tensor.matmul(bmu_p[:], onesP1[:], mu[:], start=True, stop=True)
            brs_p = bc_ps.tile([128, w], dt.float32, tag="bc")
            nc.tensor.matmul(brs_p[:], onesP1[:], rsig[:], start=True,
                             stop=True)
            bmu = bcs.tile([128, w], dt.bfloat16, tag="bmu")
            nc.scalar.copy(bmu[:], bmu_p[:])
            brs = bcs.tile([128, w], dt.bfloat16, tag="brs")
            nc.scalar.copy(brs[:], brs_p[:])
            # xn = (x - mu) * rsig   (two all-bf16 DVE ops, j-broadcast)
            cen = sqp.tile([128, jc, w], dt.bfloat16, tag="cen")
            nc.vector.tensor_tensor(
                out=cen[:], in0=xb,
                in1=bmu[:].unsqueeze(1).to_broadcast([128, jc, w]),
                op=ALU.subtract)
            with nc.allow_low_precision(reason="xn storage dtype (bf16/fp8) "
                                        "is the matmul operand precision"):
                nc.vector.tensor_tensor(
                    out=xn[:, :, lo:hi], in0=cen[:],
                    in1=brs[:].unsqueeze(1).to_broadcast([128, jc, w]),
                    op=ALU.mult)

        # chunk 0's apply is emitted right after its stats so the first xn
        # chunk (the qk-phase gate) is produced as early as possible
        emit_stats(0, *nsl_list[0])
        emit_apply(0, *nsl_list[0])
        for ci in range(1, nch):
            emit_stats(ci, *nsl_list[ci])
        for ci in range(1, nch):
            emit_apply(ci, *nsl_list[ci])


def build_attn(with_vbias=True):
    """Dispatch A: LN1 + qkv + windowed attention (+rel-pos) + proj."""
    nc = bacc.Bacc("TRN2", target_bir_lowering=False, debug=False)
    f32, bf16 = dt.float32, dt.bfloat16

    xT = nc.dram_tensor("xT", [128, JC, T], bf16, kind="ExternalInput").ap()
    f8 = dt.float8e4
    qkW = nc.dram_tensor("qkW", [128, 12, JC * 128], f8, kind="ExternalInput").ap()
    wvT = nc.dram_tensor("wvT", [128, JC, DIM], f8, kind="ExternalInput").ap()
    wpT = nc.dram_tensor("wpT", [128, JC, JC, 128], f8, kind="ExternalInput").ap()
    bqT = nc.dram_tensor("bqT", [128, JC], f32, kind="ExternalInput").ap()
    bkT = nc.dram_tensor("bkT", [128, JC], f32, kind="ExternalInput").ap()
    bvT = nc.dram_tensor("bvT", [1, 2, 384], bf16, kind="ExternalInput").ap()
    pbT = nc.dram_tensor("pbT", [128, JC], f32, kind="ExternalInput").ap()
    RhT = nc.dram_tensor("RhT", [64, WS, WS], f8, kind="ExternalInput").ap()
    RwT = nc.dram_tensor("RwT", [64, WS, WS], f8, kind="ExternalInput").ap()
    EhT = nc.dram_tensor("EhT", [WS, JC * NW, N], f8, kind="ExternalInput").ap()
    EwT = nc.dram_tensor("EwT", [WS, JC * NW, N], f8, kind="ExternalInput").ap()
    xoT = nc.dram_tensor("xoT", [128, JC, T], f32, kind="ExternalOutput").ap()

    with tile.TileContext(nc) as tc, ExitStack() as ctx:
        const = ctx.enter_context(tc.tile_pool(name="const", bufs=1))
        big = ctx.enter_context(tc.tile_pool(name="big", bufs=1))
        lnrows = ctx.enter_context(tc.tile_pool(name="lnrows", bufs=1))

        # ---- big persistent tensors (declared first so memsets start at t=0)
        # qhat/khat live in a partition-paired fp8 layout [64, two, j, t]:
        # logical contraction row r maps to (r % 64, r // 64), so the score
        # matmuls run fp8 DoubleRow.  Plane assignment per parity:
        #   even head: q = plane 0, rel_h rows 0:14 / rel_w 32:46 on plane 1
        #   odd head:  q = plane 1, rel_h rows 0:14 / rel_w 32:46 on plane 0
        xn = big.tile([128, JC, TPAD], dt.float8e4)
        qhE = big.tile([64, 2, JC, TPAD], dt.float8e4)
        khE = big.tile([64, 2, JC, TPAD], dt.float8e4)
        qhB = big.tile([64, 2, JC, TPAD], dt.float8e4)
        khB = big.tile([64, 2, JC, TPAD], dt.float8e4)
        vtok = big.tile([128, NW, 2, HEADS, HD + 1], bf16)

        # ---- constants ----
        ones1 = const.tile([128, 1], bf16)
        nc.vector.memset(ones1[:], 1.0)
        onesP1 = const.tile([1, 128], bf16)
        nc.vector.memset(onesP1[:], 1.0)
        onesT = const.tile([1, 128], bf16)
        nc.vector.memset(onesT[:], 1.0)
        ident = const.tile([128, 128], bf16)
        make_identity(nc, ident[:])

        rh = const.tile([64, WS, WS], f8)
        nc.scalar.dma_start(rh[:], RhT)
        rw = const.tile([64, WS, WS], f8)
        nc.scalar.dma_start(rw[:], RwT)
        bq = const.tile([128, JC], f32)
        nc.scalar.dma_start(bq[:], bqT)
        bk = const.tile([128, JC], f32)
        nc.scalar.dma_start(bk[:], bkT)
        bv = const.tile([1, 2, 384], bf16)
        nc.scalar.dma_start(bv[:], bvT)
        pb = const.tile([128, JC], f32)
        nc.scalar.dma_start(pb[:], pbT)
        # wv/wp tiles are created here but their loads are issued on the sync
        # queue after the x/qk-weight DMAs so the global DMA device serves x
        # first (sync-queue program order == DMA device order).
        wv = const.tile([128, JC, DIM], f8)
        wp = const.tile([128, JC, JC, 128], f8)

        # ---- LN1 (pure standardize; affine absorbed into weights) ----
        # x arrives already bf16 (host cast) -> no on-device cast, half DMA;
        # window-sized chunks get the first xn out early for the qk start
        with tc.tile_pool(name="ln_x", bufs=4) as xp:
            def load_chunk(ci, lo, hi):
                xt = xp.tile([128, JC, hi - lo], bf16, tag="x", name=f"x{ci}")
                nc.sync.dma_start(xt[:], xT[:, :, lo:hi])
                return xt[:]
            _standardize(nc, tc, _NSL, JC, load_chunk, xn, ones1, onesP1,
                         lnrows, bf16_in=True)

        # dummy exp right after LN: the Act engine loads the exp activation
        # table here (post-LN idle) instead of serially at the core start
        dume = const.tile([1, 1], f32)
        nc.scalar.activation(dume[:], ones1[0:1, :], AF.Exp)

        # zero the rel/one-hot halves (gaps must be exactly 0; rel rows and
        # one-hot rows overlay these ranges later).  Issued after the LN body
        # so the Pool queue serves the LN casts first; the Tile deps still
        # order these before the mask DMAs / rel evictions below.
        nc.gpsimd.memset(khE[:, 1, :, :], 0.0)
        nc.gpsimd.memset(khB[:, 0, :, :], 0.0)
        nc.gpsimd.memset(qhE[:, 1, :, :], 0.0)
        nc.gpsimd.memset(qhB[:, 0, :, :], 0.0)
        # ones column in vtok (AV matmul also yields the softmax denominator)
        nc.gpsimd.memset(
            vtok[:].rearrange("p w s h o -> p (w s h) o")[:, :, HD:HD + 1], 1.0)

        # ---- q/k (feature-major, split by parity) + v (token-major) ----
        # PE program order is tuned so the in-order PE queue never waits on
        # slow producers: qk c0,c1 | v w0,w1 | qk c2,c3 | rel matmuls |
        # v w2..w6 (covers the rel-eviction drain) | pipelined core.
        _WHALF = [(0, 4), (4, 7)]
        with tc.tile_pool(name="qk_w", bufs=12) as wqk_sb, \
             tc.tile_pool(name="qk_stg", bufs=2) as stg_sb, \
             tc.tile_pool(name="qk_ps", bufs=2, space="PSUM") as qk_ps, \
             tc.tile_pool(name="v_ps", bufs=2, space="PSUM") as v_ps, \
             tc.tile_pool(name="rel_ps", bufs=4, space="PSUM") as rel_ps:
            wms = []
            for m in range(12):
                wm = wqk_sb.tile([128, JC, 128], f8, tag="wqk", name=f"w{m}")
                nc.sync.dma_start(
                    wm[:].rearrange("p j c -> p (j c)"), qkW[:, m, :])
                wms.append(wm)
            nc.sync.dma_start(wv[:], wvT)
            nc.sync.dma_start(wp[:], wpT)
            # one-hot key-position masks into khat rel rows; issued last on
            # the sync queue so x/weight transfers win the DMA device first
            for (msrc, mdst, tw, r0) in ((EhT, khE, 1, 0), (EwT, khE, 1, 32),
                                         (EhT, khB, 0, 0), (EwT, khB, 0, 32)):
                nc.sync.dma_start(
                    mdst[r0:r0 + WS, tw, :, 0:T].rearrange(
                        "p j (w n) -> p j w n", n=N),
                    msrc.rearrange("p (j w) n -> p j w n", w=NW))

            def emit_qk_chunk(ci):
                lo, hi = _NSL[ci]
                w = hi - lo
                # full-height evicts into per-chunk staging tiles (the bias
                # AP is per-partition, so one op covers both parity halves);
                # per chunk just 4 SBUF->SBUF DMAs distribute the halves
                qstg = stg_sb.tile([128, JC, 392], dt.float8e4, tag="qstg")
                kstg = stg_sb.tile([128, JC, 392], dt.float8e4, tag="kstg")
                for m in range(12):
                    is_q = m < JC
                    e = m % JC
                    pt = qk_ps.tile([128, 392], f32, tag="qk")
                    for jp in range(JC // 2):
                        nc.tensor.matmul(pt[:, :w],
                                         wms[m][:, 2 * jp:2 * jp + 2, :],
                                         xn[:, 2 * jp:2 * jp + 2, lo:hi],
                                         start=(jp == 0),
                                         stop=(jp == JC // 2 - 1),
                                         perf_mode=mybir.MatmulPerfMode.DoubleRow)
                    if is_q:
                        nc.scalar.activation(qstg[:, e, :w], pt[:, :w],
                                             AF.Identity, bias=bq[:, m:m + 1],
                                             scale=SCALE ** 0.5 / W8)
                    else:
                        nc.vector.tensor_scalar(
                            out=kstg[:, e, :w], in0=pt[:, :w],
                            scalar1=SCALE ** 0.5 / W8, scalar2=bk[:, e:e + 1],
                            op0=ALU.mult, op1=ALU.add)
                nc.sync.dma_start(qhE[:, 0, :, lo:hi], qstg[0:64, :, :w])
                nc.sync.dma_start(qhB[:, 1, :, lo:hi], qstg[64:128, :, :w])
                nc.sync.dma_start(khE[:, 0, :, lo:hi], kstg[0:64, :, :w])
                nc.sync.dma_start(khB[:, 1, :, lo:hi], kstg[64:128, :, :w])

            def emit_v(win):
                for si, (slo, shi) in enumerate(_SUBS):
                    ssz = shi - slo
                    base = win * N + slo
                    for half in range(2):
                        pv = v_ps.tile([128, 384], f32, tag="v")
                        for jp in range(JC // 2):
                            nc.tensor.matmul(
                                pv[:ssz, :],
                                xn[:, 2 * jp:2 * jp + 2, base:base + ssz],
                                wv[:, 2 * jp:2 * jp + 2,
                                   half * 384:(half + 1) * 384],
                                start=(jp == 0),
                                stop=(not with_vbias
                                      and jp == JC // 2 - 1),
                                perf_mode=mybir.MatmulPerfMode.DoubleRow)
                        if with_vbias:
                            nc.tensor.matmul(
                                pv[:ssz, :], onesT[:, :ssz], bv[:, half, :],
                                start=False, stop=True)
                        if True:
                            nc.scalar.activation(
                                vtok[0:ssz, win, si,
                                     6 * half:6 * half + 6, 0:HD],
                                pv[:ssz, :].rearrange("p (h d) -> p h d",
                                                      d=HD),
                                AF.Copy, scale=1.0 / W8)
                        else:
                            nc.vector.tensor_scalar(
                                out=vtok[0:ssz, win, si,
                                         6 * half:6 * half + 6, 0:HD],
                                in0=pv[:ssz, :].rearrange(
                                    "p (h d) -> p h d", d=HD),
                                scalar1=1.0 / W8, scalar2=None, op0=ALU.mult)

            def emit_rel():
                # rel-pos rows into qhat planes, batched per h (rel_h) / per
                # w (rel_w); window halves keep PSUM cols <= 336.  Both
                # parities read q from partitions 0:64 of their q plane and
                # write rel rows 0:14 (rel_h) / 32:46 (rel_w) of the other.
                ri = 0
                for (w0, w1) in _WHALF:   # window halves outermost: the core
                    # can start on windows 0..3 while half 4..7 still drains
                    for par in range(2):
                        qh = qhE if par == 0 else qhB
                        qtw = 0 if par == 0 else 1       # q plane
                        rtw = 1 - qtw                    # rel plane
                        for typ in range(2):
                            r0 = 0 if typ == 0 else 32
                            stat = rh if typ == 0 else rw
                            for hh in range(WS):
                                nwn = w1 - w0
                                if typ == 0:
                                    mov = qh[:, qtw, :, 0:T].rearrange(
                                        "p j (win n) -> p j win n", n=N)[
                                        :, :, w0:w1, hh * WS:(hh + 1) * WS]
                                else:
                                    mov = qh[:, qtw, :, 0:T].rearrange(
                                        "p j (win kh kw) -> p j win kh kw",
                                        kh=WS, kw=WS)[:, :, w0:w1, :, hh]
                                ncols = JC * nwn * WS
                                prel = rel_ps.tile([128, 336], f32, tag="rel")
                                # skip_group_check: sim-only guard; its
                                # flat-address region view aliases across
                                # banks for <128-partition outputs
                                nc.tensor.matmul(
                                    prel[r0:r0 + WS, :ncols],
                                    stat[0:64, hh, :], mov,
                                    start=True, stop=True,
                                    tile_position=(0, r0),
                                    skip_group_check=True)
                                if typ == 0:
                                    dst = qh[r0:r0 + WS, rtw, :, 0:T].rearrange(
                                        "p j (win n) -> p j win n", n=N)[
                                        :, :, w0:w1, hh * WS:(hh + 1) * WS]
                                else:
                                    dst = qh[r0:r0 + WS, rtw, :, 0:T].rearrange(
                                        "p j (win kh kw) -> p j win kh kw",
                                        kh=WS, kw=WS)[:, :, w0:w1, :, hh]
                                src = prel[r0:r0 + WS, :ncols].rearrange(
                                    "p (j win k) -> p j win k", j=JC, win=nwn)
                                with nc.allow_low_precision(
                                        reason="fp8 rel rows; absolute score "
                                        "error ~3e-3 vs budget 0.1"):
                                    if ri % 2 == 0:
                                        nc.scalar.activation(
                                            dst, src, AF.Copy, scale=1.0 / W8)
                                    else:
                                        nc.vector.tensor_scalar(
                                            out=dst, in0=src,
                                            scalar1=1.0 / W8, scalar2=None,
                                            op0=ALU.mult)
                                ri += 1

            emit_qk_chunk(0)
            emit_qk_chunk(1)
            emit_v(0)
            emit_v(1)
            emit_qk_chunk(2)
            emit_qk_chunk(3)
            emit_rel()
            for win in range(2, NW):
                emit_v(win)

        # ---- attention core + proj: software-pipelined across windows ----
        # per iteration: AV+normalize(w) | scores+exp(w+1) | transp+proj(w);
        # window w+1's score matmuls keep PE busy while w's softmax
        # normalization drains on DVE.
        with tc.tile_pool(name="s_ps", bufs=1, space="PSUM") as s_ps, \
             tc.tile_pool(name="av_ps", bufs=2, space="PSUM") as av_ps, \
             tc.tile_pool(name="t_ps", bufs=2, space="PSUM") as t_ps, \
             tc.tile_pool(name="pj_ps", bufs=2, space="PSUM") as pj_ps, \
             tc.tile_pool(name="pt_sb", bufs=26) as pt_sb, \
             tc.tile_pool(name="ao_sb", bufs=2) as ao_sb, \
             tc.tile_pool(name="at_sb", bufs=2) as at_sb, \
             tc.tile_pool(name="xo_sb", bufs=2) as xo_sb, \
             tc.tile_pool(name="r_sb", bufs=4) as r_sb:
            # two persistent score tiles, rotated manually: the exp reads the
            # full [128, 392] tile, so the region no matmul covers (rows
            # 68:128 of the second key chunk) is zeroed exactly once
            sbufs = []
            for i in range(2):
                st = s_ps.tile([128, 392], f32, tag=f"s{i}", name=f"s{i}")
                # partition start must be 32-aligned on PSUM; rows 64:68 are
                # re-written by every second score matmul afterwards
                nc.vector.memset(st[64:128, 196:392], 0.0)
                sbufs.append(st)
            state = {"hidx": 0}

            def emit_scores(win):
                pts = []
                for head in range(HEADS):
                    blk = head // 2
                    par = head % 2
                    qh = qhE if par == 0 else qhB
                    kh = khE if par == 0 else khB
                    ps_t = sbufs[state["hidx"] % 2]
                    state["hidx"] += 1
                    for si, (slo, shi) in enumerate(_SUBS):
                        ssz = shi - slo
                        nc.tensor.matmul(
                            ps_t[:ssz, si * N:si * N + N],
                            kh[:, :, blk, win * N + slo:win * N + shi],
                            qh[:, :, blk, win * N:(win + 1) * N],
                            start=True, stop=True,
                            perf_mode=mybir.MatmulPerfMode.DoubleRow)
                    ptile = pt_sb.tile([128, 392], bf16, tag="pt")
                    nc.scalar.activation(ptile[:], ps_t[:], AF.Exp)
                    pts.append(ptile)
                return pts

            def emit_av(win, pts):
                ao0 = ao_sb.tile([128, DIM], bf16, tag="ao0")
                ao1 = ao_sb.tile([68, DIM], bf16, tag="ao1")
                for qi, (qlo, qhi) in enumerate(_SUBS):
                    qsz = qhi - qlo
                    ao = ao0 if qi == 0 else ao1
                    for hg in range(2):
                        pav = av_ps.tile([128, 6, HD + 1], f32, tag="av")
                        for hl in range(6):
                            head = hg * 6 + hl
                            for si, (slo, shi) in enumerate(_SUBS):
                                ssz = shi - slo
                                nc.tensor.matmul(
                                    pav[:qsz, hl, :],
                                    pts[head][0:ssz, si * N + qlo:si * N + qhi],
                                    vtok[0:ssz, win, si, head, :],
                                    start=(si == 0), stop=(si == 1))
                        rec = r_sb.tile([128, 6], f32, tag="rec")
                        nc.vector.reciprocal(
                            rec[:qsz, :],
                            pav[:qsz, :, HD:HD + 1].rearrange(
                                "p h o -> p (h o)"))
                        nc.vector.tensor_tensor(
                            out=ao[0:qsz, hg * 384:(hg + 1) * 384].rearrange(
                                "p (h d) -> p h d", d=HD),
                            in0=pav[:qsz, :, 0:HD],
                            in1=rec[:qsz, :].unsqueeze(2).to_broadcast(
                                [qsz, 6, HD]),
                            op=ALU.mult)
                return ao0, ao1

            def emit_transp_proj(win, ao0, ao1):
                aT = at_sb.tile([128, JC, NPAD], dt.float8e4, tag="at")
                ti = 0
                for j in range(JC):
                    for qi, (qlo, qhi) in enumerate(_SUBS):
                        qsz = qhi - qlo
                        src = ao0 if qi == 0 else ao1
                        ptt = t_ps.tile([128, 128], bf16, tag="tp")
                        nc.tensor.transpose(ptt[:, :qsz],
                                            src[0:qsz, j * 128:(j + 1) * 128],
                                            ident[0:qsz, 0:qsz])
                        if False:
                            nc.scalar.copy(aT[:, j, qlo:qhi], ptt[:, :qsz])
                        else:
                            nc.vector.tensor_copy(aT[:, j, qlo:qhi],
                                                  ptt[:, :qsz])
                        ti += 1
                xo_t = xo_sb.tile([128, JC, N], f32, tag="xo")
                for m in range(JC):
                    pp = pj_ps.tile([128, N], f32, tag="pj")
                    for jp in range(JC // 2):
                        nc.tensor.matmul(
                            pp[:], wp[:, 2 * jp:2 * jp + 2, m, :],
                            aT[:, 2 * jp:2 * jp + 2, 0:N],
                            start=(jp == 0), stop=(jp == JC // 2 - 1),
                            perf_mode=mybir.MatmulPerfMode.DoubleRow)
                    if m % 2 == 0:
                        nc.scalar.activation(xo_t[:, m, :], pp[:],
                                             AF.Identity,
                                             bias=pb[:, m:m + 1],
                                             scale=1.0 / W8)
                    else:
                        nc.vector.tensor_scalar(
                            out=xo_t[:, m, :], in0=pp[:],
                            scalar1=1.0 / W8, scalar2=pb[:, m:m + 1],
                            op0=ALU.mult, op1=ALU.add)
                nc.sync.dma_start(
                    xoT.rearrange("p j (w n) -> p j w n", n=N)[:, :, win, :],
                    xo_t[:])

            pts = emit_scores(0)
            for win in range(NW):
                ao0, ao1 = emit_av(win, pts)
                if win + 1 < NW:
                    pts = emit_scores(win + 1)
                emit_transp_proj(win, ao0, ao1)
    nc.compile()
    return nc


def build_mlp():
    """Dispatch B: y = x + fc2(gelu(fc1(LN2(x)))), 1024 tokens/core.
    LN2 affine is absorbed into fc1 host-side."""
    nc = bacc.Bacc("TRN2", target_bir_lowering=False, debug=False)
    f32, bf16 = dt.float32, dt.bfloat16

    xT = nc.dram_tensor("xT", [128, JC, TB], bf16, kind="ExternalInput").ap()
    f8 = dt.float8e4
    fc1W = nc.dram_tensor("fc1W", [128, HC, DIM], f8, kind="ExternalInput").ap()
    fc1S = nc.dram_tensor("fc1S", [128, HC], f32, kind="ExternalInput").ap()
    fc2W = nc.dram_tensor("fc2W", [128, HC, DIM], bf16, kind="ExternalInput").ap()
    fc1B = nc.dram_tensor("fc1B", [128, HC], f32, kind="ExternalInput").ap()
    fc2B = nc.dram_tensor("fc2B", [128, JC], f32, kind="ExternalInput").ap()
    yT = nc.dram_tensor("yT", [128, JC, TB], f32, kind="ExternalOutput").ap()

    with tile.TileContext(nc) as tc, ExitStack() as ctx:
        const = ctx.enter_context(tc.tile_pool(name="const", bufs=1))
        big = ctx.enter_context(tc.tile_pool(name="big", bufs=1))
        lnrows = ctx.enter_context(tc.tile_pool(name="lnrows", bufs=1))

        ones1 = const.tile([128, 1], bf16)
        nc.vector.memset(ones1[:], 1.0)
        onesP1 = const.tile([1, 128], bf16)
        nc.vector.memset(onesP1[:], 1.0)
        b1t = const.tile([128, HC], f32)
        nc.scalar.dma_start(b1t[:], fc1B)
        s1t = const.tile([128, HC], f32)
        nc.scalar.dma_start(s1t[:], fc1S)
        b2t = const.tile([128, JC], f32)
        nc.scalar.dma_start(b2t[:], fc2B)

        xtiles = big.tile([128, JC, TB], bf16)
        xn = big.tile([128, JC, TB], dt.float8e4)
        h = big.tile([128, HC, TB], bf16)

        with tc.tile_pool(name="w1_sb", bufs=HC) as w1p, \
             tc.tile_pool(name="w2_sb", bufs=1) as w2p:
            # x chunks first on the sync queue -> served first by the DMA
            # device; weights follow in need order (w1 chunks, then w2)
            for lo, hi in _NSL_B:
                nc.sync.dma_start(xtiles[:, :, lo:hi], xT[:, :, lo:hi])
            w1s = []
            for m in range(HC):
                w1m = w1p.tile([128, JC, 128], f8, tag="w1", name=f"w1_{m}")
                nc.sync.dma_start(
                    w1m[:].rearrange("p j c -> p (j c)"), fc1W[:, m, :])
                w1s.append(w1m)
            w2t = w2p.tile([128, HC, DIM], bf16)
            nc.sync.dma_start(w2t[:], fc2W)

            def load_chunk(ci, lo, hi):
                return xtiles[:, :, lo:hi]

            _standardize(nc, tc, _NSL_B, JC, load_chunk, xn, ones1, onesP1,
                         lnrows, bf16_in=True)
            # dummy gelu: load the gelu activation table during LN-tail idle
            # instead of serially before the first fc1 eviction
            dumg = const.tile([1, 1], f32)
            nc.scalar.activation(dumg[:], ones1[0:1, :], AF.Gelu)

            with tc.tile_pool(name="f1_ps", bufs=5, space="PSUM") as f1_ps, \
                 tc.tile_pool(name="f2_ps", bufs=3, space="PSUM") as f2_ps, \
                 tc.tile_pool(name="out_sb", bufs=3) as out_sb:
                for ci, (lo, hi) in enumerate(_FC_B):
                    w = hi - lo
                    for m in range(HC):
                        pt = f1_ps.tile([128, w], f32, tag="f1")
                        for jp in range(JC // 2):
                            nc.tensor.matmul(
                                pt[:], w1s[m][:, 2 * jp:2 * jp + 2, :],
                                xn[:, 2 * jp:2 * jp + 2, lo:hi],
                                start=(jp == 0), stop=(jp == JC // 2 - 1),
                                perf_mode=mybir.MatmulPerfMode.DoubleRow)
                        # per-out-channel fp8 descale via the activation
                        # scale AP; bias applies after the scale
                        nc.scalar.activation(h[:, m, lo:hi], pt[:], AF.Gelu,
                                             bias=b1t[:, m:m + 1],
                                             scale=s1t[:, m:m + 1])
                for ci, (lo, hi) in enumerate(_FC_B):
                    for m in range(JC):
                        last = (ci == len(_FC_B) - 1 and m == JC - 1)
                        parts = ([(lo, (lo + hi) // 2), ((lo + hi) // 2, hi)]
                                 if last else [(lo, hi)])
                        for plo, phi in parts:
                            w = phi - plo
                            pt = f2_ps.tile([128, 512], f32, tag="f2")
                            for j in range(HC):
                                nc.tensor.matmul(
                                    pt[:, :w], w2t[:, j, m * 128:(m + 1) * 128],
                                    h[:, j, plo:phi],
                                    start=(j == 0), stop=(j == HC - 1))
                            ot = out_sb.tile([128, 512], f32, tag="out")
                            # ot = (psum + fc2_b) + x   in one DVE pass
                            nc.vector.scalar_tensor_tensor(
                                out=ot[:, :w], in0=pt[:, :w],
                                scalar=b2t[:, m:m + 1],
                                in1=xtiles[:, m, plo:phi],
                                op0=ALU.add, op1=ALU.add)
                            nc.sync.dma_start(yT[:, m, plo:phi], ot[:, :w])
    nc.compile()
    return nc


# ---------------- host glue ----------------

_CACHE = {}


def _get(name, builder):
    if name not in _CACHE:
        _CACHE[name] = builder()
    return _CACHE[name]


def _featmajor(a):
    """(T, 768) fp32 -> [128, 6, T]"""
    Tn = a.shape[0]
    return np.ascontiguousarray(a.T.reshape(JC, 128, Tn).transpose(1, 0, 2))


def _unfeat(aT):
    """[128, 6, T] -> (T, 768)"""
    return np.asarray(aT).transpose(1, 0, 2).reshape(DIM, -1).T


def _chunkvec(v):
    """(c*128,) -> [128, c] fp32"""
    v = np.asarray(v, np.float32)
    return np.ascontiguousarray(v.reshape(-1, 128).T)


def _wchunk(w, nchunk, dtype=BF16):
    """(768, nchunk*128) weight -> [128, nchunk, 768] (m-major chunks:
    out[p, m, j*128 + c] = w[j*128 + p, m*128 + c])."""
    w = np.asarray(w, np.float32)
    kin = w.shape[0] // 128
    out = w.reshape(kin, 128, nchunk, 128).transpose(1, 2, 0, 3)
    return np.ascontiguousarray(out.reshape(128, nchunk, kin * 128)).astype(dtype)


def _bf16(a):
    return np.asarray(a, dtype=BF16)


def _build_rel(rel_pos, ws=WS):
    """[64, 14, 14] fp8: out[c, h, k] = rel_pos[idx[h,k], c] * W8 / sqrt(SCALE)
    (qhat holds sqrt(SCALE)*q; W8 prescale is undone at the rel eviction)."""
    idx = np.arange(ws)[:, None] - np.arange(ws)[None, :] + (ws - 1)
    R = np.asarray(rel_pos, np.float32)[idx] * (W8 / SCALE ** 0.5)
    return R.transpose(2, 0, 1).astype(F8)


def _build_onehots():
    """Eh[r, :, k] = 1 if k//14 == r;  Ew[r, :, k] = 1 if k%14 == r,
    pre-expanded over the (j, win) axis for big contiguous DMA runs."""
    k = np.arange(N)
    Eh = (k[None, :] // WS == np.arange(WS)[:, None]).astype(np.float32)
    Ew = (k[None, :] % WS == np.arange(WS)[:, None]).astype(np.float32)
    Eh = np.ascontiguousarray(np.broadcast_to(Eh[:, None, :], (WS, JC * NW, N)))
    Ew = np.ascontiguousarray(np.broadcast_to(Ew[:, None, :], (WS, JC * NW, N)))
    return Eh.astype(F8), Ew.astype(F8)


kernel_last_perf = {}

try:
    from antenv.axon_hooks import get_axon_ntff_profile_hook as _hook  # noqa: F401
    _HAVE_TRACE = True
except ImportError:
    _HAVE_TRACE = False
    import os as _os
    _os.environ["BASS_NEVER_TRACE"] = "1"   # bass_utils re-reads BASS_TRACE


def window_x(x):
    """(2, 64, 64, 768) -> (56, 196, 768) padded window tokens."""
    B, H, W, C = x.shape
    xp = np.zeros((B, 70, 70, C), np.float32)
    xp[:, :64, :64] = x
    xw = xp.reshape(B, 5, WS, 5, WS, C).transpose(0, 1, 3, 2, 4, 5).reshape(50, N, C)
    xall = np.zeros((56, N, C), np.float32)
    xall[:50] = xw
    return xall


def attn_consts(norm1_w, norm1_b, qkv_w, qkv_b, proj_w, proj_b,
                rel_pos_h, rel_pos_w):
    """Host-side constant tensors for dispatch A (LN1 affine absorbed)."""
    n1w = np.asarray(norm1_w, np.float32)
    n1b = np.asarray(norm1_b, np.float32)
    qkvw = np.asarray(qkv_w, np.float32)
    qkvb = np.asarray(qkv_b, np.float32)
    Wq = n1w[:, None] * qkvw                 # (768, 2304)
    bfull = n1b @ qkvw + qkvb                # (2304,)
    Eh, Ew = _build_onehots()
    return {
        "qkW": _wchunk(Wq[:, 0:2 * DIM] * W8, 12, F8),
        "wvT": np.ascontiguousarray(
            Wq[:, 2 * DIM:].reshape(JC, 128, DIM).transpose(1, 0, 2)
            * W8).astype(F8),
        "wpT": np.ascontiguousarray(
            np.asarray(proj_w, np.float32).reshape(JC, 128, JC, 128)
            .transpose(1, 0, 2, 3) * W8).astype(F8),
        "bqT": _chunkvec(bfull[0:DIM] * SCALE ** 0.5),
        "bkT": _chunkvec(bfull[DIM:2 * DIM] * SCALE ** 0.5),
        "bvT": _bf16(bfull[2 * DIM:].reshape(1, 2, 384) * W8),
        "pbT": _chunkvec(proj_b),
        "RhT": _build_rel(rel_pos_h),
        "RwT": _build_rel(rel_pos_w),
        "EhT": Eh,
        "EwT": Ew,
    }


def mlp_consts(norm2_w, norm2_b, fc1_w, fc1_b, fc2_w, fc2_b):
    """Host-side constant tensors for dispatch B (LN2 affine absorbed)."""
    n2w = np.asarray(norm2_w, np.float32)
    n2b = np.asarray(norm2_b, np.float32)
    f1w = np.asarray(fc1_w, np.float32)
    W1 = n2w[:, None] * f1w                  # (768, 3072)
    b1 = n2b @ f1w + np.asarray(fc1_b, np.float32)
    # per-out-channel power-of-2 fp8 scaling for fc1 (exactly undone by the
    # gelu activation's per-partition scale AP)
    colmax = np.abs(W1).max(axis=0)                        # (3072,)
    sexp = np.clip(np.floor(np.log2(224.0 / np.maximum(colmax, 1e-30))),
                   -20, 20)
    wscale = np.exp2(sexp)                                 # (3072,)
    return {
        "fc1W": _wchunk(W1 * wscale[None, :], HC, F8),
        "fc1S": _chunkvec(1.0 / wscale),
        "fc2W": _bf16(np.ascontiguousarray(
            np.asarray(fc2_w, np.float32).reshape(HC, 128, DIM)
            .transpose(1, 0, 2))),
        "fc1B": _chunkvec(b1),
        "fc2B": _chunkvec(fc2_b),
    }


def kernel(x, norm1_w, norm1_b, qkv_w, qkv_b, proj_w, proj_b,
           rel_pos_h, rel_pos_w, norm2_w, norm2_b,
           fc1_w, fc1_b, fc2_w, fc2_b):
    import os
    trace = bool(os.environ.get("BASS_TRACE")) and _HAVE_TRACE
    x = np.asarray(x, np.float32)
    B, H, W, C = x.shape
    assert (B, H, W, C) == (2, 64, 64, DIM)

    # ---- dispatch A: windowed attention ----
    xall = window_x(x)
    consts_a = attn_consts(norm1_w, norm1_b, qkv_w, qkv_b, proj_w, proj_b,
                           rel_pos_h, rel_pos_w)
    with_vbias = bool(np.any(np.asarray(consts_a["bvT"], np.float32)))
    nc_a = _get(f"attn{int(with_vbias)}",
                lambda: build_attn(with_vbias=with_vbias))
    in_maps = []
    for c in range(NCORES):
        m = dict(consts_a)
        m["xT"] = _featmajor(
            xall[c * NW:(c + 1) * NW].reshape(T, C)).astype(BF16)
        in_maps.append(m)
    res_a = run_bass_kernel_spmd(nc_a, in_maps, core_ids=list(range(NCORES)),
                                 trace=trace)
    kernel_last_perf["attn"] = res_a.exec_time_ns
    xo_all = np.stack([_unfeat(res_a.results[c]["xoT"]) for c in range(NCORES)])
    xo = xo_all.reshape(56, N, C)[:50]
    xo = xo.reshape(B, 5, 5, WS, WS, C).transpose(0, 1, 3, 2, 4, 5).reshape(B, 70, 70, C)
    x2 = x + xo[:, :64, :64]

    # ---- dispatch B: MLP ----
    nc_b = _get("mlp", build_mlp)
    consts_b = mlp_consts(norm2_w, norm2_b, fc1_w, fc1_b, fc2_w, fc2_b)
    x2f = np.ascontiguousarray(x2.reshape(B * H * W, C))
    in_maps = []
    for c in range(NCORES):
        m = dict(consts_b)
        m["xT"] = _featmajor(x2f[c * TB:(c + 1) * TB]).astype(BF16)
        in_maps.append(m)
    res_b = run_bass_kernel_spmd(nc_b, in_maps, core_ids=list(range(NCORES)),
                                 trace=trace)
    kernel_last_perf["mlp"] = res_b.exec_time_ns
    y = np.concatenate([_unfeat(res_b.results[c]["yT"]) for c in range(NCORES)])
    return y.reshape(B, H, W, C).astype(np.float32)
